# revision 13
# baseline (speedup 1.0000x reference)
"""GCN layer (gather -> weighted scatter-sum -> dense transform) on 8 trn2 cores.

Strategy (1-D row partitioning of destination nodes):
  - Core c owns destination nodes [c*NPW, (c+1)*NPW). edge_dst is sorted, so
    each core's edges are a contiguous slice of the edge list.
  - Within a core, dst nodes are processed in windows of 128 (the PSUM
    partition size). Every window's edges are padded to a fixed number of
    128-edge groups so all 8 cores run the same program.
  - Per 128-edge group:
      * dma_gather pulls the 128 source rows H[src] (fp16) from HBM into an
        SBUF tile G [128 edges x 128 feat] (edge e=j*128+p lands on
        partition p, slot j).
      * DVE builds S [128 edges x 128 nodes] = (iota == dstrel) * w with one
        fused tensor_scalar op.
      * TensorE accumulates aggT[feat, node] += G.T @ S in PSUM.
  - dma_gather indices are int16 (< 32768), so edges are split into a "lo"
    stream (src < 32768, gathered from H[:32768]) and a "hi" stream
    (src >= 32768, gathered from H[32768:]); both accumulate into the same
    PSUM window.
  - Final transform: out.T = W.T @ aggT (+ b) with W stationary, computed in
    512-column chunks; bias is added during the PSUM->SBUF copy (per-partition
    ACT bias, since the output is transposed: partitions = out features).
    The kernel writes out.T [128, NWIN*128] per core; the host transposes and
    concatenates.
"""

import os
import numpy as np

N_CORES = 8
N_NODES = 50000
D = 128
NPW = N_NODES // N_CORES  # 6250 dst nodes per core
WIN = 128
SPLIT = 32768  # int16-addressable row limit for dma_gather

# gather dtype: "f16" (half gather traffic, rel err ~3e-4) or "f32" (exact)
GDTYPE = os.environ.get("GCN_GDTYPE", "f16")

LAST_EXEC_NS = None  # set when GCN_TRACE=1
LAST_RESULTS = None


def _ceil_div(a, b):
    return -(-a // b)


def _prep(H, edge_src, edge_dst, edge_weight, n_cores=N_CORES):
    """Host-side sharding: per-core, per-window, per-stream edge lists with
    padding to common sizes. Returns per-core arrays + common geometry."""
    nwin = _ceil_div(NPW, WIN)
    # per (core, window, stream) edge index lists
    per_core = []
    max_lo = 0
    max_hi = 0
    max_all = 0
    for c in range(n_cores):
        n0, n1 = c * NPW, (c + 1) * NPW
        e0, e1 = np.searchsorted(edge_dst, [n0, n1])
        d = edge_dst[e0:e1] - n0
        s = edge_src[e0:e1]
        w = edge_weight[e0:e1]
        wins = []
        for wi in range(nwin):
            i0, i1 = np.searchsorted(d, [wi * WIN, wi * WIN + WIN])
            sw, dw, ww = s[i0:i1], d[i0:i1] - wi * WIN, w[i0:i1]
            lo = sw < SPLIT
            wins.append(
                (
                    (sw[lo], dw[lo], ww[lo]),
                    (sw[~lo] - SPLIT, dw[~lo], ww[~lo]),
                    (sw, dw, ww),
                )
            )
            max_lo = max(max_lo, int(lo.sum()))
            max_hi = max(max_hi, int((~lo).sum()))
            max_all = max(max_all, int(i1 - i0))
        per_core.append(wins)
    g_lo = max(1, _ceil_div(max_lo, 128))
    g_hi = max(1, _ceil_div(max_hi, 128))
    g_all = max(1, _ceil_div(max_all, 128))
    return per_core, nwin, g_lo, g_hi, g_all


def _chunks(g, maxg):
    """Split g groups into near-even chunks of <= maxg groups."""
    n = _ceil_div(g, maxg)
    base, rem = divmod(g, n)
    out = []
    c0 = 0
    for i in range(n):
        k = base + (1 if i < rem else 0)
        out.append((c0, k))
        c0 += k
    return out


def _device_arrays(wins, nwin, g, stream, chunks, np_meta_dtype, regs=None):
    """Build idx (wrapped-16 per gather call; call = (window, chunk)) +
    dstrel/weight arrays for one core and one stream ('lo'=0, 'hi'=1).

    regs is unused (kept for signature compat)."""
    ne = g * 128
    idx = np.zeros((nwin, ne), np.int16)
    drel = np.zeros((nwin, ne), np_meta_dtype)
    wgt = np.zeros((nwin, ne), np_meta_dtype)
    for wi in range(nwin):
        sw, dw, ww = wins[wi][stream]
        n = len(sw)
        idx[wi, :n] = sw.astype(np.int16)
        drel[wi, :n] = dw.astype(np_meta_dtype)
        wgt[wi, :n] = ww.astype(np_meta_dtype)
    parts = []
    for wi in range(nwin):
        for (c0, k) in chunks:
            flat = idx[wi, c0 * 128 : (c0 + k) * 128]
            parts.append(flat.reshape(-1, 16).T)  # [16, k*8]
    idx_dev = np.tile(np.concatenate(parts, axis=1), (8, 1))  # [128, nwin*g*8]
    # meta: [p, w*g + j] = value of edge e=j*128+p in window w
    drel_dev = np.ascontiguousarray(
        drel.reshape(nwin, g, 128).transpose(2, 0, 1).reshape(128, nwin * g)
    )
    wgt_dev = np.ascontiguousarray(
        wgt.reshape(nwin, g, 128).transpose(2, 0, 1).reshape(128, nwin * g)
    )
    return idx_dev, drel_dev, wgt_dev


def _device_arrays_ind(wins, nwin, g, np_meta_dtype):
    """idx (int32, natural [p, w*g+j] layout) + dstrel/weight arrays for the
    combined stream (indirect_dma_start variant)."""
    ne = g * 128
    idx = np.zeros((nwin, ne), np.int32)
    drel = np.zeros((nwin, ne), np_meta_dtype)
    wgt = np.zeros((nwin, ne), np_meta_dtype)
    for wi in range(nwin):
        sw, dw, ww = wins[wi][2]
        n = len(sw)
        idx[wi, :n] = sw
        drel[wi, :n] = dw.astype(np_meta_dtype)
        wgt[wi, :n] = ww.astype(np_meta_dtype)

    def dev(a):
        return np.ascontiguousarray(
            a.reshape(nwin, g, 128).transpose(2, 0, 1).reshape(128, nwin * g)
        )

    return dev(idx), dev(drel), dev(wgt)


def _build_program_ind(nwin, g_all, ch, n_src_rows, n_cores=N_CORES):
    """Indirect_dma_start variant: int32 indices, no lo/hi split."""
    from contextlib import ExitStack

    import concourse.bass as bass
    import concourse.tile as tile
    from concourse import bacc, mybir

    f32 = mybir.dt.float32
    gdt = mybir.dt.float16 if GDTYPE == "f16" else mybir.dt.float32
    i32 = mybir.dt.int32

    nc = bacc.Bacc(
        "TRN2", target_bir_lowering=False, debug=False, num_devices=n_cores,
    )

    npad = nwin * WIN
    h_t = nc.dram_tensor("h_src", [n_src_rows, D], gdt, kind="ExternalInput")
    idx_t = nc.dram_tensor("idx_all", [128, nwin * g_all], i32, kind="ExternalInput")
    drel_t = nc.dram_tensor("drel", [128, nwin * g_all], gdt, kind="ExternalInput")
    wgt_t = nc.dram_tensor("wgt", [128, nwin * g_all], gdt, kind="ExternalInput")
    iota_t = nc.dram_tensor("iota", [128, 128], gdt, kind="ExternalInput")
    w_t = nc.dram_tensor("wmat", [D, D], gdt, kind="ExternalInput")
    b_t = nc.dram_tensor("bcol", [D, 1], f32, kind="ExternalInput")
    out_t = nc.dram_tensor("outT", [D, npad], f32, kind="ExternalOutput")

    with tile.TileContext(nc) as tc:
        with ExitStack() as ctx:
            const = ctx.enter_context(tc.tile_pool(name="const", bufs=1))
            gpool = ctx.enter_context(tc.tile_pool(name="gather", bufs=6))
            spool = ctx.enter_context(tc.tile_pool(name="sel", bufs=3))
            opool = ctx.enter_context(tc.tile_pool(name="outsb", bufs=2))
            ps_agg = ctx.enter_context(tc.tile_pool(name="ps_agg", bufs=2, space="PSUM"))
            ps_out = ctx.enter_context(tc.tile_pool(name="ps_out", bufs=2, space="PSUM"))

            idx = const.tile(list(idx_t.shape), i32)
            drel = const.tile(list(drel_t.shape), gdt)
            wgt = const.tile(list(wgt_t.shape), gdt)
            iota = const.tile([128, 128], gdt)
            wmat = const.tile([D, D], gdt)
            bcol = const.tile([D, 1], f32)
            agg_all = const.tile([128, npad], gdt, tag="agg_all")

            for sb, dr in ((idx, idx_t), (drel, drel_t), (wgt, wgt_t),
                           (iota, iota_t), (wmat, w_t), (bcol, b_t)):
                nc.sync.dma_start(sb[:], dr[:])

            for wg in range(nwin):
                gtiles = []
                for (c0, k) in ch:
                    gt = gpool.tile([128, ch[0][1], 128], gdt, tag="g")
                    nc.gpsimd.indirect_dma_start(
                        out=gt[:, :k, :],
                        out_offset=None,
                        in_=h_t[:],
                        in_offset=bass.IndirectOffsetOnAxis(
                            ap=idx[:, wg * g_all + c0 : wg * g_all + c0 + k],
                            axis=0,
                        ),
                    )
                    gtiles.append((gt, c0, k))

                sh = (128, g_all, 128)
                c0m = wg * g_all
                s = spool.tile([128, g_all, 128], gdt, tag="sel")
                nc.vector.tensor_tensor(
                    s[:], iota[:, None, :].broadcast_to(sh),
                    drel[:, c0m : c0m + g_all, None].broadcast_to(sh),
                    mybir.AluOpType.is_equal,
                )
                nc.vector.tensor_tensor(
                    s[:], s[:], wgt[:, c0m : c0m + g_all, None].broadcast_to(sh),
                    mybir.AluOpType.mult,
                )

                psum = ps_agg.tile([128, 128], f32, tag="psagg")
                k_idx = 0
                for (gt, c0, k) in gtiles:
                    for j in range(k):
                        nc.tensor.matmul(
                            psum[:], gt[:, j, :], s[:, c0 + j, :],
                            start=(k_idx == 0), stop=(k_idx == g_all - 1),
                        )
                        k_idx += 1
                nc.scalar.copy(agg_all[:, wg * WIN : (wg + 1) * WIN], psum[:])

            CH = 512
            for t0 in range(0, npad, CH):
                n = min(CH, npad - t0)
                po = ps_out.tile([128, CH], f32, tag="psout")
                nc.tensor.matmul(
                    po[:, :n], wmat[:], agg_all[:, t0 : t0 + n],
                    start=True, stop=True,
                )
                ob = opool.tile([128, CH], f32, tag="outsb")
                nc.scalar.add(ob[:, :n], po[:, :n], bcol[:])
                nc.sync.dma_start(out_t[:, t0 : t0 + n], ob[:, :n])

    nc.compile()
    return nc


def _build_program_batched(nwin, g_lo, g_hi, gw_lo, gw_hi, n_src_rows,
                           batch=2, max_ke=63, n_cores=N_CORES,
                           scratch=32768, hi_indirect=False):
    """Batched-call variant: one dma_gather call per (batch of windows,
    stream), idx columns packed dense (only the first gw[w] groups of each
    window are gathered). single_packet=False so calls may exceed 65 ring
    descriptors; scratch sized so ring (scratch/64 descs) >= 8*max_ke+1."""
    from contextlib import ExitStack

    import concourse.bass as bass
    import concourse.tile as tile
    from concourse import bacc, mybir

    f32 = mybir.dt.float32
    gdt = mybir.dt.float16 if GDTYPE == "f16" else mybir.dt.float32
    i16 = mybir.dt.int16

    nc = bacc.Bacc(
        "TRN2", target_bir_lowering=False, debug=False, num_devices=n_cores,
        dynamic_dma_scratch_size=scratch,
    )

    npad = nwin * WIN
    n_lo_rows = min(SPLIT, n_src_rows)
    n_hi_rows = n_src_rows - n_lo_rows

    batches = [list(range(b, min(b + batch, nwin))) for b in range(0, nwin, batch)]
    # per-batch dense group counts and per-window offsets
    lo_off = {}
    hi_off = {}
    lo_tot = []
    hi_tot = []
    for bi, ws in enumerate(batches):
        o = 0
        for w in ws:
            lo_off[w] = o
            o += gw_lo[w]
        lo_tot.append(o)
        o = 0
        for w in ws:
            hi_off[w] = o
            o += gw_hi[w]
        hi_tot.append(o)
    glo_max = max(lo_tot)
    ghi_max = max(hi_tot)
    ncol_lo = sum(lo_tot)
    ncol_hi = sum(hi_tot)

    h_t = nc.dram_tensor("h_src", [n_src_rows, D], gdt, kind="ExternalInput")
    idx_lo_t = nc.dram_tensor("idx_lo", [128, ncol_lo * 8], i16, kind="ExternalInput")
    if hi_indirect:
        idx_hi_t = nc.dram_tensor(
            "idx_hi", [128, ncol_hi], mybir.dt.int32, kind="ExternalInput")
    else:
        idx_hi_t = nc.dram_tensor(
            "idx_hi", [128, ncol_hi * 8], i16, kind="ExternalInput")
    drel_lo_t = nc.dram_tensor("drel_lo", [128, nwin * g_lo], gdt, kind="ExternalInput")
    wgt_lo_t = nc.dram_tensor("wgt_lo", [128, nwin * g_lo], gdt, kind="ExternalInput")
    drel_hi_t = nc.dram_tensor("drel_hi", [128, nwin * g_hi], gdt, kind="ExternalInput")
    wgt_hi_t = nc.dram_tensor("wgt_hi", [128, nwin * g_hi], gdt, kind="ExternalInput")
    iota_t = nc.dram_tensor("iota", [128, 128], gdt, kind="ExternalInput")
    w_t = nc.dram_tensor("wmat", [D, D], gdt, kind="ExternalInput")
    b_t = nc.dram_tensor("bcol", [D, 1], f32, kind="ExternalInput")
    out_t = nc.dram_tensor("outT", [D, npad], f32, kind="ExternalOutput")

    with tile.TileContext(nc) as tc:
        with ExitStack() as ctx:
            const = ctx.enter_context(tc.tile_pool(name="const", bufs=1))
            gpool = ctx.enter_context(tc.tile_pool(name="gather", bufs=2))
            spool = ctx.enter_context(tc.tile_pool(name="sel", bufs=3))
            opool = ctx.enter_context(tc.tile_pool(name="outsb", bufs=2))
            ps_agg = ctx.enter_context(tc.tile_pool(name="ps_agg", bufs=2, space="PSUM"))
            ps_out = ctx.enter_context(tc.tile_pool(name="ps_out", bufs=2, space="PSUM"))

            idx_lo = const.tile(list(idx_lo_t.shape), i16)
            idx_hi = const.tile(
                list(idx_hi_t.shape),
                mybir.dt.int32 if hi_indirect else i16)
            drel_lo = const.tile(list(drel_lo_t.shape), gdt)
            wgt_lo = const.tile(list(wgt_lo_t.shape), gdt)
            drel_hi = const.tile(list(drel_hi_t.shape), gdt)
            wgt_hi = const.tile(list(wgt_hi_t.shape), gdt)
            iota = const.tile([128, 128], gdt)
            wmat = const.tile([D, D], gdt)
            bcol = const.tile([D, 1], f32)
            agg_all = const.tile([128, npad], gdt, tag="agg_all")

            for sb, dr in (
                (idx_lo, idx_lo_t), (idx_hi, idx_hi_t),
                (drel_lo, drel_lo_t), (wgt_lo, wgt_lo_t),
                (drel_hi, drel_hi_t), (wgt_hi, wgt_hi_t),
                (iota, iota_t), (wmat, w_t), (bcol, b_t),
            ):
                nc.sync.dma_start(sb[:], dr[:])

            h_lo = h_t[0:n_lo_rows, :]
            h_hi = h_t[n_lo_rows:n_src_rows, :] if n_hi_rows > 0 else None

            col_lo = 0
            col_hi = 0
            for bi, ws in enumerate(batches):
                # one gather call per stream per batch (split at max_ke)
                bsp = os.environ.get("GCN_BSP", "0") == "1"
                glo = gpool.tile([128, glo_max, 128], gdt, tag="glo")
                c0 = 0
                while c0 < lo_tot[bi]:
                    ke = min(max_ke, lo_tot[bi] - c0)
                    nc.gpsimd.dma_gather(
                        glo[:, c0 : c0 + ke, :], h_lo,
                        idx_lo[:, (col_lo + c0) * 8 : (col_lo + c0 + ke) * 8],
                        num_idxs=ke * 128, num_idxs_reg=ke * 128, elem_size=D,
                        single_packet=bsp,
                    )
                    c0 += ke
                ghi = gpool.tile([128, ghi_max, 128], gdt, tag="ghi")
                marker = None
                if hi_indirect:
                    nc.gpsimd.indirect_dma_start(
                        out=ghi[:, : hi_tot[bi], :],
                        out_offset=None,
                        in_=h_t[:],
                        in_offset=bass.IndirectOffsetOnAxis(
                            ap=idx_hi[:, col_hi : col_hi + hi_tot[bi]],
                            axis=0,
                        ),
                    )
                    # FIFO-ordering completion marker: a tiny SWDGE gather on
                    # the same queue whose (working) DMA semaphore fires only
                    # after the ring has drained past the indirect's
                    # descriptors. Every hi matmul is made to depend on it via
                    # a bypass op over s_hi.
                    marker = gpool.tile([128, 1, 128], gdt, tag="mrk")
                    nc.gpsimd.dma_gather(
                        marker[:, :1, :], h_lo, idx_lo[:, 0:8],
                        num_idxs=128, num_idxs_reg=128, elem_size=D,
                        single_packet=False,
                    )
                else:
                    c0 = 0
                    while c0 < hi_tot[bi]:
                        ke = min(max_ke, hi_tot[bi] - c0)
                        nc.gpsimd.dma_gather(
                            ghi[:, c0 : c0 + ke, :], h_hi,
                            idx_hi[:, (col_hi + c0) * 8 : (col_hi + c0 + ke) * 8],
                            num_idxs=ke * 128, num_idxs_reg=ke * 128, elem_size=D,
                            single_packet=bsp,
                        )
                        c0 += ke
                col_lo += lo_tot[bi]
                col_hi += hi_tot[bi]

                for wg in ws:
                    def build_s(meta_d, meta_w, g, tag):
                        s = spool.tile([128, g, 128], gdt, tag=tag)
                        sh = (128, g, 128)
                        c0m = wg * g
                        nc.vector.tensor_tensor(
                            s[:], iota[:, None, :].broadcast_to(sh),
                            meta_d[:, c0m : c0m + g, None].broadcast_to(sh),
                            mybir.AluOpType.is_equal,
                        )
                        nc.vector.tensor_tensor(
                            s[:], s[:],
                            meta_w[:, c0m : c0m + g, None].broadcast_to(sh),
                            mybir.AluOpType.mult,
                        )
                        return s

                    s_lo = build_s(drel_lo, wgt_lo, g_lo, "slo")
                    s_hi = build_s(drel_hi, wgt_hi, g_hi, "shi") if h_hi is not None else None
                    if s_hi is not None and marker is not None:
                        # bypass: out = in0 (s_hi unchanged) but creates a dep
                        # on the marker tile for every s_hi subtile.
                        nc.vector.tensor_tensor(
                            s_hi[:], s_hi[:],
                            marker[:, 0, None, 0:1].broadcast_to((128, g_hi, 128)),
                            mybir.AluOpType.bypass,
                        )

                    psum = ps_agg.tile([128, 128], f32, tag="psagg")
                    n_groups = gw_lo[wg] + (gw_hi[wg] if s_hi is not None else 0)
                    k_idx = 0
                    for j in range(gw_lo[wg]):
                        nc.tensor.matmul(
                            psum[:], glo[:, lo_off[wg] + j, :], s_lo[:, j, :],
                            start=(k_idx == 0), stop=(k_idx == n_groups - 1),
                        )
                        k_idx += 1
                    if s_hi is not None:
                        for j in range(gw_hi[wg]):
                            nc.tensor.matmul(
                                psum[:], ghi[:, hi_off[wg] + j, :], s_hi[:, j, :],
                                start=(k_idx == 0), stop=(k_idx == n_groups - 1),
                            )
                            k_idx += 1
                    nc.scalar.copy(agg_all[:, wg * WIN : (wg + 1) * WIN], psum[:])

            CH = 512
            for t0 in range(0, npad, CH):
                n = min(CH, npad - t0)
                po = ps_out.tile([128, CH], f32, tag="psout")
                nc.tensor.matmul(
                    po[:, :n], wmat[:], agg_all[:, t0 : t0 + n],
                    start=True, stop=True,
                )
                ob = opool.tile([128, CH], f32, tag="outsb")
                nc.scalar.add(ob[:, :n], po[:, :n], bcol[:])
                nc.sync.dma_start(out_t[:, t0 : t0 + n], ob[:, :n])

    nc.compile()
    return nc, batches, lo_tot, hi_tot


def _device_arrays_batched(wins, nwin, g, stream, gw, batches, np_meta_dtype,
                           ind_offset=None):
    """Dense-packed idx array (wrapped-16 per batch-call column blocks) plus
    per-window padded drel/wgt arrays (same layout as _device_arrays).

    ind_offset: if not None, build int32 indirect-layout idxs ([128, ncol]
    natural [p, col] order, absolute row ids = stored + ind_offset)."""
    ne = g * 128
    idx = np.zeros((nwin, ne), np.int32 if ind_offset is not None else np.int16)
    drel = np.zeros((nwin, ne), np_meta_dtype)
    wgt = np.zeros((nwin, ne), np_meta_dtype)
    for wi in range(nwin):
        sw, dw, ww = wins[wi][stream]
        n = len(sw)
        if ind_offset is not None:
            idx[wi, :n] = sw.astype(np.int32) + ind_offset
        else:
            idx[wi, :n] = sw.astype(np.int16)
        drel[wi, :n] = dw.astype(np_meta_dtype)
        wgt[wi, :n] = ww.astype(np_meta_dtype)
    if ind_offset is not None:
        # [p, dense col] where col runs over (batch, window, group j<gw[w])
        parts = []
        for ws in batches:
            for wi in ws:
                parts.append(idx[wi, : gw[wi] * 128].reshape(gw[wi], 128).T)
        idx_dev = np.ascontiguousarray(np.concatenate(parts, axis=1))
    else:
        # wrapped in 16 partitions, tiled x8 (SWDGE layout)
        parts = []
        for ws in batches:
            for wi in ws:
                flat = idx[wi, : gw[wi] * 128]
                parts.append(flat.reshape(-1, 16).T)  # [16, gw*8]
        idx_dev = np.tile(np.concatenate(parts, axis=1), (8, 1))
    drel_dev = np.ascontiguousarray(
        drel.reshape(nwin, g, 128).transpose(2, 0, 1).reshape(128, nwin * g)
    )
    wgt_dev = np.ascontiguousarray(
        wgt.reshape(nwin, g, 128).transpose(2, 0, 1).reshape(128, nwin * g)
    )
    return idx_dev, drel_dev, wgt_dev


def _build_program(nwin, g_lo, g_hi, ch_lo, ch_hi, n_src_rows, n_cores=N_CORES,
                   gw_lo=None, gw_hi=None):
    """Trace the (single, SPMD-shared) Bass program."""
    from contextlib import ExitStack

    import concourse.bass as bass
    import concourse.tile as tile
    from concourse import bacc, mybir

    f32 = mybir.dt.float32
    gdt = mybir.dt.float16 if GDTYPE == "f16" else mybir.dt.float32
    i16 = mybir.dt.int16

    nc = bacc.Bacc(
        "TRN2",
        target_bir_lowering=False,
        debug=False,
        num_devices=n_cores,
    )

    npad = nwin * WIN
    n_lo_rows = min(SPLIT, n_src_rows)
    n_hi_rows = n_src_rows - n_lo_rows

    h_t = nc.dram_tensor("h_src", [n_src_rows, D], gdt, kind="ExternalInput")
    idx_lo_t = nc.dram_tensor(
        "idx_lo", [128, nwin * g_lo * 8], i16, kind="ExternalInput",
    )
    idx_hi_t = nc.dram_tensor(
        "idx_hi", [128, nwin * g_hi * 8], i16, kind="ExternalInput",
    )
    drel_lo_t = nc.dram_tensor("drel_lo", [128, nwin * g_lo], gdt, kind="ExternalInput")
    wgt_lo_t = nc.dram_tensor("wgt_lo", [128, nwin * g_lo], gdt, kind="ExternalInput")
    drel_hi_t = nc.dram_tensor("drel_hi", [128, nwin * g_hi], gdt, kind="ExternalInput")
    wgt_hi_t = nc.dram_tensor("wgt_hi", [128, nwin * g_hi], gdt, kind="ExternalInput")
    iota_t = nc.dram_tensor("iota", [128, 128], gdt, kind="ExternalInput")
    w_t = nc.dram_tensor("wmat", [D, D], gdt, kind="ExternalInput")
    b_t = nc.dram_tensor("bcol", [D, 1], f32, kind="ExternalInput")
    out_t = nc.dram_tensor("outT", [D, npad], f32, kind="ExternalOutput")

    with tile.TileContext(nc) as tc:
        with ExitStack() as ctx:
            const = ctx.enter_context(tc.tile_pool(name="const", bufs=1))
            gpool = ctx.enter_context(tc.tile_pool(name="gather", bufs=6))
            spool = ctx.enter_context(tc.tile_pool(name="sel", bufs=3))
            opool = ctx.enter_context(tc.tile_pool(name="outsb", bufs=2))
            ps_agg = ctx.enter_context(
                tc.tile_pool(name="ps_agg", bufs=2, space="PSUM")
            )
            ps_out = ctx.enter_context(
                tc.tile_pool(name="ps_out", bufs=2, space="PSUM")
            )

            # resident constants / metadata
            idx_lo = const.tile(list(idx_lo_t.shape), i16)
            idx_hi = const.tile(list(idx_hi_t.shape), i16)
            drel_lo = const.tile(list(drel_lo_t.shape), gdt)
            wgt_lo = const.tile(list(wgt_lo_t.shape), gdt)
            drel_hi = const.tile(list(drel_hi_t.shape), gdt)
            wgt_hi = const.tile(list(wgt_hi_t.shape), gdt)
            iota = const.tile([128, 128], gdt)
            wmat = const.tile([D, D], gdt)
            bcol = const.tile([D, 1], f32)
            agg_all = const.tile([128, npad], gdt, tag="agg_all")

            for sb, dr in (
                (idx_lo, idx_lo_t), (idx_hi, idx_hi_t),
                (drel_lo, drel_lo_t), (wgt_lo, wgt_lo_t),
                (drel_hi, drel_hi_t), (wgt_hi, wgt_hi_t),
                (iota, iota_t), (wmat, w_t), (bcol, b_t),
            ):
                nc.sync.dma_start(sb[:], dr[:])

            h_lo = h_t[0:n_lo_rows, :]
            h_hi = h_t[n_lo_rows:n_src_rows, :] if n_hi_rows > 0 else None
            use_hi = h_hi is not None

            for wg in range(nwin):
                # gather this window's edges: one SWDGE call per chunk.
                # A call of k*128 idxs needs 8k+1 SWDGE ring entries; calls
                # with 97 entries (k=12) crash the exec unit on HW, k<=8 is
                # proven safe.
                # effective groups this window (shared across cores): groups
                # beyond the max valid count are pure padding -> not gathered,
                # not matmul'd. Every issued call is fully valid, so no tile
                # region is ever read without having been written.
                gwl = gw_lo[wg] if gw_lo else g_lo
                gwh = gw_hi[wg] if gw_hi else g_hi
                sp = os.environ.get("GCN_SP", "1") == "1"
                gtiles_lo = []
                for (c0, k) in ch_lo:
                    ke = min(max(gwl - c0, 0), k)
                    if ke == 0:
                        continue
                    gt = gpool.tile([128, ch_lo[0][1], 128], gdt, tag="glo")
                    col = (wg * g_lo + c0) * 8
                    nc.gpsimd.dma_gather(
                        gt[:, :ke, :], h_lo, idx_lo[:, col : col + ke * 8],
                        num_idxs=ke * 128, num_idxs_reg=ke * 128, elem_size=D,
                        single_packet=sp,
                    )
                    gtiles_lo.append((gt, c0, ke))
                gtiles_hi = []
                if use_hi:
                    for (c0, k) in ch_hi:
                        ke = min(max(gwh - c0, 0), k)
                        if ke == 0:
                            continue
                        gt = gpool.tile([128, ch_hi[0][1], 128], gdt, tag="ghi")
                        col = (wg * g_hi + c0) * 8
                        nc.gpsimd.dma_gather(
                            gt[:, :ke, :], h_hi, idx_hi[:, col : col + ke * 8],
                            num_idxs=ke * 128, num_idxs_reg=ke * 128, elem_size=D,
                            single_packet=sp,
                        )
                        gtiles_hi.append((gt, c0, ke))

                # S for the whole window in 2 DVE ops per stream:
                # S[p, j, n] = (n == drel[p, j]) * w[p, j] via step-0
                # broadcast APs on both operands.
                def build_s(meta_d, meta_w, g, tag):
                    s = spool.tile([128, g, 128], gdt, tag=tag)
                    sh = (128, g, 128)
                    c0m = wg * g
                    nc.vector.tensor_tensor(
                        s[:], iota[:, None, :].broadcast_to(sh),
                        meta_d[:, c0m : c0m + g, None].broadcast_to(sh),
                        mybir.AluOpType.is_equal,
                    )
                    nc.vector.tensor_tensor(
                        s[:], s[:],
                        meta_w[:, c0m : c0m + g, None].broadcast_to(sh),
                        mybir.AluOpType.mult,
                    )
                    return s

                s_lo = build_s(drel_lo, wgt_lo, g_lo, "slo")
                s_hi = build_s(drel_hi, wgt_hi, g_hi, "shi") if use_hi else None

                psum = ps_agg.tile([128, 128], f32, tag="psagg")
                n_groups = sum(k for _, _, k in gtiles_lo)
                n_groups += sum(k for _, _, k in gtiles_hi)
                k_idx = 0
                for (gt, c0, k), s_all in (
                    [(t, s_lo) for t in gtiles_lo]
                    + [(t, s_hi) for t in gtiles_hi]
                ):
                    for j in range(k):
                        nc.tensor.matmul(
                            psum[:], gt[:, j, :], s_all[:, c0 + j, :],
                            start=(k_idx == 0), stop=(k_idx == n_groups - 1),
                        )
                        k_idx += 1
                # aggT window -> SBUF (cast to gather dtype)
                nc.scalar.copy(agg_all[:, wg * WIN : (wg + 1) * WIN], psum[:])

            # out.T = W.T @ aggT + b, in 512-column chunks
            CH = 512
            for t0 in range(0, npad, CH):
                n = min(CH, npad - t0)
                po = ps_out.tile([128, CH], f32, tag="psout")
                nc.tensor.matmul(
                    po[:, :n], wmat[:], agg_all[:, t0 : t0 + n],
                    start=True, stop=True,
                )
                ob = opool.tile([128, CH], f32, tag="outsb")
                nc.scalar.add(ob[:, :n], po[:, :n], bcol[:])
                nc.sync.dma_start(out_t[:, t0 : t0 + n], ob[:, :n])

    nc.compile()
    return nc


def _make_in_maps(H, edge_src, edge_dst, edge_weight, W, b, per_core, nwin,
                  g_lo, g_hi, ch_lo, ch_hi):
    np_g = np.float16 if GDTYPE == "f16" else np.float32
    h_src = np.ascontiguousarray(H.astype(np_g))
    iota = np.tile(np.arange(128, dtype=np_g), (128, 1))
    wmat = np.ascontiguousarray(W.astype(np_g))
    bcol = np.ascontiguousarray(b.astype(np.float32).reshape(D, 1))
    in_maps = []
    for wins in per_core:
        idx_lo, drel_lo, wgt_lo = _device_arrays(wins, nwin, g_lo, 0, ch_lo, np_g)
        idx_hi, drel_hi, wgt_hi = _device_arrays(wins, nwin, g_hi, 1, ch_hi, np_g)
        in_maps.append(
            {
                "h_src": h_src,
                "idx_lo": idx_lo, "idx_hi": idx_hi,
                "drel_lo": drel_lo, "wgt_lo": wgt_lo,
                "drel_hi": drel_hi, "wgt_hi": wgt_hi,
                "iota": iota, "wmat": wmat, "bcol": bcol,
            }
        )
    return in_maps


def kernel(H, edge_src, edge_dst, edge_weight, W, b):
    global LAST_EXEC_NS
    from concourse import bass_utils

    H = np.asarray(H, dtype=np.float32)
    edge_src = np.asarray(edge_src, dtype=np.int32)
    edge_dst = np.asarray(edge_dst, dtype=np.int32)
    edge_weight = np.asarray(edge_weight, dtype=np.float32)
    W = np.asarray(W, dtype=np.float32)
    b = np.asarray(b, dtype=np.float32)

    per_core, nwin, g_lo, g_hi, g_all = _prep(H, edge_src, edge_dst, edge_weight)
    mode = os.environ.get("GCN_GATHER", "batched")
    if mode == "batched":
        batch = int(os.environ.get("GCN_BATCH", "2"))
        max_ke = int(os.environ.get("GCN_MAXKE", "63"))
        scratch = int(os.environ.get("GCN_SCRATCH", "32768"))
        gw_lo = []
        gw_hi = []
        for wi in range(nwin):
            m_lo = max(len(wins[wi][0][0]) for wins in per_core)
            m_hi = max(len(wins[wi][1][0]) for wins in per_core)
            gw_lo.append(min(g_lo, max(1, _ceil_div(m_lo, 128))))
            gw_hi.append(min(g_hi, max(1, _ceil_div(m_hi, 128))))
        hi_ind = os.environ.get("GCN_HI_IND", "0") == "1"
        nc, batches, lo_tot, hi_tot = _build_program_batched(
            nwin, g_lo, g_hi, gw_lo, gw_hi, N_NODES,
            batch=batch, max_ke=max_ke, scratch=scratch, hi_indirect=hi_ind,
        )
        np_g = np.float16 if GDTYPE == "f16" else np.float32
        h_src = np.ascontiguousarray(H.astype(np_g))
        iota = np.tile(np.arange(128, dtype=np_g), (128, 1))
        wmat = np.ascontiguousarray(W.astype(np_g))
        bcol = np.ascontiguousarray(b.astype(np.float32).reshape(D, 1))
        in_maps = []
        for wins in per_core:
            idx_lo, drel_lo, wgt_lo = _device_arrays_batched(
                wins, nwin, g_lo, 0, gw_lo, batches, np_g)
            idx_hi, drel_hi, wgt_hi = _device_arrays_batched(
                wins, nwin, g_hi, 1, gw_hi, batches, np_g,
                ind_offset=SPLIT if hi_ind else None)
            in_maps.append(
                {
                    "h_src": h_src,
                    "idx_lo": idx_lo, "idx_hi": idx_hi,
                    "drel_lo": drel_lo, "wgt_lo": wgt_lo,
                    "drel_hi": drel_hi, "wgt_hi": wgt_hi,
                    "iota": iota, "wmat": wmat, "bcol": bcol,
                }
            )
    elif mode == "indirect":
        maxg = int(os.environ.get("GCN_MAXG", "8"))
        ch = _chunks(g_all, maxg)
        nc = _build_program_ind(nwin, g_all, ch, N_NODES)
        np_g = np.float16 if GDTYPE == "f16" else np.float32
        h_src = np.ascontiguousarray(H.astype(np_g))
        iota = np.tile(np.arange(128, dtype=np_g), (128, 1))
        wmat = np.ascontiguousarray(W.astype(np_g))
        bcol = np.ascontiguousarray(b.astype(np.float32).reshape(D, 1))
        in_maps = []
        for wins in per_core:
            idx_all, drel, wgt = _device_arrays_ind(wins, nwin, g_all, np_g)
            in_maps.append({
                "h_src": h_src, "idx_all": idx_all, "drel": drel, "wgt": wgt,
                "iota": iota, "wmat": wmat, "bcol": bcol,
            })
    else:
        maxg = int(os.environ.get("GCN_MAXG", "8"))
        # round group counts up so every chunk has equal size: a gather tile
        # slot must always be written over its FULL extent when fully valid,
        # otherwise a smaller earlier write leaves never-written (non-finite)
        # columns that a later partially-valid call exposes to the matmul.
        g_lo = _ceil_div(g_lo, maxg) * maxg if g_lo > maxg else g_lo
        g_hi = _ceil_div(g_hi, maxg) * maxg if g_hi > maxg else g_hi
        ch_lo = _chunks(g_lo, maxg)
        ch_hi = _chunks(g_hi, maxg)
        assert len({k for _, k in ch_lo}) == 1 and len({k for _, k in ch_hi}) == 1
        # per-window effective group counts (shared across cores): only
        # gather/matmul groups that contain at least one real edge on the
        # max-count core; the rest are pure padding.
        trim = os.environ.get("GCN_TRIM", "1") == "1"
        gw_lo = []
        gw_hi = []
        for wi in range(nwin):
            m_lo = max(len(wins[wi][0][0]) for wins in per_core)
            m_hi = max(len(wins[wi][1][0]) for wins in per_core)
            gw_lo.append(min(g_lo, max(1, _ceil_div(m_lo, 128))) if trim else g_lo)
            gw_hi.append(min(g_hi, max(1, _ceil_div(m_hi, 128))) if trim else g_hi)
        nc = _build_program(nwin, g_lo, g_hi, ch_lo, ch_hi, N_NODES,
                            gw_lo=gw_lo, gw_hi=gw_hi)
        in_maps = _make_in_maps(
            H, edge_src, edge_dst, edge_weight, W, b, per_core, nwin, g_lo,
            g_hi, ch_lo, ch_hi,
        )

    if os.environ.get("GCN_SIM", "0") == "1":  # CoreSim path for testing
        from concourse.bass_interp import CoreSim

        out = np.empty((N_NODES, D), np.float32)
        for c in range(N_CORES):
            sim = CoreSim(nc)
            for k2, v2 in in_maps[c].items():
                sim.tensor(k2)[:] = v2
            sim.simulate()
            out[c * NPW : (c + 1) * NPW, :] = np.array(
                sim.tensor("outT")).T[:NPW]
        return out

    trace = os.environ.get("GCN_TRACE", "0") == "1"
    kw = {}
    if trace:
        import shutil
        td = "/tmp/gcn_ntff"
        shutil.rmtree(td, ignore_errors=True)
        os.makedirs(td, exist_ok=True)
        kw["tmpdir"] = td
    # a previously crashed NEFF can leave the exec unit transiently
    # unrecoverable; recovery has been observed to take up to a few minutes,
    # so retry with escalating backoff
    import time as _time
    last_err = None
    for backoff in (15, 45, 90, 0):
        try:
            res = bass_utils.run_bass_kernel_spmd(
                nc, in_maps, core_ids=list(range(N_CORES)), trace=trace, **kw
            )
            break
        except Exception as e:
            last_err = e
            if backoff:
                _time.sleep(backoff)
    else:
        raise last_err
    LAST_EXEC_NS = res.exec_time_ns
    global LAST_RESULTS
    LAST_RESULTS = res

    out = np.empty((N_NODES, D), np.float32)
    for c in range(N_CORES):
        outT = res.results[c]["outT"]
        out[c * NPW : (c + 1) * NPW, :] = outT.T[:NPW]
    return out



# revision 14
# speedup vs baseline: 1.0878x; 1.0878x over previous
"""GCN layer (gather -> weighted scatter-sum -> dense transform) on 8 trn2 cores.

Strategy (1-D row partitioning of destination nodes):
  - Core c owns destination nodes [c*NPW, (c+1)*NPW). edge_dst is sorted, so
    each core's edges are a contiguous slice of the edge list.
  - Within a core, dst nodes are processed in windows of 128 (the PSUM
    partition size). Every window's edges are padded to a fixed number of
    128-edge groups so all 8 cores run the same program.
  - Per 128-edge group:
      * dma_gather pulls the 128 source rows H[src] (fp16) from HBM into an
        SBUF tile G [128 edges x 128 feat] (edge e=j*128+p lands on
        partition p, slot j).
      * DVE builds S [128 edges x 128 nodes] = (iota == dstrel) * w with one
        fused tensor_scalar op.
      * TensorE accumulates aggT[feat, node] += G.T @ S in PSUM.
  - dma_gather indices are int16 (< 32768), so edges are split into a "lo"
    stream (src < 32768, gathered from H[:32768]) and a "hi" stream
    (src >= 32768, gathered from H[32768:]); both accumulate into the same
    PSUM window.
  - Final transform: out.T = W.T @ aggT (+ b) with W stationary, computed in
    512-column chunks; bias is added during the PSUM->SBUF copy (per-partition
    ACT bias, since the output is transposed: partitions = out features).
    The kernel writes out.T [128, NWIN*128] per core; the host transposes and
    concatenates.
"""

import os
import numpy as np

N_CORES = 8
N_NODES = 50000
D = 128
NPW = N_NODES // N_CORES  # 6250 dst nodes per core
WIN = 128
SPLIT = 32768  # int16-addressable row limit for dma_gather

# gather dtype: "f16" (half gather traffic, rel err ~3e-4) or "f32" (exact)
GDTYPE = os.environ.get("GCN_GDTYPE", "f16")

LAST_EXEC_NS = None  # set when GCN_TRACE=1
LAST_RESULTS = None


def _ceil_div(a, b):
    return -(-a // b)


def _prep(H, edge_src, edge_dst, edge_weight, n_cores=N_CORES):
    """Host-side sharding: per-core, per-window, per-stream edge lists with
    padding to common sizes. Returns per-core arrays + common geometry."""
    nwin = _ceil_div(NPW, WIN)
    # per (core, window, stream) edge index lists
    per_core = []
    max_lo = 0
    max_hi = 0
    max_all = 0
    for c in range(n_cores):
        n0, n1 = c * NPW, (c + 1) * NPW
        e0, e1 = np.searchsorted(edge_dst, [n0, n1])
        d = edge_dst[e0:e1] - n0
        s = edge_src[e0:e1]
        w = edge_weight[e0:e1]
        wins = []
        for wi in range(nwin):
            i0, i1 = np.searchsorted(d, [wi * WIN, wi * WIN + WIN])
            sw, dw, ww = s[i0:i1], d[i0:i1] - wi * WIN, w[i0:i1]
            lo = sw < SPLIT
            wins.append(
                (
                    (sw[lo], dw[lo], ww[lo]),
                    (sw[~lo] - SPLIT, dw[~lo], ww[~lo]),
                    (sw, dw, ww),
                )
            )
            max_lo = max(max_lo, int(lo.sum()))
            max_hi = max(max_hi, int((~lo).sum()))
            max_all = max(max_all, int(i1 - i0))
        per_core.append(wins)
    g_lo = max(1, _ceil_div(max_lo, 128))
    g_hi = max(1, _ceil_div(max_hi, 128))
    g_all = max(1, _ceil_div(max_all, 128))
    return per_core, nwin, g_lo, g_hi, g_all


def _chunks(g, maxg):
    """Split g groups into near-even chunks of <= maxg groups."""
    n = _ceil_div(g, maxg)
    base, rem = divmod(g, n)
    out = []
    c0 = 0
    for i in range(n):
        k = base + (1 if i < rem else 0)
        out.append((c0, k))
        c0 += k
    return out


def _device_arrays(wins, nwin, g, stream, chunks, np_meta_dtype, regs=None):
    """Build idx (wrapped-16 per gather call; call = (window, chunk)) +
    dstrel/weight arrays for one core and one stream ('lo'=0, 'hi'=1).

    regs is unused (kept for signature compat)."""
    ne = g * 128
    idx = np.zeros((nwin, ne), np.int16)
    drel = np.zeros((nwin, ne), np_meta_dtype)
    wgt = np.zeros((nwin, ne), np_meta_dtype)
    for wi in range(nwin):
        sw, dw, ww = wins[wi][stream]
        n = len(sw)
        idx[wi, :n] = sw.astype(np.int16)
        drel[wi, :n] = dw.astype(np_meta_dtype)
        wgt[wi, :n] = ww.astype(np_meta_dtype)
    parts = []
    for wi in range(nwin):
        for (c0, k) in chunks:
            flat = idx[wi, c0 * 128 : (c0 + k) * 128]
            parts.append(flat.reshape(-1, 16).T)  # [16, k*8]
    idx_dev = np.tile(np.concatenate(parts, axis=1), (8, 1))  # [128, nwin*g*8]
    # meta: [p, w*g + j] = value of edge e=j*128+p in window w
    drel_dev = np.ascontiguousarray(
        drel.reshape(nwin, g, 128).transpose(2, 0, 1).reshape(128, nwin * g)
    )
    wgt_dev = np.ascontiguousarray(
        wgt.reshape(nwin, g, 128).transpose(2, 0, 1).reshape(128, nwin * g)
    )
    return idx_dev, drel_dev, wgt_dev


def _device_arrays_ind(wins, nwin, g, np_meta_dtype):
    """idx (int32, natural [p, w*g+j] layout) + dstrel/weight arrays for the
    combined stream (indirect_dma_start variant)."""
    ne = g * 128
    idx = np.zeros((nwin, ne), np.int32)
    drel = np.zeros((nwin, ne), np_meta_dtype)
    wgt = np.zeros((nwin, ne), np_meta_dtype)
    for wi in range(nwin):
        sw, dw, ww = wins[wi][2]
        n = len(sw)
        idx[wi, :n] = sw
        drel[wi, :n] = dw.astype(np_meta_dtype)
        wgt[wi, :n] = ww.astype(np_meta_dtype)

    def dev(a):
        return np.ascontiguousarray(
            a.reshape(nwin, g, 128).transpose(2, 0, 1).reshape(128, nwin * g)
        )

    return dev(idx), dev(drel), dev(wgt)


def _build_program_ind(nwin, g_all, ch, n_src_rows, n_cores=N_CORES):
    """Indirect_dma_start variant: int32 indices, no lo/hi split."""
    from contextlib import ExitStack

    import concourse.bass as bass
    import concourse.tile as tile
    from concourse import bacc, mybir

    f32 = mybir.dt.float32
    gdt = mybir.dt.float16 if GDTYPE == "f16" else mybir.dt.float32
    i32 = mybir.dt.int32

    nc = bacc.Bacc(
        "TRN2", target_bir_lowering=False, debug=False, num_devices=n_cores,
    )

    npad = nwin * WIN
    h_t = nc.dram_tensor("h_src", [n_src_rows, D], gdt, kind="ExternalInput")
    idx_t = nc.dram_tensor("idx_all", [128, nwin * g_all], i32, kind="ExternalInput")
    drel_t = nc.dram_tensor("drel", [128, nwin * g_all], gdt, kind="ExternalInput")
    wgt_t = nc.dram_tensor("wgt", [128, nwin * g_all], gdt, kind="ExternalInput")
    iota_t = nc.dram_tensor("iota", [128, 128], gdt, kind="ExternalInput")
    w_t = nc.dram_tensor("wmat", [D, D], gdt, kind="ExternalInput")
    b_t = nc.dram_tensor("bcol", [D, 1], f32, kind="ExternalInput")
    out_t = nc.dram_tensor("outT", [D, npad], f32, kind="ExternalOutput")

    with tile.TileContext(nc) as tc:
        with ExitStack() as ctx:
            const = ctx.enter_context(tc.tile_pool(name="const", bufs=1))
            gpool = ctx.enter_context(tc.tile_pool(name="gather", bufs=6))
            spool = ctx.enter_context(tc.tile_pool(name="sel", bufs=3))
            opool = ctx.enter_context(tc.tile_pool(name="outsb", bufs=2))
            ps_agg = ctx.enter_context(tc.tile_pool(name="ps_agg", bufs=2, space="PSUM"))
            ps_out = ctx.enter_context(tc.tile_pool(name="ps_out", bufs=2, space="PSUM"))

            idx = const.tile(list(idx_t.shape), i32)
            drel = const.tile(list(drel_t.shape), gdt)
            wgt = const.tile(list(wgt_t.shape), gdt)
            iota = const.tile([128, 128], gdt)
            wmat = const.tile([D, D], gdt)
            bcol = const.tile([D, 1], f32)
            agg_all = const.tile([128, npad], gdt, tag="agg_all")

            for sb, dr in ((idx, idx_t), (drel, drel_t), (wgt, wgt_t),
                           (iota, iota_t), (wmat, w_t), (bcol, b_t)):
                nc.sync.dma_start(sb[:], dr[:])

            for wg in range(nwin):
                gtiles = []
                for (c0, k) in ch:
                    gt = gpool.tile([128, ch[0][1], 128], gdt, tag="g")
                    nc.gpsimd.indirect_dma_start(
                        out=gt[:, :k, :],
                        out_offset=None,
                        in_=h_t[:],
                        in_offset=bass.IndirectOffsetOnAxis(
                            ap=idx[:, wg * g_all + c0 : wg * g_all + c0 + k],
                            axis=0,
                        ),
                    )
                    gtiles.append((gt, c0, k))

                sh = (128, g_all, 128)
                c0m = wg * g_all
                s = spool.tile([128, g_all, 128], gdt, tag="sel")
                nc.vector.tensor_tensor(
                    s[:], iota[:, None, :].broadcast_to(sh),
                    drel[:, c0m : c0m + g_all, None].broadcast_to(sh),
                    mybir.AluOpType.is_equal,
                )
                nc.vector.tensor_tensor(
                    s[:], s[:], wgt[:, c0m : c0m + g_all, None].broadcast_to(sh),
                    mybir.AluOpType.mult,
                )

                psum = ps_agg.tile([128, 128], f32, tag="psagg")
                k_idx = 0
                for (gt, c0, k) in gtiles:
                    for j in range(k):
                        nc.tensor.matmul(
                            psum[:], gt[:, j, :], s[:, c0 + j, :],
                            start=(k_idx == 0), stop=(k_idx == g_all - 1),
                        )
                        k_idx += 1
                nc.scalar.copy(agg_all[:, wg * WIN : (wg + 1) * WIN], psum[:])

            CH = 512
            for t0 in range(0, npad, CH):
                n = min(CH, npad - t0)
                po = ps_out.tile([128, CH], f32, tag="psout")
                nc.tensor.matmul(
                    po[:, :n], wmat[:], agg_all[:, t0 : t0 + n],
                    start=True, stop=True,
                )
                ob = opool.tile([128, CH], f32, tag="outsb")
                nc.scalar.add(ob[:, :n], po[:, :n], bcol[:])
                nc.sync.dma_start(out_t[:, t0 : t0 + n], ob[:, :n])

    nc.compile()
    return nc


def _build_program_batched(nwin, g_lo, g_hi, gw_lo, gw_hi, n_src_rows,
                           batch=2, max_ke=63, n_cores=N_CORES,
                           scratch=32768, hi_indirect=False):
    """Batched-call variant: one dma_gather call per (batch of windows,
    stream), idx columns packed dense (only the first gw[w] groups of each
    window are gathered). single_packet=False so calls may exceed 65 ring
    descriptors; scratch sized so ring (scratch/64 descs) >= 8*max_ke+1."""
    from contextlib import ExitStack

    import concourse.bass as bass
    import concourse.tile as tile
    from concourse import bacc, mybir

    f32 = mybir.dt.float32
    gdt = mybir.dt.float16 if GDTYPE == "f16" else mybir.dt.float32
    i16 = mybir.dt.int16

    nc = bacc.Bacc(
        "TRN2", target_bir_lowering=False, debug=False, num_devices=n_cores,
        dynamic_dma_scratch_size=scratch,
    )

    npad = nwin * WIN
    n_lo_rows = min(SPLIT, n_src_rows)
    n_hi_rows = n_src_rows - n_lo_rows

    batches = [list(range(b, min(b + batch, nwin))) for b in range(0, nwin, batch)]
    # per-batch dense group counts and per-window offsets
    lo_off = {}
    hi_off = {}
    lo_tot = []
    hi_tot = []
    for bi, ws in enumerate(batches):
        o = 0
        for w in ws:
            lo_off[w] = o
            o += gw_lo[w]
        lo_tot.append(o)
        o = 0
        for w in ws:
            hi_off[w] = o
            o += gw_hi[w]
        hi_tot.append(o)
    glo_max = max(lo_tot)
    ghi_max = max(hi_tot)
    ncol_lo = sum(lo_tot)
    ncol_hi = sum(hi_tot)

    h_t = nc.dram_tensor("h_src", [n_src_rows, D], gdt, kind="ExternalInput")
    idx_lo_t = nc.dram_tensor("idx_lo", [128, ncol_lo * 8], i16, kind="ExternalInput")
    if hi_indirect:
        idx_hi_t = nc.dram_tensor(
            "idx_hi", [128, ncol_hi], mybir.dt.int32, kind="ExternalInput")
    else:
        idx_hi_t = nc.dram_tensor(
            "idx_hi", [128, ncol_hi * 8], i16, kind="ExternalInput")
    drel_lo_t = nc.dram_tensor("drel_lo", [128, nwin * g_lo], gdt, kind="ExternalInput")
    wgt_lo_t = nc.dram_tensor("wgt_lo", [128, nwin * g_lo], gdt, kind="ExternalInput")
    drel_hi_t = nc.dram_tensor("drel_hi", [128, nwin * g_hi], gdt, kind="ExternalInput")
    wgt_hi_t = nc.dram_tensor("wgt_hi", [128, nwin * g_hi], gdt, kind="ExternalInput")
    iota_t = nc.dram_tensor("iota", [128, 128], gdt, kind="ExternalInput")
    w_t = nc.dram_tensor("wmat", [D, D], gdt, kind="ExternalInput")
    b_t = nc.dram_tensor("bcol", [D, 1], f32, kind="ExternalInput")
    out_t = nc.dram_tensor("outT", [D, npad], f32, kind="ExternalOutput")

    with tile.TileContext(nc) as tc:
        with ExitStack() as ctx:
            const = ctx.enter_context(tc.tile_pool(name="const", bufs=1))
            gpool = ctx.enter_context(tc.tile_pool(name="gather", bufs=2))
            spool = ctx.enter_context(tc.tile_pool(name="sel", bufs=3))
            opool = ctx.enter_context(tc.tile_pool(name="outsb", bufs=2))
            ps_agg = ctx.enter_context(tc.tile_pool(name="ps_agg", bufs=2, space="PSUM"))
            ps_out = ctx.enter_context(tc.tile_pool(name="ps_out", bufs=2, space="PSUM"))

            idx_lo = const.tile(list(idx_lo_t.shape), i16)
            idx_hi = const.tile(
                list(idx_hi_t.shape),
                mybir.dt.int32 if hi_indirect else i16)
            drel_lo = const.tile(list(drel_lo_t.shape), gdt)
            wgt_lo = const.tile(list(wgt_lo_t.shape), gdt)
            drel_hi = const.tile(list(drel_hi_t.shape), gdt)
            wgt_hi = const.tile(list(wgt_hi_t.shape), gdt)
            iota = const.tile([128, 128], gdt)
            wmat = const.tile([D, D], gdt)
            bcol = const.tile([D, 1], f32)
            agg_all = const.tile([128, npad], gdt, tag="agg_all")

            for sb, dr in (
                (idx_lo, idx_lo_t), (idx_hi, idx_hi_t),
                (drel_lo, drel_lo_t), (wgt_lo, wgt_lo_t),
                (drel_hi, drel_hi_t), (wgt_hi, wgt_hi_t),
                (iota, iota_t), (wmat, w_t), (bcol, b_t),
            ):
                nc.sync.dma_start(sb[:], dr[:])

            h_lo = h_t[0:n_lo_rows, :]
            h_hi = h_t[n_lo_rows:n_src_rows, :] if n_hi_rows > 0 else None

            col_lo = 0
            col_hi = 0
            for bi, ws in enumerate(batches):
                # one gather call per stream per batch (split at max_ke)
                bsp = os.environ.get("GCN_BSP", "0") == "1"
                glo = gpool.tile([128, glo_max, 128], gdt, tag="glo")
                c0 = 0
                while c0 < lo_tot[bi]:
                    ke = min(max_ke, lo_tot[bi] - c0)
                    nc.gpsimd.dma_gather(
                        glo[:, c0 : c0 + ke, :], h_lo,
                        idx_lo[:, (col_lo + c0) * 8 : (col_lo + c0 + ke) * 8],
                        num_idxs=ke * 128, num_idxs_reg=ke * 128, elem_size=D,
                        single_packet=bsp,
                    )
                    c0 += ke
                ghi = gpool.tile([128, ghi_max, 128], gdt, tag="ghi")
                marker = None
                if hi_indirect:
                    nc.gpsimd.indirect_dma_start(
                        out=ghi[:, : hi_tot[bi], :],
                        out_offset=None,
                        in_=h_t[:],
                        in_offset=bass.IndirectOffsetOnAxis(
                            ap=idx_hi[:, col_hi : col_hi + hi_tot[bi]],
                            axis=0,
                        ),
                    )
                    # FIFO-ordering completion marker: a tiny SWDGE gather on
                    # the same queue whose (working) DMA semaphore fires only
                    # after the ring has drained past the indirect's
                    # descriptors. Every hi matmul is made to depend on it via
                    # a bypass op over s_hi.
                    marker = gpool.tile([128, 1, 128], gdt, tag="mrk")
                    nc.gpsimd.dma_gather(
                        marker[:, :1, :], h_lo, idx_lo[:, 0:8],
                        num_idxs=128, num_idxs_reg=128, elem_size=D,
                        single_packet=False,
                    )
                else:
                    c0 = 0
                    while c0 < hi_tot[bi]:
                        ke = min(max_ke, hi_tot[bi] - c0)
                        nc.gpsimd.dma_gather(
                            ghi[:, c0 : c0 + ke, :], h_hi,
                            idx_hi[:, (col_hi + c0) * 8 : (col_hi + c0 + ke) * 8],
                            num_idxs=ke * 128, num_idxs_reg=ke * 128, elem_size=D,
                            single_packet=bsp,
                        )
                        c0 += ke
                col_lo += lo_tot[bi]
                col_hi += hi_tot[bi]

                for wg in ws:
                    def build_s(meta_d, meta_w, g, tag):
                        s = spool.tile([128, g, 128], gdt, tag=tag)
                        sh = (128, g, 128)
                        c0m = wg * g
                        nc.vector.tensor_tensor(
                            s[:], iota[:, None, :].broadcast_to(sh),
                            meta_d[:, c0m : c0m + g, None].broadcast_to(sh),
                            mybir.AluOpType.is_equal,
                        )
                        nc.vector.tensor_tensor(
                            s[:], s[:],
                            meta_w[:, c0m : c0m + g, None].broadcast_to(sh),
                            mybir.AluOpType.mult,
                        )
                        return s

                    s_lo = build_s(drel_lo, wgt_lo, g_lo, "slo")
                    s_hi = build_s(drel_hi, wgt_hi, g_hi, "shi") if h_hi is not None else None
                    if s_hi is not None and marker is not None:
                        # bypass: out = in0 (s_hi unchanged) but creates a dep
                        # on the marker tile for every s_hi subtile.
                        nc.vector.tensor_tensor(
                            s_hi[:], s_hi[:],
                            marker[:, 0, None, 0:1].broadcast_to((128, g_hi, 128)),
                            mybir.AluOpType.bypass,
                        )

                    psum = ps_agg.tile([128, 128], f32, tag="psagg")
                    n_groups = gw_lo[wg] + (gw_hi[wg] if s_hi is not None else 0)
                    k_idx = 0
                    for j in range(gw_lo[wg]):
                        nc.tensor.matmul(
                            psum[:], glo[:, lo_off[wg] + j, :], s_lo[:, j, :],
                            start=(k_idx == 0), stop=(k_idx == n_groups - 1),
                        )
                        k_idx += 1
                    if s_hi is not None:
                        for j in range(gw_hi[wg]):
                            nc.tensor.matmul(
                                psum[:], ghi[:, hi_off[wg] + j, :], s_hi[:, j, :],
                                start=(k_idx == 0), stop=(k_idx == n_groups - 1),
                            )
                            k_idx += 1
                    nc.scalar.copy(agg_all[:, wg * WIN : (wg + 1) * WIN], psum[:])

                    # incremental output transform: as soon as a 512-col chunk
                    # of agg_all is complete, run W.T @ chunk so the tail
                    # after the last gather is just one chunk.
                    CH = 512
                    done = (wg + 1) * WIN
                    t0 = (done // CH - 1) * CH
                    if t0 >= 0 and done % CH == 0:
                        n = min(CH, npad - t0)
                        po = ps_out.tile([128, CH], f32, tag="psout")
                        nc.tensor.matmul(
                            po[:, :n], wmat[:], agg_all[:, t0 : t0 + n],
                            start=True, stop=True,
                        )
                        ob = opool.tile([128, CH], f32, tag="outsb")
                        nc.scalar.add(ob[:, :n], po[:, :n], bcol[:])
                        nc.sync.dma_start(out_t[:, t0 : t0 + n], ob[:, :n])

            CH = 512
            for t0 in range((npad // CH) * CH - (CH if npad % CH == 0 else 0),
                            npad, CH):
                if t0 < 0:
                    continue
                n = min(CH, npad - t0)
                if n <= 0:
                    continue
                po = ps_out.tile([128, CH], f32, tag="psout")
                nc.tensor.matmul(
                    po[:, :n], wmat[:], agg_all[:, t0 : t0 + n],
                    start=True, stop=True,
                )
                ob = opool.tile([128, CH], f32, tag="outsb")
                nc.scalar.add(ob[:, :n], po[:, :n], bcol[:])
                nc.sync.dma_start(out_t[:, t0 : t0 + n], ob[:, :n])

    nc.compile()
    return nc, batches, lo_tot, hi_tot


def _device_arrays_batched(wins, nwin, g, stream, gw, batches, np_meta_dtype,
                           ind_offset=None):
    """Dense-packed idx array (wrapped-16 per batch-call column blocks) plus
    per-window padded drel/wgt arrays (same layout as _device_arrays).

    ind_offset: if not None, build int32 indirect-layout idxs ([128, ncol]
    natural [p, col] order, absolute row ids = stored + ind_offset)."""
    ne = g * 128
    idx = np.zeros((nwin, ne), np.int32 if ind_offset is not None else np.int16)
    drel = np.zeros((nwin, ne), np_meta_dtype)
    wgt = np.zeros((nwin, ne), np_meta_dtype)
    for wi in range(nwin):
        sw, dw, ww = wins[wi][stream]
        n = len(sw)
        if ind_offset is not None:
            idx[wi, :n] = sw.astype(np.int32) + ind_offset
        else:
            idx[wi, :n] = sw.astype(np.int16)
        drel[wi, :n] = dw.astype(np_meta_dtype)
        wgt[wi, :n] = ww.astype(np_meta_dtype)
    if ind_offset is not None:
        # [p, dense col] where col runs over (batch, window, group j<gw[w])
        parts = []
        for ws in batches:
            for wi in ws:
                parts.append(idx[wi, : gw[wi] * 128].reshape(gw[wi], 128).T)
        idx_dev = np.ascontiguousarray(np.concatenate(parts, axis=1))
    else:
        # wrapped in 16 partitions, tiled x8 (SWDGE layout)
        parts = []
        for ws in batches:
            for wi in ws:
                flat = idx[wi, : gw[wi] * 128]
                parts.append(flat.reshape(-1, 16).T)  # [16, gw*8]
        idx_dev = np.tile(np.concatenate(parts, axis=1), (8, 1))
    drel_dev = np.ascontiguousarray(
        drel.reshape(nwin, g, 128).transpose(2, 0, 1).reshape(128, nwin * g)
    )
    wgt_dev = np.ascontiguousarray(
        wgt.reshape(nwin, g, 128).transpose(2, 0, 1).reshape(128, nwin * g)
    )
    return idx_dev, drel_dev, wgt_dev


def _build_program(nwin, g_lo, g_hi, ch_lo, ch_hi, n_src_rows, n_cores=N_CORES,
                   gw_lo=None, gw_hi=None):
    """Trace the (single, SPMD-shared) Bass program."""
    from contextlib import ExitStack

    import concourse.bass as bass
    import concourse.tile as tile
    from concourse import bacc, mybir

    f32 = mybir.dt.float32
    gdt = mybir.dt.float16 if GDTYPE == "f16" else mybir.dt.float32
    i16 = mybir.dt.int16

    nc = bacc.Bacc(
        "TRN2",
        target_bir_lowering=False,
        debug=False,
        num_devices=n_cores,
    )

    npad = nwin * WIN
    n_lo_rows = min(SPLIT, n_src_rows)
    n_hi_rows = n_src_rows - n_lo_rows

    h_t = nc.dram_tensor("h_src", [n_src_rows, D], gdt, kind="ExternalInput")
    idx_lo_t = nc.dram_tensor(
        "idx_lo", [128, nwin * g_lo * 8], i16, kind="ExternalInput",
    )
    idx_hi_t = nc.dram_tensor(
        "idx_hi", [128, nwin * g_hi * 8], i16, kind="ExternalInput",
    )
    drel_lo_t = nc.dram_tensor("drel_lo", [128, nwin * g_lo], gdt, kind="ExternalInput")
    wgt_lo_t = nc.dram_tensor("wgt_lo", [128, nwin * g_lo], gdt, kind="ExternalInput")
    drel_hi_t = nc.dram_tensor("drel_hi", [128, nwin * g_hi], gdt, kind="ExternalInput")
    wgt_hi_t = nc.dram_tensor("wgt_hi", [128, nwin * g_hi], gdt, kind="ExternalInput")
    iota_t = nc.dram_tensor("iota", [128, 128], gdt, kind="ExternalInput")
    w_t = nc.dram_tensor("wmat", [D, D], gdt, kind="ExternalInput")
    b_t = nc.dram_tensor("bcol", [D, 1], f32, kind="ExternalInput")
    out_t = nc.dram_tensor("outT", [D, npad], f32, kind="ExternalOutput")

    with tile.TileContext(nc) as tc:
        with ExitStack() as ctx:
            const = ctx.enter_context(tc.tile_pool(name="const", bufs=1))
            gpool = ctx.enter_context(tc.tile_pool(name="gather", bufs=6))
            spool = ctx.enter_context(tc.tile_pool(name="sel", bufs=3))
            opool = ctx.enter_context(tc.tile_pool(name="outsb", bufs=2))
            ps_agg = ctx.enter_context(
                tc.tile_pool(name="ps_agg", bufs=2, space="PSUM")
            )
            ps_out = ctx.enter_context(
                tc.tile_pool(name="ps_out", bufs=2, space="PSUM")
            )

            # resident constants / metadata
            idx_lo = const.tile(list(idx_lo_t.shape), i16)
            idx_hi = const.tile(list(idx_hi_t.shape), i16)
            drel_lo = const.tile(list(drel_lo_t.shape), gdt)
            wgt_lo = const.tile(list(wgt_lo_t.shape), gdt)
            drel_hi = const.tile(list(drel_hi_t.shape), gdt)
            wgt_hi = const.tile(list(wgt_hi_t.shape), gdt)
            iota = const.tile([128, 128], gdt)
            wmat = const.tile([D, D], gdt)
            bcol = const.tile([D, 1], f32)
            agg_all = const.tile([128, npad], gdt, tag="agg_all")

            for sb, dr in (
                (idx_lo, idx_lo_t), (idx_hi, idx_hi_t),
                (drel_lo, drel_lo_t), (wgt_lo, wgt_lo_t),
                (drel_hi, drel_hi_t), (wgt_hi, wgt_hi_t),
                (iota, iota_t), (wmat, w_t), (bcol, b_t),
            ):
                nc.sync.dma_start(sb[:], dr[:])

            h_lo = h_t[0:n_lo_rows, :]
            h_hi = h_t[n_lo_rows:n_src_rows, :] if n_hi_rows > 0 else None
            use_hi = h_hi is not None

            for wg in range(nwin):
                # gather this window's edges: one SWDGE call per chunk.
                # A call of k*128 idxs needs 8k+1 SWDGE ring entries; calls
                # with 97 entries (k=12) crash the exec unit on HW, k<=8 is
                # proven safe.
                # effective groups this window (shared across cores): groups
                # beyond the max valid count are pure padding -> not gathered,
                # not matmul'd. Every issued call is fully valid, so no tile
                # region is ever read without having been written.
                gwl = gw_lo[wg] if gw_lo else g_lo
                gwh = gw_hi[wg] if gw_hi else g_hi
                sp = os.environ.get("GCN_SP", "1") == "1"
                gtiles_lo = []
                for (c0, k) in ch_lo:
                    ke = min(max(gwl - c0, 0), k)
                    if ke == 0:
                        continue
                    gt = gpool.tile([128, ch_lo[0][1], 128], gdt, tag="glo")
                    col = (wg * g_lo + c0) * 8
                    nc.gpsimd.dma_gather(
                        gt[:, :ke, :], h_lo, idx_lo[:, col : col + ke * 8],
                        num_idxs=ke * 128, num_idxs_reg=ke * 128, elem_size=D,
                        single_packet=sp,
                    )
                    gtiles_lo.append((gt, c0, ke))
                gtiles_hi = []
                if use_hi:
                    for (c0, k) in ch_hi:
                        ke = min(max(gwh - c0, 0), k)
                        if ke == 0:
                            continue
                        gt = gpool.tile([128, ch_hi[0][1], 128], gdt, tag="ghi")
                        col = (wg * g_hi + c0) * 8
                        nc.gpsimd.dma_gather(
                            gt[:, :ke, :], h_hi, idx_hi[:, col : col + ke * 8],
                            num_idxs=ke * 128, num_idxs_reg=ke * 128, elem_size=D,
                            single_packet=sp,
                        )
                        gtiles_hi.append((gt, c0, ke))

                # S for the whole window in 2 DVE ops per stream:
                # S[p, j, n] = (n == drel[p, j]) * w[p, j] via step-0
                # broadcast APs on both operands.
                def build_s(meta_d, meta_w, g, tag):
                    s = spool.tile([128, g, 128], gdt, tag=tag)
                    sh = (128, g, 128)
                    c0m = wg * g
                    nc.vector.tensor_tensor(
                        s[:], iota[:, None, :].broadcast_to(sh),
                        meta_d[:, c0m : c0m + g, None].broadcast_to(sh),
                        mybir.AluOpType.is_equal,
                    )
                    nc.vector.tensor_tensor(
                        s[:], s[:],
                        meta_w[:, c0m : c0m + g, None].broadcast_to(sh),
                        mybir.AluOpType.mult,
                    )
                    return s

                s_lo = build_s(drel_lo, wgt_lo, g_lo, "slo")
                s_hi = build_s(drel_hi, wgt_hi, g_hi, "shi") if use_hi else None

                psum = ps_agg.tile([128, 128], f32, tag="psagg")
                n_groups = sum(k for _, _, k in gtiles_lo)
                n_groups += sum(k for _, _, k in gtiles_hi)
                k_idx = 0
                for (gt, c0, k), s_all in (
                    [(t, s_lo) for t in gtiles_lo]
                    + [(t, s_hi) for t in gtiles_hi]
                ):
                    for j in range(k):
                        nc.tensor.matmul(
                            psum[:], gt[:, j, :], s_all[:, c0 + j, :],
                            start=(k_idx == 0), stop=(k_idx == n_groups - 1),
                        )
                        k_idx += 1
                # aggT window -> SBUF (cast to gather dtype)
                nc.scalar.copy(agg_all[:, wg * WIN : (wg + 1) * WIN], psum[:])

            # out.T = W.T @ aggT + b, in 512-column chunks
            CH = 512
            for t0 in range(0, npad, CH):
                n = min(CH, npad - t0)
                po = ps_out.tile([128, CH], f32, tag="psout")
                nc.tensor.matmul(
                    po[:, :n], wmat[:], agg_all[:, t0 : t0 + n],
                    start=True, stop=True,
                )
                ob = opool.tile([128, CH], f32, tag="outsb")
                nc.scalar.add(ob[:, :n], po[:, :n], bcol[:])
                nc.sync.dma_start(out_t[:, t0 : t0 + n], ob[:, :n])

    nc.compile()
    return nc


def _make_in_maps(H, edge_src, edge_dst, edge_weight, W, b, per_core, nwin,
                  g_lo, g_hi, ch_lo, ch_hi):
    np_g = np.float16 if GDTYPE == "f16" else np.float32
    h_src = np.ascontiguousarray(H.astype(np_g))
    iota = np.tile(np.arange(128, dtype=np_g), (128, 1))
    wmat = np.ascontiguousarray(W.astype(np_g))
    bcol = np.ascontiguousarray(b.astype(np.float32).reshape(D, 1))
    in_maps = []
    for wins in per_core:
        idx_lo, drel_lo, wgt_lo = _device_arrays(wins, nwin, g_lo, 0, ch_lo, np_g)
        idx_hi, drel_hi, wgt_hi = _device_arrays(wins, nwin, g_hi, 1, ch_hi, np_g)
        in_maps.append(
            {
                "h_src": h_src,
                "idx_lo": idx_lo, "idx_hi": idx_hi,
                "drel_lo": drel_lo, "wgt_lo": wgt_lo,
                "drel_hi": drel_hi, "wgt_hi": wgt_hi,
                "iota": iota, "wmat": wmat, "bcol": bcol,
            }
        )
    return in_maps


def kernel(H, edge_src, edge_dst, edge_weight, W, b):
    global LAST_EXEC_NS
    from concourse import bass_utils

    H = np.asarray(H, dtype=np.float32)
    edge_src = np.asarray(edge_src, dtype=np.int32)
    edge_dst = np.asarray(edge_dst, dtype=np.int32)
    edge_weight = np.asarray(edge_weight, dtype=np.float32)
    W = np.asarray(W, dtype=np.float32)
    b = np.asarray(b, dtype=np.float32)

    per_core, nwin, g_lo, g_hi, g_all = _prep(H, edge_src, edge_dst, edge_weight)
    mode = os.environ.get("GCN_GATHER", "batched")
    if mode == "batched":
        batch = int(os.environ.get("GCN_BATCH", "2"))
        max_ke = int(os.environ.get("GCN_MAXKE", "63"))
        scratch = int(os.environ.get("GCN_SCRATCH", "32768"))
        gw_lo = []
        gw_hi = []
        for wi in range(nwin):
            m_lo = max(len(wins[wi][0][0]) for wins in per_core)
            m_hi = max(len(wins[wi][1][0]) for wins in per_core)
            gw_lo.append(min(g_lo, max(1, _ceil_div(m_lo, 128))))
            gw_hi.append(min(g_hi, max(1, _ceil_div(m_hi, 128))))
        hi_ind = os.environ.get("GCN_HI_IND", "0") == "1"
        nc, batches, lo_tot, hi_tot = _build_program_batched(
            nwin, g_lo, g_hi, gw_lo, gw_hi, N_NODES,
            batch=batch, max_ke=max_ke, scratch=scratch, hi_indirect=hi_ind,
        )
        np_g = np.float16 if GDTYPE == "f16" else np.float32
        h_src = np.ascontiguousarray(H.astype(np_g))
        iota = np.tile(np.arange(128, dtype=np_g), (128, 1))
        wmat = np.ascontiguousarray(W.astype(np_g))
        bcol = np.ascontiguousarray(b.astype(np.float32).reshape(D, 1))
        in_maps = []
        for wins in per_core:
            idx_lo, drel_lo, wgt_lo = _device_arrays_batched(
                wins, nwin, g_lo, 0, gw_lo, batches, np_g)
            idx_hi, drel_hi, wgt_hi = _device_arrays_batched(
                wins, nwin, g_hi, 1, gw_hi, batches, np_g,
                ind_offset=SPLIT if hi_ind else None)
            in_maps.append(
                {
                    "h_src": h_src,
                    "idx_lo": idx_lo, "idx_hi": idx_hi,
                    "drel_lo": drel_lo, "wgt_lo": wgt_lo,
                    "drel_hi": drel_hi, "wgt_hi": wgt_hi,
                    "iota": iota, "wmat": wmat, "bcol": bcol,
                }
            )
    elif mode == "indirect":
        maxg = int(os.environ.get("GCN_MAXG", "8"))
        ch = _chunks(g_all, maxg)
        nc = _build_program_ind(nwin, g_all, ch, N_NODES)
        np_g = np.float16 if GDTYPE == "f16" else np.float32
        h_src = np.ascontiguousarray(H.astype(np_g))
        iota = np.tile(np.arange(128, dtype=np_g), (128, 1))
        wmat = np.ascontiguousarray(W.astype(np_g))
        bcol = np.ascontiguousarray(b.astype(np.float32).reshape(D, 1))
        in_maps = []
        for wins in per_core:
            idx_all, drel, wgt = _device_arrays_ind(wins, nwin, g_all, np_g)
            in_maps.append({
                "h_src": h_src, "idx_all": idx_all, "drel": drel, "wgt": wgt,
                "iota": iota, "wmat": wmat, "bcol": bcol,
            })
    else:
        maxg = int(os.environ.get("GCN_MAXG", "8"))
        # round group counts up so every chunk has equal size: a gather tile
        # slot must always be written over its FULL extent when fully valid,
        # otherwise a smaller earlier write leaves never-written (non-finite)
        # columns that a later partially-valid call exposes to the matmul.
        g_lo = _ceil_div(g_lo, maxg) * maxg if g_lo > maxg else g_lo
        g_hi = _ceil_div(g_hi, maxg) * maxg if g_hi > maxg else g_hi
        ch_lo = _chunks(g_lo, maxg)
        ch_hi = _chunks(g_hi, maxg)
        assert len({k for _, k in ch_lo}) == 1 and len({k for _, k in ch_hi}) == 1
        # per-window effective group counts (shared across cores): only
        # gather/matmul groups that contain at least one real edge on the
        # max-count core; the rest are pure padding.
        trim = os.environ.get("GCN_TRIM", "1") == "1"
        gw_lo = []
        gw_hi = []
        for wi in range(nwin):
            m_lo = max(len(wins[wi][0][0]) for wins in per_core)
            m_hi = max(len(wins[wi][1][0]) for wins in per_core)
            gw_lo.append(min(g_lo, max(1, _ceil_div(m_lo, 128))) if trim else g_lo)
            gw_hi.append(min(g_hi, max(1, _ceil_div(m_hi, 128))) if trim else g_hi)
        nc = _build_program(nwin, g_lo, g_hi, ch_lo, ch_hi, N_NODES,
                            gw_lo=gw_lo, gw_hi=gw_hi)
        in_maps = _make_in_maps(
            H, edge_src, edge_dst, edge_weight, W, b, per_core, nwin, g_lo,
            g_hi, ch_lo, ch_hi,
        )

    if os.environ.get("GCN_SIM", "0") == "1":  # CoreSim path for testing
        from concourse.bass_interp import CoreSim

        out = np.empty((N_NODES, D), np.float32)
        for c in range(N_CORES):
            sim = CoreSim(nc)
            for k2, v2 in in_maps[c].items():
                sim.tensor(k2)[:] = v2
            sim.simulate()
            out[c * NPW : (c + 1) * NPW, :] = np.array(
                sim.tensor("outT")).T[:NPW]
        return out

    trace = os.environ.get("GCN_TRACE", "0") == "1"
    kw = {}
    if trace:
        import shutil
        td = "/tmp/gcn_ntff"
        shutil.rmtree(td, ignore_errors=True)
        os.makedirs(td, exist_ok=True)
        kw["tmpdir"] = td
    # a previously crashed NEFF can leave the exec unit transiently
    # unrecoverable; recovery has been observed to take up to a few minutes,
    # so retry with escalating backoff
    import time as _time
    last_err = None
    for backoff in (15, 45, 90, 0):
        try:
            res = bass_utils.run_bass_kernel_spmd(
                nc, in_maps, core_ids=list(range(N_CORES)), trace=trace, **kw
            )
            break
        except Exception as e:
            last_err = e
            if backoff:
                _time.sleep(backoff)
    else:
        raise last_err
    LAST_EXEC_NS = res.exec_time_ns
    global LAST_RESULTS
    LAST_RESULTS = res

    out = np.empty((N_NODES, D), np.float32)
    for c in range(N_CORES):
        outT = res.results[c]["outT"]
        out[c * NPW : (c + 1) * NPW, :] = outT.T[:NPW]
    return out



# revision 16
# speedup vs baseline: 1.1060x; 1.0168x over previous
"""GCN layer (gather -> weighted scatter-sum -> dense transform) on 8 trn2 cores.

Default path (GCN_GATHER=batched): same algorithm as the per-window path
below, but gather calls are batched — one dma_gather per (4-window batch,
stream) with single_packet=False (multi-packet rings; calls may exceed the
65-descriptor single-packet limit) and idx columns packed dense (per-window
trimmed group counts). Performance notes (HW-measured):
  - SWDGE dma_gather costs ~5.5-7.9 ns/idx, engine-serial on the Q7 pair;
    with per-desc packets (single_packet=False) the SDMA drain of 256B
    descriptors (~125ns/desc/engine) binds at ~7.8ns/idx. All SWDGE
    configurations converge to ~1.66-1.71ms for the ~212k idxs/core.
  - gpsimd.indirect_dma_start is NOT usable: on HW it lands rows on
    partition 0 only (CoreSim models it differently) and signals no DMA
    completion semaphores.
  - SBUF-source dma_gather (transpose=True) crashes the exec unit
    (NRT_EXEC_UNIT_UNRECOVERABLE).

Strategy (1-D row partitioning of destination nodes):
  - Core c owns destination nodes [c*NPW, (c+1)*NPW). edge_dst is sorted, so
    each core's edges are a contiguous slice of the edge list.
  - Within a core, dst nodes are processed in windows of 128 (the PSUM
    partition size). Every window's edges are padded to a fixed number of
    128-edge groups so all 8 cores run the same program.
  - Per 128-edge group:
      * dma_gather pulls the 128 source rows H[src] (fp16) from HBM into an
        SBUF tile G [128 edges x 128 feat] (edge e=j*128+p lands on
        partition p, slot j).
      * DVE builds S [128 edges x 128 nodes] = (iota == dstrel) * w with one
        fused tensor_scalar op.
      * TensorE accumulates aggT[feat, node] += G.T @ S in PSUM.
  - dma_gather indices are int16 (< 32768), so edges are split into a "lo"
    stream (src < 32768, gathered from H[:32768]) and a "hi" stream
    (src >= 32768, gathered from H[32768:]); both accumulate into the same
    PSUM window.
  - Final transform: out.T = W.T @ aggT (+ b) with W stationary, computed in
    512-column chunks; bias is added during the PSUM->SBUF copy (per-partition
    ACT bias, since the output is transposed: partitions = out features).
    The kernel writes out.T [128, NWIN*128] per core; the host transposes and
    concatenates.
"""

import os
import numpy as np

N_CORES = 8
N_NODES = 50000
D = 128
NPW = N_NODES // N_CORES  # 6250 dst nodes per core
WIN = 128
SPLIT = 32768  # int16-addressable row limit for dma_gather

# gather dtype: "f16" (half gather traffic, rel err ~3e-4) or "f32" (exact)
GDTYPE = os.environ.get("GCN_GDTYPE", "f16")

LAST_EXEC_NS = None  # set when GCN_TRACE=1
LAST_RESULTS = None


def _ceil_div(a, b):
    return -(-a // b)


def _prep(H, edge_src, edge_dst, edge_weight, n_cores=N_CORES):
    """Host-side sharding: per-core, per-window, per-stream edge lists with
    padding to common sizes. Returns per-core arrays + common geometry."""
    nwin = _ceil_div(NPW, WIN)
    # per (core, window, stream) edge index lists
    per_core = []
    max_lo = 0
    max_hi = 0
    max_all = 0
    for c in range(n_cores):
        n0, n1 = c * NPW, (c + 1) * NPW
        e0, e1 = np.searchsorted(edge_dst, [n0, n1])
        d = edge_dst[e0:e1] - n0
        s = edge_src[e0:e1]
        w = edge_weight[e0:e1]
        wins = []
        for wi in range(nwin):
            i0, i1 = np.searchsorted(d, [wi * WIN, wi * WIN + WIN])
            sw, dw, ww = s[i0:i1], d[i0:i1] - wi * WIN, w[i0:i1]
            lo = sw < SPLIT
            wins.append(
                (
                    (sw[lo], dw[lo], ww[lo]),
                    (sw[~lo] - SPLIT, dw[~lo], ww[~lo]),
                    (sw, dw, ww),
                )
            )
            max_lo = max(max_lo, int(lo.sum()))
            max_hi = max(max_hi, int((~lo).sum()))
            max_all = max(max_all, int(i1 - i0))
        per_core.append(wins)
    g_lo = max(1, _ceil_div(max_lo, 128))
    g_hi = max(1, _ceil_div(max_hi, 128))
    g_all = max(1, _ceil_div(max_all, 128))
    return per_core, nwin, g_lo, g_hi, g_all


def _chunks(g, maxg):
    """Split g groups into near-even chunks of <= maxg groups."""
    n = _ceil_div(g, maxg)
    base, rem = divmod(g, n)
    out = []
    c0 = 0
    for i in range(n):
        k = base + (1 if i < rem else 0)
        out.append((c0, k))
        c0 += k
    return out


def _device_arrays(wins, nwin, g, stream, chunks, np_meta_dtype, regs=None):
    """Build idx (wrapped-16 per gather call; call = (window, chunk)) +
    dstrel/weight arrays for one core and one stream ('lo'=0, 'hi'=1).

    regs is unused (kept for signature compat)."""
    ne = g * 128
    idx = np.zeros((nwin, ne), np.int16)
    drel = np.zeros((nwin, ne), np_meta_dtype)
    wgt = np.zeros((nwin, ne), np_meta_dtype)
    for wi in range(nwin):
        sw, dw, ww = wins[wi][stream]
        n = len(sw)
        idx[wi, :n] = sw.astype(np.int16)
        drel[wi, :n] = dw.astype(np_meta_dtype)
        wgt[wi, :n] = ww.astype(np_meta_dtype)
    parts = []
    for wi in range(nwin):
        for (c0, k) in chunks:
            flat = idx[wi, c0 * 128 : (c0 + k) * 128]
            parts.append(flat.reshape(-1, 16).T)  # [16, k*8]
    idx_dev = np.tile(np.concatenate(parts, axis=1), (8, 1))  # [128, nwin*g*8]
    # meta: [p, w*g + j] = value of edge e=j*128+p in window w
    drel_dev = np.ascontiguousarray(
        drel.reshape(nwin, g, 128).transpose(2, 0, 1).reshape(128, nwin * g)
    )
    wgt_dev = np.ascontiguousarray(
        wgt.reshape(nwin, g, 128).transpose(2, 0, 1).reshape(128, nwin * g)
    )
    return idx_dev, drel_dev, wgt_dev


def _device_arrays_ind(wins, nwin, g, np_meta_dtype):
    """idx (int32, natural [p, w*g+j] layout) + dstrel/weight arrays for the
    combined stream (indirect_dma_start variant)."""
    ne = g * 128
    idx = np.zeros((nwin, ne), np.int32)
    drel = np.zeros((nwin, ne), np_meta_dtype)
    wgt = np.zeros((nwin, ne), np_meta_dtype)
    for wi in range(nwin):
        sw, dw, ww = wins[wi][2]
        n = len(sw)
        idx[wi, :n] = sw
        drel[wi, :n] = dw.astype(np_meta_dtype)
        wgt[wi, :n] = ww.astype(np_meta_dtype)

    def dev(a):
        return np.ascontiguousarray(
            a.reshape(nwin, g, 128).transpose(2, 0, 1).reshape(128, nwin * g)
        )

    return dev(idx), dev(drel), dev(wgt)


def _build_program_ind(nwin, g_all, ch, n_src_rows, n_cores=N_CORES):
    """Indirect_dma_start variant: int32 indices, no lo/hi split."""
    from contextlib import ExitStack

    import concourse.bass as bass
    import concourse.tile as tile
    from concourse import bacc, mybir

    f32 = mybir.dt.float32
    gdt = mybir.dt.float16 if GDTYPE == "f16" else mybir.dt.float32
    i32 = mybir.dt.int32

    nc = bacc.Bacc(
        "TRN2", target_bir_lowering=False, debug=False, num_devices=n_cores,
    )

    npad = nwin * WIN
    h_t = nc.dram_tensor("h_src", [n_src_rows, D], gdt, kind="ExternalInput")
    idx_t = nc.dram_tensor("idx_all", [128, nwin * g_all], i32, kind="ExternalInput")
    drel_t = nc.dram_tensor("drel", [128, nwin * g_all], gdt, kind="ExternalInput")
    wgt_t = nc.dram_tensor("wgt", [128, nwin * g_all], gdt, kind="ExternalInput")
    iota_t = nc.dram_tensor("iota", [128, 128], gdt, kind="ExternalInput")
    w_t = nc.dram_tensor("wmat", [D, D], gdt, kind="ExternalInput")
    b_t = nc.dram_tensor("bcol", [D, 1], f32, kind="ExternalInput")
    out_t = nc.dram_tensor("outT", [D, npad], f32, kind="ExternalOutput")

    with tile.TileContext(nc) as tc:
        with ExitStack() as ctx:
            const = ctx.enter_context(tc.tile_pool(name="const", bufs=1))
            gpool = ctx.enter_context(tc.tile_pool(name="gather", bufs=6))
            spool = ctx.enter_context(tc.tile_pool(name="sel", bufs=3))
            opool = ctx.enter_context(tc.tile_pool(name="outsb", bufs=2))
            ps_agg = ctx.enter_context(tc.tile_pool(name="ps_agg", bufs=2, space="PSUM"))
            ps_out = ctx.enter_context(tc.tile_pool(name="ps_out", bufs=2, space="PSUM"))

            idx = const.tile(list(idx_t.shape), i32)
            drel = const.tile(list(drel_t.shape), gdt)
            wgt = const.tile(list(wgt_t.shape), gdt)
            iota = const.tile([128, 128], gdt)
            wmat = const.tile([D, D], gdt)
            bcol = const.tile([D, 1], f32)
            agg_all = const.tile([128, npad], gdt, tag="agg_all")

            for sb, dr in ((idx, idx_t), (drel, drel_t), (wgt, wgt_t),
                           (iota, iota_t), (wmat, w_t), (bcol, b_t)):
                nc.sync.dma_start(sb[:], dr[:])

            for wg in range(nwin):
                gtiles = []
                for (c0, k) in ch:
                    gt = gpool.tile([128, ch[0][1], 128], gdt, tag="g")
                    nc.gpsimd.indirect_dma_start(
                        out=gt[:, :k, :],
                        out_offset=None,
                        in_=h_t[:],
                        in_offset=bass.IndirectOffsetOnAxis(
                            ap=idx[:, wg * g_all + c0 : wg * g_all + c0 + k],
                            axis=0,
                        ),
                    )
                    gtiles.append((gt, c0, k))

                sh = (128, g_all, 128)
                c0m = wg * g_all
                s = spool.tile([128, g_all, 128], gdt, tag="sel")
                nc.vector.tensor_tensor(
                    s[:], iota[:, None, :].broadcast_to(sh),
                    drel[:, c0m : c0m + g_all, None].broadcast_to(sh),
                    mybir.AluOpType.is_equal,
                )
                nc.vector.tensor_tensor(
                    s[:], s[:], wgt[:, c0m : c0m + g_all, None].broadcast_to(sh),
                    mybir.AluOpType.mult,
                )

                psum = ps_agg.tile([128, 128], f32, tag="psagg")
                k_idx = 0
                for (gt, c0, k) in gtiles:
                    for j in range(k):
                        nc.tensor.matmul(
                            psum[:], gt[:, j, :], s[:, c0 + j, :],
                            start=(k_idx == 0), stop=(k_idx == g_all - 1),
                        )
                        k_idx += 1
                nc.scalar.copy(agg_all[:, wg * WIN : (wg + 1) * WIN], psum[:])

            CH = 512
            for t0 in range(0, npad, CH):
                n = min(CH, npad - t0)
                po = ps_out.tile([128, CH], f32, tag="psout")
                nc.tensor.matmul(
                    po[:, :n], wmat[:], agg_all[:, t0 : t0 + n],
                    start=True, stop=True,
                )
                ob = opool.tile([128, CH], f32, tag="outsb")
                nc.scalar.add(ob[:, :n], po[:, :n], bcol[:])
                nc.sync.dma_start(out_t[:, t0 : t0 + n], ob[:, :n])

    nc.compile()
    return nc


def _build_program_batched(nwin, g_lo, g_hi, gw_lo, gw_hi, n_src_rows,
                           batch=2, max_ke=63, n_cores=N_CORES,
                           scratch=32768, hi_indirect=False):
    """Batched-call variant: one dma_gather call per (batch of windows,
    stream), idx columns packed dense (only the first gw[w] groups of each
    window are gathered). single_packet=False so calls may exceed 65 ring
    descriptors; scratch sized so ring (scratch/64 descs) >= 8*max_ke+1."""
    from contextlib import ExitStack

    import concourse.bass as bass
    import concourse.tile as tile
    from concourse import bacc, mybir

    f32 = mybir.dt.float32
    gdt = mybir.dt.float16 if GDTYPE == "f16" else mybir.dt.float32
    i16 = mybir.dt.int16

    nc = bacc.Bacc(
        "TRN2", target_bir_lowering=False, debug=False, num_devices=n_cores,
        dynamic_dma_scratch_size=scratch,
    )

    npad = nwin * WIN
    n_lo_rows = min(SPLIT, n_src_rows)
    n_hi_rows = n_src_rows - n_lo_rows

    batches = [list(range(b, min(b + batch, nwin))) for b in range(0, nwin, batch)]
    # per-batch dense group counts and per-window offsets
    lo_off = {}
    hi_off = {}
    lo_tot = []
    hi_tot = []
    for bi, ws in enumerate(batches):
        o = 0
        for w in ws:
            lo_off[w] = o
            o += gw_lo[w]
        lo_tot.append(o)
        o = 0
        for w in ws:
            hi_off[w] = o
            o += gw_hi[w]
        hi_tot.append(o)
    glo_max = max(lo_tot)
    ghi_max = max(hi_tot)
    ncol_lo = sum(lo_tot)
    ncol_hi = sum(hi_tot)

    h_t = nc.dram_tensor("h_src", [n_src_rows, D], gdt, kind="ExternalInput")
    idx_lo_t = nc.dram_tensor("idx_lo", [128, ncol_lo * 8], i16, kind="ExternalInput")
    if hi_indirect:
        idx_hi_t = nc.dram_tensor(
            "idx_hi", [128, ncol_hi], mybir.dt.int32, kind="ExternalInput")
    else:
        idx_hi_t = nc.dram_tensor(
            "idx_hi", [128, ncol_hi * 8], i16, kind="ExternalInput")
    drel_lo_t = nc.dram_tensor("drel_lo", [128, nwin * g_lo], gdt, kind="ExternalInput")
    wgt_lo_t = nc.dram_tensor("wgt_lo", [128, nwin * g_lo], gdt, kind="ExternalInput")
    drel_hi_t = nc.dram_tensor("drel_hi", [128, nwin * g_hi], gdt, kind="ExternalInput")
    wgt_hi_t = nc.dram_tensor("wgt_hi", [128, nwin * g_hi], gdt, kind="ExternalInput")
    iota_t = nc.dram_tensor("iota", [128, 128], gdt, kind="ExternalInput")
    w_t = nc.dram_tensor("wmat", [D, D], gdt, kind="ExternalInput")
    b_t = nc.dram_tensor("bcol", [D, 1], f32, kind="ExternalInput")
    out_t = nc.dram_tensor("outT", [D, npad], f32, kind="ExternalOutput")

    with tile.TileContext(nc) as tc:
        with ExitStack() as ctx:
            const = ctx.enter_context(tc.tile_pool(name="const", bufs=1))
            gpool = ctx.enter_context(tc.tile_pool(name="gather", bufs=2))
            spool = ctx.enter_context(tc.tile_pool(name="sel", bufs=3))
            opool = ctx.enter_context(tc.tile_pool(name="outsb", bufs=2))
            ps_agg = ctx.enter_context(tc.tile_pool(name="ps_agg", bufs=2, space="PSUM"))
            ps_out = ctx.enter_context(tc.tile_pool(name="ps_out", bufs=2, space="PSUM"))

            idx_lo = const.tile(list(idx_lo_t.shape), i16)
            idx_hi = const.tile(
                list(idx_hi_t.shape),
                mybir.dt.int32 if hi_indirect else i16)
            drel_lo = const.tile(list(drel_lo_t.shape), gdt)
            wgt_lo = const.tile(list(wgt_lo_t.shape), gdt)
            drel_hi = const.tile(list(drel_hi_t.shape), gdt)
            wgt_hi = const.tile(list(wgt_hi_t.shape), gdt)
            iota = const.tile([128, 128], gdt)
            wmat = const.tile([D, D], gdt)
            bcol = const.tile([D, 1], f32)
            agg_all = const.tile([128, npad], gdt, tag="agg_all")

            for sb, dr in (
                (idx_lo, idx_lo_t), (idx_hi, idx_hi_t),
                (drel_lo, drel_lo_t), (wgt_lo, wgt_lo_t),
                (drel_hi, drel_hi_t), (wgt_hi, wgt_hi_t),
                (iota, iota_t), (wmat, w_t), (bcol, b_t),
            ):
                nc.sync.dma_start(sb[:], dr[:])

            h_lo = h_t[0:n_lo_rows, :]
            h_hi = h_t[n_lo_rows:n_src_rows, :] if n_hi_rows > 0 else None

            col_lo = 0
            col_hi = 0
            for bi, ws in enumerate(batches):
                # one gather call per stream per batch (split at max_ke)
                bsp = os.environ.get("GCN_BSP", "0") == "1"
                glo = gpool.tile([128, glo_max, 128], gdt, tag="glo")
                c0 = 0
                while c0 < lo_tot[bi]:
                    ke = min(max_ke, lo_tot[bi] - c0)
                    nc.gpsimd.dma_gather(
                        glo[:, c0 : c0 + ke, :], h_lo,
                        idx_lo[:, (col_lo + c0) * 8 : (col_lo + c0 + ke) * 8],
                        num_idxs=ke * 128, num_idxs_reg=ke * 128, elem_size=D,
                        single_packet=bsp,
                    )
                    c0 += ke
                ghi = gpool.tile([128, ghi_max, 128], gdt, tag="ghi")
                marker = None
                if hi_indirect:
                    nc.gpsimd.indirect_dma_start(
                        out=ghi[:, : hi_tot[bi], :],
                        out_offset=None,
                        in_=h_t[:],
                        in_offset=bass.IndirectOffsetOnAxis(
                            ap=idx_hi[:, col_hi : col_hi + hi_tot[bi]],
                            axis=0,
                        ),
                    )
                    # FIFO-ordering completion marker: a tiny SWDGE gather on
                    # the same queue whose (working) DMA semaphore fires only
                    # after the ring has drained past the indirect's
                    # descriptors. Every hi matmul is made to depend on it via
                    # a bypass op over s_hi.
                    marker = gpool.tile([128, 1, 128], gdt, tag="mrk")
                    nc.gpsimd.dma_gather(
                        marker[:, :1, :], h_lo, idx_lo[:, 0:8],
                        num_idxs=128, num_idxs_reg=128, elem_size=D,
                        single_packet=False,
                    )
                else:
                    c0 = 0
                    while c0 < hi_tot[bi]:
                        ke = min(max_ke, hi_tot[bi] - c0)
                        nc.gpsimd.dma_gather(
                            ghi[:, c0 : c0 + ke, :], h_hi,
                            idx_hi[:, (col_hi + c0) * 8 : (col_hi + c0 + ke) * 8],
                            num_idxs=ke * 128, num_idxs_reg=ke * 128, elem_size=D,
                            single_packet=bsp,
                        )
                        c0 += ke
                col_lo += lo_tot[bi]
                col_hi += hi_tot[bi]

                for wg in ws:
                    def build_s(meta_d, meta_w, g, tag):
                        s = spool.tile([128, g, 128], gdt, tag=tag)
                        sh = (128, g, 128)
                        c0m = wg * g
                        nc.vector.tensor_tensor(
                            s[:], iota[:, None, :].broadcast_to(sh),
                            meta_d[:, c0m : c0m + g, None].broadcast_to(sh),
                            mybir.AluOpType.is_equal,
                        )
                        nc.vector.tensor_tensor(
                            s[:], s[:],
                            meta_w[:, c0m : c0m + g, None].broadcast_to(sh),
                            mybir.AluOpType.mult,
                        )
                        return s

                    s_lo = build_s(drel_lo, wgt_lo, g_lo, "slo")
                    s_hi = build_s(drel_hi, wgt_hi, g_hi, "shi") if h_hi is not None else None
                    if s_hi is not None and marker is not None:
                        # bypass: out = in0 (s_hi unchanged) but creates a dep
                        # on the marker tile for every s_hi subtile.
                        nc.vector.tensor_tensor(
                            s_hi[:], s_hi[:],
                            marker[:, 0, None, 0:1].broadcast_to((128, g_hi, 128)),
                            mybir.AluOpType.bypass,
                        )

                    psum = ps_agg.tile([128, 128], f32, tag="psagg")
                    n_groups = gw_lo[wg] + (gw_hi[wg] if s_hi is not None else 0)
                    k_idx = 0
                    for j in range(gw_lo[wg]):
                        nc.tensor.matmul(
                            psum[:], glo[:, lo_off[wg] + j, :], s_lo[:, j, :],
                            start=(k_idx == 0), stop=(k_idx == n_groups - 1),
                        )
                        k_idx += 1
                    if s_hi is not None:
                        for j in range(gw_hi[wg]):
                            nc.tensor.matmul(
                                psum[:], ghi[:, hi_off[wg] + j, :], s_hi[:, j, :],
                                start=(k_idx == 0), stop=(k_idx == n_groups - 1),
                            )
                            k_idx += 1
                    nc.scalar.copy(agg_all[:, wg * WIN : (wg + 1) * WIN], psum[:])

                    # incremental output transform: as soon as a 512-col chunk
                    # of agg_all is complete, run W.T @ chunk so the tail
                    # after the last gather is just one chunk.
                    CH = 512
                    done = (wg + 1) * WIN
                    t0 = (done // CH - 1) * CH
                    if t0 >= 0 and done % CH == 0:
                        n = min(CH, npad - t0)
                        po = ps_out.tile([128, CH], f32, tag="psout")
                        nc.tensor.matmul(
                            po[:, :n], wmat[:], agg_all[:, t0 : t0 + n],
                            start=True, stop=True,
                        )
                        ob = opool.tile([128, CH], f32, tag="outsb")
                        nc.scalar.add(ob[:, :n], po[:, :n], bcol[:])
                        nc.sync.dma_start(out_t[:, t0 : t0 + n], ob[:, :n])

            CH = 512
            for t0 in range((npad // CH) * CH - (CH if npad % CH == 0 else 0),
                            npad, CH):
                if t0 < 0:
                    continue
                n = min(CH, npad - t0)
                if n <= 0:
                    continue
                po = ps_out.tile([128, CH], f32, tag="psout")
                nc.tensor.matmul(
                    po[:, :n], wmat[:], agg_all[:, t0 : t0 + n],
                    start=True, stop=True,
                )
                ob = opool.tile([128, CH], f32, tag="outsb")
                nc.scalar.add(ob[:, :n], po[:, :n], bcol[:])
                nc.sync.dma_start(out_t[:, t0 : t0 + n], ob[:, :n])

    nc.compile()
    return nc, batches, lo_tot, hi_tot


def _device_arrays_batched(wins, nwin, g, stream, gw, batches, np_meta_dtype,
                           ind_offset=None):
    """Dense-packed idx array (wrapped-16 per batch-call column blocks) plus
    per-window padded drel/wgt arrays (same layout as _device_arrays).

    ind_offset: if not None, build int32 indirect-layout idxs ([128, ncol]
    natural [p, col] order, absolute row ids = stored + ind_offset)."""
    ne = g * 128
    idx = np.zeros((nwin, ne), np.int32 if ind_offset is not None else np.int16)
    drel = np.zeros((nwin, ne), np_meta_dtype)
    wgt = np.zeros((nwin, ne), np_meta_dtype)
    for wi in range(nwin):
        sw, dw, ww = wins[wi][stream]
        n = len(sw)
        if ind_offset is not None:
            idx[wi, :n] = sw.astype(np.int32) + ind_offset
        else:
            idx[wi, :n] = sw.astype(np.int16)
        drel[wi, :n] = dw.astype(np_meta_dtype)
        wgt[wi, :n] = ww.astype(np_meta_dtype)
    if ind_offset is not None:
        # [p, dense col] where col runs over (batch, window, group j<gw[w])
        parts = []
        for ws in batches:
            for wi in ws:
                parts.append(idx[wi, : gw[wi] * 128].reshape(gw[wi], 128).T)
        idx_dev = np.ascontiguousarray(np.concatenate(parts, axis=1))
    else:
        # wrapped in 16 partitions, tiled x8 (SWDGE layout)
        parts = []
        for ws in batches:
            for wi in ws:
                flat = idx[wi, : gw[wi] * 128]
                parts.append(flat.reshape(-1, 16).T)  # [16, gw*8]
        idx_dev = np.tile(np.concatenate(parts, axis=1), (8, 1))
    drel_dev = np.ascontiguousarray(
        drel.reshape(nwin, g, 128).transpose(2, 0, 1).reshape(128, nwin * g)
    )
    wgt_dev = np.ascontiguousarray(
        wgt.reshape(nwin, g, 128).transpose(2, 0, 1).reshape(128, nwin * g)
    )
    return idx_dev, drel_dev, wgt_dev


def _build_program(nwin, g_lo, g_hi, ch_lo, ch_hi, n_src_rows, n_cores=N_CORES,
                   gw_lo=None, gw_hi=None):
    """Trace the (single, SPMD-shared) Bass program."""
    from contextlib import ExitStack

    import concourse.bass as bass
    import concourse.tile as tile
    from concourse import bacc, mybir

    f32 = mybir.dt.float32
    gdt = mybir.dt.float16 if GDTYPE == "f16" else mybir.dt.float32
    i16 = mybir.dt.int16

    nc = bacc.Bacc(
        "TRN2",
        target_bir_lowering=False,
        debug=False,
        num_devices=n_cores,
    )

    npad = nwin * WIN
    n_lo_rows = min(SPLIT, n_src_rows)
    n_hi_rows = n_src_rows - n_lo_rows

    h_t = nc.dram_tensor("h_src", [n_src_rows, D], gdt, kind="ExternalInput")
    idx_lo_t = nc.dram_tensor(
        "idx_lo", [128, nwin * g_lo * 8], i16, kind="ExternalInput",
    )
    idx_hi_t = nc.dram_tensor(
        "idx_hi", [128, nwin * g_hi * 8], i16, kind="ExternalInput",
    )
    drel_lo_t = nc.dram_tensor("drel_lo", [128, nwin * g_lo], gdt, kind="ExternalInput")
    wgt_lo_t = nc.dram_tensor("wgt_lo", [128, nwin * g_lo], gdt, kind="ExternalInput")
    drel_hi_t = nc.dram_tensor("drel_hi", [128, nwin * g_hi], gdt, kind="ExternalInput")
    wgt_hi_t = nc.dram_tensor("wgt_hi", [128, nwin * g_hi], gdt, kind="ExternalInput")
    iota_t = nc.dram_tensor("iota", [128, 128], gdt, kind="ExternalInput")
    w_t = nc.dram_tensor("wmat", [D, D], gdt, kind="ExternalInput")
    b_t = nc.dram_tensor("bcol", [D, 1], f32, kind="ExternalInput")
    out_t = nc.dram_tensor("outT", [D, npad], f32, kind="ExternalOutput")

    with tile.TileContext(nc) as tc:
        with ExitStack() as ctx:
            const = ctx.enter_context(tc.tile_pool(name="const", bufs=1))
            gpool = ctx.enter_context(tc.tile_pool(name="gather", bufs=6))
            spool = ctx.enter_context(tc.tile_pool(name="sel", bufs=3))
            opool = ctx.enter_context(tc.tile_pool(name="outsb", bufs=2))
            ps_agg = ctx.enter_context(
                tc.tile_pool(name="ps_agg", bufs=2, space="PSUM")
            )
            ps_out = ctx.enter_context(
                tc.tile_pool(name="ps_out", bufs=2, space="PSUM")
            )

            # resident constants / metadata
            idx_lo = const.tile(list(idx_lo_t.shape), i16)
            idx_hi = const.tile(list(idx_hi_t.shape), i16)
            drel_lo = const.tile(list(drel_lo_t.shape), gdt)
            wgt_lo = const.tile(list(wgt_lo_t.shape), gdt)
            drel_hi = const.tile(list(drel_hi_t.shape), gdt)
            wgt_hi = const.tile(list(wgt_hi_t.shape), gdt)
            iota = const.tile([128, 128], gdt)
            wmat = const.tile([D, D], gdt)
            bcol = const.tile([D, 1], f32)
            agg_all = const.tile([128, npad], gdt, tag="agg_all")

            for sb, dr in (
                (idx_lo, idx_lo_t), (idx_hi, idx_hi_t),
                (drel_lo, drel_lo_t), (wgt_lo, wgt_lo_t),
                (drel_hi, drel_hi_t), (wgt_hi, wgt_hi_t),
                (iota, iota_t), (wmat, w_t), (bcol, b_t),
            ):
                nc.sync.dma_start(sb[:], dr[:])

            h_lo = h_t[0:n_lo_rows, :]
            h_hi = h_t[n_lo_rows:n_src_rows, :] if n_hi_rows > 0 else None
            use_hi = h_hi is not None

            for wg in range(nwin):
                # gather this window's edges: one SWDGE call per chunk.
                # A call of k*128 idxs needs 8k+1 SWDGE ring entries; calls
                # with 97 entries (k=12) crash the exec unit on HW, k<=8 is
                # proven safe.
                # effective groups this window (shared across cores): groups
                # beyond the max valid count are pure padding -> not gathered,
                # not matmul'd. Every issued call is fully valid, so no tile
                # region is ever read without having been written.
                gwl = gw_lo[wg] if gw_lo else g_lo
                gwh = gw_hi[wg] if gw_hi else g_hi
                sp = os.environ.get("GCN_SP", "1") == "1"
                gtiles_lo = []
                for (c0, k) in ch_lo:
                    ke = min(max(gwl - c0, 0), k)
                    if ke == 0:
                        continue
                    gt = gpool.tile([128, ch_lo[0][1], 128], gdt, tag="glo")
                    col = (wg * g_lo + c0) * 8
                    nc.gpsimd.dma_gather(
                        gt[:, :ke, :], h_lo, idx_lo[:, col : col + ke * 8],
                        num_idxs=ke * 128, num_idxs_reg=ke * 128, elem_size=D,
                        single_packet=sp,
                    )
                    gtiles_lo.append((gt, c0, ke))
                gtiles_hi = []
                if use_hi:
                    for (c0, k) in ch_hi:
                        ke = min(max(gwh - c0, 0), k)
                        if ke == 0:
                            continue
                        gt = gpool.tile([128, ch_hi[0][1], 128], gdt, tag="ghi")
                        col = (wg * g_hi + c0) * 8
                        nc.gpsimd.dma_gather(
                            gt[:, :ke, :], h_hi, idx_hi[:, col : col + ke * 8],
                            num_idxs=ke * 128, num_idxs_reg=ke * 128, elem_size=D,
                            single_packet=sp,
                        )
                        gtiles_hi.append((gt, c0, ke))

                # S for the whole window in 2 DVE ops per stream:
                # S[p, j, n] = (n == drel[p, j]) * w[p, j] via step-0
                # broadcast APs on both operands.
                def build_s(meta_d, meta_w, g, tag):
                    s = spool.tile([128, g, 128], gdt, tag=tag)
                    sh = (128, g, 128)
                    c0m = wg * g
                    nc.vector.tensor_tensor(
                        s[:], iota[:, None, :].broadcast_to(sh),
                        meta_d[:, c0m : c0m + g, None].broadcast_to(sh),
                        mybir.AluOpType.is_equal,
                    )
                    nc.vector.tensor_tensor(
                        s[:], s[:],
                        meta_w[:, c0m : c0m + g, None].broadcast_to(sh),
                        mybir.AluOpType.mult,
                    )
                    return s

                s_lo = build_s(drel_lo, wgt_lo, g_lo, "slo")
                s_hi = build_s(drel_hi, wgt_hi, g_hi, "shi") if use_hi else None

                psum = ps_agg.tile([128, 128], f32, tag="psagg")
                n_groups = sum(k for _, _, k in gtiles_lo)
                n_groups += sum(k for _, _, k in gtiles_hi)
                k_idx = 0
                for (gt, c0, k), s_all in (
                    [(t, s_lo) for t in gtiles_lo]
                    + [(t, s_hi) for t in gtiles_hi]
                ):
                    for j in range(k):
                        nc.tensor.matmul(
                            psum[:], gt[:, j, :], s_all[:, c0 + j, :],
                            start=(k_idx == 0), stop=(k_idx == n_groups - 1),
                        )
                        k_idx += 1
                # aggT window -> SBUF (cast to gather dtype)
                nc.scalar.copy(agg_all[:, wg * WIN : (wg + 1) * WIN], psum[:])

            # out.T = W.T @ aggT + b, in 512-column chunks
            CH = 512
            for t0 in range(0, npad, CH):
                n = min(CH, npad - t0)
                po = ps_out.tile([128, CH], f32, tag="psout")
                nc.tensor.matmul(
                    po[:, :n], wmat[:], agg_all[:, t0 : t0 + n],
                    start=True, stop=True,
                )
                ob = opool.tile([128, CH], f32, tag="outsb")
                nc.scalar.add(ob[:, :n], po[:, :n], bcol[:])
                nc.sync.dma_start(out_t[:, t0 : t0 + n], ob[:, :n])

    nc.compile()
    return nc


def _make_in_maps(H, edge_src, edge_dst, edge_weight, W, b, per_core, nwin,
                  g_lo, g_hi, ch_lo, ch_hi):
    np_g = np.float16 if GDTYPE == "f16" else np.float32
    h_src = np.ascontiguousarray(H.astype(np_g))
    iota = np.tile(np.arange(128, dtype=np_g), (128, 1))
    wmat = np.ascontiguousarray(W.astype(np_g))
    bcol = np.ascontiguousarray(b.astype(np.float32).reshape(D, 1))
    in_maps = []
    for wins in per_core:
        idx_lo, drel_lo, wgt_lo = _device_arrays(wins, nwin, g_lo, 0, ch_lo, np_g)
        idx_hi, drel_hi, wgt_hi = _device_arrays(wins, nwin, g_hi, 1, ch_hi, np_g)
        in_maps.append(
            {
                "h_src": h_src,
                "idx_lo": idx_lo, "idx_hi": idx_hi,
                "drel_lo": drel_lo, "wgt_lo": wgt_lo,
                "drel_hi": drel_hi, "wgt_hi": wgt_hi,
                "iota": iota, "wmat": wmat, "bcol": bcol,
            }
        )
    return in_maps


def kernel(H, edge_src, edge_dst, edge_weight, W, b):
    global LAST_EXEC_NS
    from concourse import bass_utils

    H = np.asarray(H, dtype=np.float32)
    edge_src = np.asarray(edge_src, dtype=np.int32)
    edge_dst = np.asarray(edge_dst, dtype=np.int32)
    edge_weight = np.asarray(edge_weight, dtype=np.float32)
    W = np.asarray(W, dtype=np.float32)
    b = np.asarray(b, dtype=np.float32)

    per_core, nwin, g_lo, g_hi, g_all = _prep(H, edge_src, edge_dst, edge_weight)
    mode = os.environ.get("GCN_GATHER", "batched")
    if mode == "batched":
        batch = int(os.environ.get("GCN_BATCH", "4"))
        max_ke = int(os.environ.get("GCN_MAXKE", "63"))
        scratch = int(os.environ.get("GCN_SCRATCH", "32768"))
        gw_lo = []
        gw_hi = []
        for wi in range(nwin):
            m_lo = max(len(wins[wi][0][0]) for wins in per_core)
            m_hi = max(len(wins[wi][1][0]) for wins in per_core)
            gw_lo.append(min(g_lo, max(1, _ceil_div(m_lo, 128))))
            gw_hi.append(min(g_hi, max(1, _ceil_div(m_hi, 128))))
        hi_ind = os.environ.get("GCN_HI_IND", "0") == "1"
        nc, batches, lo_tot, hi_tot = _build_program_batched(
            nwin, g_lo, g_hi, gw_lo, gw_hi, N_NODES,
            batch=batch, max_ke=max_ke, scratch=scratch, hi_indirect=hi_ind,
        )
        np_g = np.float16 if GDTYPE == "f16" else np.float32
        h_src = np.ascontiguousarray(H.astype(np_g))
        iota = np.tile(np.arange(128, dtype=np_g), (128, 1))
        wmat = np.ascontiguousarray(W.astype(np_g))
        bcol = np.ascontiguousarray(b.astype(np.float32).reshape(D, 1))
        in_maps = []
        for wins in per_core:
            idx_lo, drel_lo, wgt_lo = _device_arrays_batched(
                wins, nwin, g_lo, 0, gw_lo, batches, np_g)
            idx_hi, drel_hi, wgt_hi = _device_arrays_batched(
                wins, nwin, g_hi, 1, gw_hi, batches, np_g,
                ind_offset=SPLIT if hi_ind else None)
            in_maps.append(
                {
                    "h_src": h_src,
                    "idx_lo": idx_lo, "idx_hi": idx_hi,
                    "drel_lo": drel_lo, "wgt_lo": wgt_lo,
                    "drel_hi": drel_hi, "wgt_hi": wgt_hi,
                    "iota": iota, "wmat": wmat, "bcol": bcol,
                }
            )
    elif mode == "indirect":
        maxg = int(os.environ.get("GCN_MAXG", "8"))
        ch = _chunks(g_all, maxg)
        nc = _build_program_ind(nwin, g_all, ch, N_NODES)
        np_g = np.float16 if GDTYPE == "f16" else np.float32
        h_src = np.ascontiguousarray(H.astype(np_g))
        iota = np.tile(np.arange(128, dtype=np_g), (128, 1))
        wmat = np.ascontiguousarray(W.astype(np_g))
        bcol = np.ascontiguousarray(b.astype(np.float32).reshape(D, 1))
        in_maps = []
        for wins in per_core:
            idx_all, drel, wgt = _device_arrays_ind(wins, nwin, g_all, np_g)
            in_maps.append({
                "h_src": h_src, "idx_all": idx_all, "drel": drel, "wgt": wgt,
                "iota": iota, "wmat": wmat, "bcol": bcol,
            })
    else:
        maxg = int(os.environ.get("GCN_MAXG", "8"))
        # round group counts up so every chunk has equal size: a gather tile
        # slot must always be written over its FULL extent when fully valid,
        # otherwise a smaller earlier write leaves never-written (non-finite)
        # columns that a later partially-valid call exposes to the matmul.
        g_lo = _ceil_div(g_lo, maxg) * maxg if g_lo > maxg else g_lo
        g_hi = _ceil_div(g_hi, maxg) * maxg if g_hi > maxg else g_hi
        ch_lo = _chunks(g_lo, maxg)
        ch_hi = _chunks(g_hi, maxg)
        assert len({k for _, k in ch_lo}) == 1 and len({k for _, k in ch_hi}) == 1
        # per-window effective group counts (shared across cores): only
        # gather/matmul groups that contain at least one real edge on the
        # max-count core; the rest are pure padding.
        trim = os.environ.get("GCN_TRIM", "1") == "1"
        gw_lo = []
        gw_hi = []
        for wi in range(nwin):
            m_lo = max(len(wins[wi][0][0]) for wins in per_core)
            m_hi = max(len(wins[wi][1][0]) for wins in per_core)
            gw_lo.append(min(g_lo, max(1, _ceil_div(m_lo, 128))) if trim else g_lo)
            gw_hi.append(min(g_hi, max(1, _ceil_div(m_hi, 128))) if trim else g_hi)
        nc = _build_program(nwin, g_lo, g_hi, ch_lo, ch_hi, N_NODES,
                            gw_lo=gw_lo, gw_hi=gw_hi)
        in_maps = _make_in_maps(
            H, edge_src, edge_dst, edge_weight, W, b, per_core, nwin, g_lo,
            g_hi, ch_lo, ch_hi,
        )

    if os.environ.get("GCN_SIM", "0") == "1":  # CoreSim path for testing
        from concourse.bass_interp import CoreSim

        out = np.empty((N_NODES, D), np.float32)
        for c in range(N_CORES):
            sim = CoreSim(nc)
            for k2, v2 in in_maps[c].items():
                sim.tensor(k2)[:] = v2
            sim.simulate()
            out[c * NPW : (c + 1) * NPW, :] = np.array(
                sim.tensor("outT")).T[:NPW]
        return out

    trace = os.environ.get("GCN_TRACE", "0") == "1"
    kw = {}
    if trace:
        import shutil
        td = "/tmp/gcn_ntff"
        shutil.rmtree(td, ignore_errors=True)
        os.makedirs(td, exist_ok=True)
        kw["tmpdir"] = td
    # a previously crashed NEFF can leave the exec unit transiently
    # unrecoverable; recovery has been observed to take up to a few minutes,
    # so retry with escalating backoff
    import time as _time
    last_err = None
    for backoff in (15, 45, 90, 0):
        try:
            res = bass_utils.run_bass_kernel_spmd(
                nc, in_maps, core_ids=list(range(N_CORES)), trace=trace, **kw
            )
            break
        except Exception as e:
            last_err = e
            if backoff:
                _time.sleep(backoff)
    else:
        raise last_err
    LAST_EXEC_NS = res.exec_time_ns
    global LAST_RESULTS
    LAST_RESULTS = res

    out = np.empty((N_NODES, D), np.float32)
    for c in range(N_CORES):
        outT = res.results[c]["outT"]
        out[c * NPW : (c + 1) * NPW, :] = outT.T[:NPW]
    return out



# revision 19
# speedup vs baseline: 2.4072x; 2.1764x over previous
"""GCN layer (gather -> weighted scatter-sum -> dense transform) on 8 trn2 cores.

Default path (GCN_GATHER=batched): same algorithm as the per-window path
below, but gather calls are batched — one dma_gather per (4-window batch,
stream) with single_packet=False (multi-packet rings; calls may exceed the
65-descriptor single-packet limit) and idx columns packed dense (per-window
trimmed group counts). Performance notes (HW-measured):
  - SWDGE dma_gather costs ~5.5-7.9 ns/idx, engine-serial on the Q7 pair;
    with per-desc packets (single_packet=False) the SDMA drain of 256B
    descriptors (~125ns/desc/engine) binds at ~7.8ns/idx. All SWDGE
    configurations converge to ~1.66-1.71ms for the ~212k idxs/core.
  - gpsimd.indirect_dma_start is NOT usable: on HW it lands rows on
    partition 0 only (CoreSim models it differently) and signals no DMA
    completion semaphores.
  - SBUF-source dma_gather (transpose=True) crashes the exec unit
    (NRT_EXEC_UNIT_UNRECOVERABLE).

Strategy (1-D row partitioning of destination nodes):
  - Core c owns destination nodes [c*NPW, (c+1)*NPW). edge_dst is sorted, so
    each core's edges are a contiguous slice of the edge list.
  - Within a core, dst nodes are processed in windows of 128 (the PSUM
    partition size). Every window's edges are padded to a fixed number of
    128-edge groups so all 8 cores run the same program.
  - Per 128-edge group:
      * dma_gather pulls the 128 source rows H[src] (fp16) from HBM into an
        SBUF tile G [128 edges x 128 feat] (edge e=j*128+p lands on
        partition p, slot j).
      * DVE builds S [128 edges x 128 nodes] = (iota == dstrel) * w with one
        fused tensor_scalar op.
      * TensorE accumulates aggT[feat, node] += G.T @ S in PSUM.
  - dma_gather indices are int16 (< 32768), so edges are split into a "lo"
    stream (src < 32768, gathered from H[:32768]) and a "hi" stream
    (src >= 32768, gathered from H[32768:]); both accumulate into the same
    PSUM window.
  - Final transform: out.T = W.T @ aggT (+ b) with W stationary, computed in
    512-column chunks; bias is added during the PSUM->SBUF copy (per-partition
    ACT bias, since the output is transposed: partitions = out features).
    The kernel writes out.T [128, NWIN*128] per core; the host transposes and
    concatenates.
"""

import os
import numpy as np

N_CORES = 8
N_NODES = 50000
D = 128
NPW = N_NODES // N_CORES  # 6250 dst nodes per core
WIN = 128
SPLIT = 32768  # int16-addressable row limit for dma_gather

# gather dtype: "f16" (half gather traffic, rel err ~3e-4) or "f32" (exact)
GDTYPE = os.environ.get("GCN_GDTYPE", "f16")

LAST_EXEC_NS = None  # set when GCN_TRACE=1
LAST_RESULTS = None


def _ceil_div(a, b):
    return -(-a // b)


def _prep(H, edge_src, edge_dst, edge_weight, n_cores=N_CORES):
    """Host-side sharding: per-core, per-window, per-stream edge lists with
    padding to common sizes. Returns per-core arrays + common geometry."""
    nwin = _ceil_div(NPW, WIN)
    # per (core, window, stream) edge index lists
    per_core = []
    max_lo = 0
    max_hi = 0
    max_all = 0
    for c in range(n_cores):
        n0, n1 = c * NPW, (c + 1) * NPW
        e0, e1 = np.searchsorted(edge_dst, [n0, n1])
        d = edge_dst[e0:e1] - n0
        s = edge_src[e0:e1]
        w = edge_weight[e0:e1]
        wins = []
        for wi in range(nwin):
            i0, i1 = np.searchsorted(d, [wi * WIN, wi * WIN + WIN])
            sw, dw, ww = s[i0:i1], d[i0:i1] - wi * WIN, w[i0:i1]
            lo = sw < SPLIT
            wins.append(
                (
                    (sw[lo], dw[lo], ww[lo]),
                    (sw[~lo] - SPLIT, dw[~lo], ww[~lo]),
                    (sw, dw, ww),
                )
            )
            max_lo = max(max_lo, int(lo.sum()))
            max_hi = max(max_hi, int((~lo).sum()))
            max_all = max(max_all, int(i1 - i0))
        per_core.append(wins)
    g_lo = max(1, _ceil_div(max_lo, 128))
    g_hi = max(1, _ceil_div(max_hi, 128))
    g_all = max(1, _ceil_div(max_all, 128))
    return per_core, nwin, g_lo, g_hi, g_all


def _chunks(g, maxg):
    """Split g groups into near-even chunks of <= maxg groups."""
    n = _ceil_div(g, maxg)
    base, rem = divmod(g, n)
    out = []
    c0 = 0
    for i in range(n):
        k = base + (1 if i < rem else 0)
        out.append((c0, k))
        c0 += k
    return out


def _device_arrays(wins, nwin, g, stream, chunks, np_meta_dtype, regs=None):
    """Build idx (wrapped-16 per gather call; call = (window, chunk)) +
    dstrel/weight arrays for one core and one stream ('lo'=0, 'hi'=1).

    regs is unused (kept for signature compat)."""
    ne = g * 128
    idx = np.zeros((nwin, ne), np.int16)
    drel = np.zeros((nwin, ne), np_meta_dtype)
    wgt = np.zeros((nwin, ne), np_meta_dtype)
    for wi in range(nwin):
        sw, dw, ww = wins[wi][stream]
        n = len(sw)
        idx[wi, :n] = sw.astype(np.int16)
        drel[wi, :n] = dw.astype(np_meta_dtype)
        wgt[wi, :n] = ww.astype(np_meta_dtype)
    parts = []
    for wi in range(nwin):
        for (c0, k) in chunks:
            flat = idx[wi, c0 * 128 : (c0 + k) * 128]
            parts.append(flat.reshape(-1, 16).T)  # [16, k*8]
    idx_dev = np.tile(np.concatenate(parts, axis=1), (8, 1))  # [128, nwin*g*8]
    # meta: [p, w*g + j] = value of edge e=j*128+p in window w
    drel_dev = np.ascontiguousarray(
        drel.reshape(nwin, g, 128).transpose(2, 0, 1).reshape(128, nwin * g)
    )
    wgt_dev = np.ascontiguousarray(
        wgt.reshape(nwin, g, 128).transpose(2, 0, 1).reshape(128, nwin * g)
    )
    return idx_dev, drel_dev, wgt_dev


def _device_arrays_ind(wins, nwin, g, np_meta_dtype):
    """idx (int32, natural [p, w*g+j] layout) + dstrel/weight arrays for the
    combined stream (indirect_dma_start variant)."""
    ne = g * 128
    idx = np.zeros((nwin, ne), np.int32)
    drel = np.zeros((nwin, ne), np_meta_dtype)
    wgt = np.zeros((nwin, ne), np_meta_dtype)
    for wi in range(nwin):
        sw, dw, ww = wins[wi][2]
        n = len(sw)
        idx[wi, :n] = sw
        drel[wi, :n] = dw.astype(np_meta_dtype)
        wgt[wi, :n] = ww.astype(np_meta_dtype)

    def dev(a):
        return np.ascontiguousarray(
            a.reshape(nwin, g, 128).transpose(2, 0, 1).reshape(128, nwin * g)
        )

    return dev(idx), dev(drel), dev(wgt)


def _build_program_ind(nwin, g_all, ch, n_src_rows, n_cores=N_CORES):
    """Indirect_dma_start variant: int32 indices, no lo/hi split."""
    from contextlib import ExitStack

    import concourse.bass as bass
    import concourse.tile as tile
    from concourse import bacc, mybir

    f32 = mybir.dt.float32
    gdt = mybir.dt.float16 if GDTYPE == "f16" else mybir.dt.float32
    i32 = mybir.dt.int32

    nc = bacc.Bacc(
        "TRN2", target_bir_lowering=False, debug=False, num_devices=n_cores,
    )

    npad = nwin * WIN
    h_t = nc.dram_tensor("h_src", [n_src_rows, D], gdt, kind="ExternalInput")
    idx_t = nc.dram_tensor("idx_all", [128, nwin * g_all], i32, kind="ExternalInput")
    drel_t = nc.dram_tensor("drel", [128, nwin * g_all], gdt, kind="ExternalInput")
    wgt_t = nc.dram_tensor("wgt", [128, nwin * g_all], gdt, kind="ExternalInput")
    iota_t = nc.dram_tensor("iota", [128, 128], gdt, kind="ExternalInput")
    w_t = nc.dram_tensor("wmat", [D, D], gdt, kind="ExternalInput")
    b_t = nc.dram_tensor("bcol", [D, 1], f32, kind="ExternalInput")
    out_t = nc.dram_tensor("outT", [D, npad], f32, kind="ExternalOutput")

    with tile.TileContext(nc) as tc:
        with ExitStack() as ctx:
            const = ctx.enter_context(tc.tile_pool(name="const", bufs=1))
            gpool = ctx.enter_context(tc.tile_pool(name="gather", bufs=6))
            spool = ctx.enter_context(tc.tile_pool(name="sel", bufs=3))
            opool = ctx.enter_context(tc.tile_pool(name="outsb", bufs=2))
            ps_agg = ctx.enter_context(tc.tile_pool(name="ps_agg", bufs=2, space="PSUM"))
            ps_out = ctx.enter_context(tc.tile_pool(name="ps_out", bufs=2, space="PSUM"))

            idx = const.tile(list(idx_t.shape), i32)
            drel = const.tile(list(drel_t.shape), gdt)
            wgt = const.tile(list(wgt_t.shape), gdt)
            iota = const.tile([128, 128], gdt)
            wmat = const.tile([D, D], gdt)
            bcol = const.tile([D, 1], f32)
            agg_all = const.tile([128, npad], gdt, tag="agg_all")

            for sb, dr in ((idx, idx_t), (drel, drel_t), (wgt, wgt_t),
                           (iota, iota_t), (wmat, w_t), (bcol, b_t)):
                nc.sync.dma_start(sb[:], dr[:])

            for wg in range(nwin):
                gtiles = []
                for (c0, k) in ch:
                    gt = gpool.tile([128, ch[0][1], 128], gdt, tag="g")
                    nc.gpsimd.indirect_dma_start(
                        out=gt[:, :k, :],
                        out_offset=None,
                        in_=h_t[:],
                        in_offset=bass.IndirectOffsetOnAxis(
                            ap=idx[:, wg * g_all + c0 : wg * g_all + c0 + k],
                            axis=0,
                        ),
                    )
                    gtiles.append((gt, c0, k))

                sh = (128, g_all, 128)
                c0m = wg * g_all
                s = spool.tile([128, g_all, 128], gdt, tag="sel")
                nc.vector.tensor_tensor(
                    s[:], iota[:, None, :].broadcast_to(sh),
                    drel[:, c0m : c0m + g_all, None].broadcast_to(sh),
                    mybir.AluOpType.is_equal,
                )
                nc.vector.tensor_tensor(
                    s[:], s[:], wgt[:, c0m : c0m + g_all, None].broadcast_to(sh),
                    mybir.AluOpType.mult,
                )

                psum = ps_agg.tile([128, 128], f32, tag="psagg")
                k_idx = 0
                for (gt, c0, k) in gtiles:
                    for j in range(k):
                        nc.tensor.matmul(
                            psum[:], gt[:, j, :], s[:, c0 + j, :],
                            start=(k_idx == 0), stop=(k_idx == g_all - 1),
                        )
                        k_idx += 1
                nc.scalar.copy(agg_all[:, wg * WIN : (wg + 1) * WIN], psum[:])

            CH = 512
            for t0 in range(0, npad, CH):
                n = min(CH, npad - t0)
                po = ps_out.tile([128, CH], f32, tag="psout")
                nc.tensor.matmul(
                    po[:, :n], wmat[:], agg_all[:, t0 : t0 + n],
                    start=True, stop=True,
                )
                ob = opool.tile([128, CH], f32, tag="outsb")
                nc.scalar.add(ob[:, :n], po[:, :n], bcol[:])
                nc.sync.dma_start(out_t[:, t0 : t0 + n], ob[:, :n])

    nc.compile()
    return nc


def _build_program_batched(nwin, g_lo, g_hi, gw_lo, gw_hi, n_src_rows,
                           batch=2, max_ke=63, n_cores=N_CORES,
                           scratch=32768, hi_indirect=False):
    """Batched-call variant: one dma_gather call per (batch of windows,
    stream), idx columns packed dense (only the first gw[w] groups of each
    window are gathered). single_packet=False so calls may exceed 65 ring
    descriptors; scratch sized so ring (scratch/64 descs) >= 8*max_ke+1."""
    from contextlib import ExitStack

    import concourse.bass as bass
    import concourse.tile as tile
    from concourse import bacc, mybir

    f32 = mybir.dt.float32
    gdt = mybir.dt.float16 if GDTYPE == "f16" else mybir.dt.float32
    i16 = mybir.dt.int16

    nq = int(os.environ.get("GCN_NQ", "4"))
    nc = bacc.Bacc(
        "TRN2", target_bir_lowering=False, debug=False, num_devices=n_cores,
        dynamic_dma_scratch_size=scratch, num_swdge_queues=nq,
    )

    npad = nwin * WIN
    n_lo_rows = min(SPLIT, n_src_rows)
    n_hi_rows = n_src_rows - n_lo_rows

    batches = [list(range(b, min(b + batch, nwin))) for b in range(0, nwin, batch)]
    # per-batch dense group counts and per-window offsets
    lo_off = {}
    hi_off = {}
    lo_tot = []
    hi_tot = []
    for bi, ws in enumerate(batches):
        o = 0
        for w in ws:
            lo_off[w] = o
            o += gw_lo[w]
        lo_tot.append(o)
        o = 0
        for w in ws:
            hi_off[w] = o
            o += gw_hi[w]
        hi_tot.append(o)
    glo_max = max(lo_tot)
    ghi_max = max(hi_tot)
    ncol_lo = sum(lo_tot)
    ncol_hi = sum(hi_tot)

    h_t = nc.dram_tensor("h_src", [n_src_rows, D], gdt, kind="ExternalInput")
    idx_lo_t = nc.dram_tensor("idx_lo", [128, ncol_lo * 8], i16, kind="ExternalInput")
    if hi_indirect:
        idx_hi_t = nc.dram_tensor(
            "idx_hi", [128, ncol_hi], mybir.dt.int32, kind="ExternalInput")
    else:
        idx_hi_t = nc.dram_tensor(
            "idx_hi", [128, ncol_hi * 8], i16, kind="ExternalInput")
    drel_lo_t = nc.dram_tensor("drel_lo", [128, nwin * g_lo], gdt, kind="ExternalInput")
    wgt_lo_t = nc.dram_tensor("wgt_lo", [128, nwin * g_lo], gdt, kind="ExternalInput")
    drel_hi_t = nc.dram_tensor("drel_hi", [128, nwin * g_hi], gdt, kind="ExternalInput")
    wgt_hi_t = nc.dram_tensor("wgt_hi", [128, nwin * g_hi], gdt, kind="ExternalInput")
    iota_t = nc.dram_tensor("iota", [128, 128], gdt, kind="ExternalInput")
    w_t = nc.dram_tensor("wmat", [D, D], gdt, kind="ExternalInput")
    b_t = nc.dram_tensor("bcol", [D, 1], f32, kind="ExternalInput")
    out_t = nc.dram_tensor("outT", [D, npad], f32, kind="ExternalOutput")

    with tile.TileContext(nc) as tc:
        with ExitStack() as ctx:
            const = ctx.enter_context(tc.tile_pool(name="const", bufs=1))
            gpool = ctx.enter_context(tc.tile_pool(name="gather", bufs=2))
            spool = ctx.enter_context(tc.tile_pool(name="sel", bufs=3))
            opool = ctx.enter_context(tc.tile_pool(name="outsb", bufs=2))
            ps_agg = ctx.enter_context(tc.tile_pool(name="ps_agg", bufs=2, space="PSUM"))
            ps_out = ctx.enter_context(tc.tile_pool(name="ps_out", bufs=2, space="PSUM"))

            idx_lo = const.tile(list(idx_lo_t.shape), i16)
            idx_hi = const.tile(
                list(idx_hi_t.shape),
                mybir.dt.int32 if hi_indirect else i16)
            drel_lo = const.tile(list(drel_lo_t.shape), gdt)
            wgt_lo = const.tile(list(wgt_lo_t.shape), gdt)
            drel_hi = const.tile(list(drel_hi_t.shape), gdt)
            wgt_hi = const.tile(list(wgt_hi_t.shape), gdt)
            iota = const.tile([128, 128], gdt)
            wmat = const.tile([D, D], gdt)
            bcol = const.tile([D, 1], f32)
            agg_all = const.tile([128, npad], gdt, tag="agg_all")

            for sb, dr in (
                (idx_lo, idx_lo_t), (idx_hi, idx_hi_t),
                (drel_lo, drel_lo_t), (wgt_lo, wgt_lo_t),
                (drel_hi, drel_hi_t), (wgt_hi, wgt_hi_t),
                (iota, iota_t), (wmat, w_t), (bcol, b_t),
            ):
                nc.sync.dma_start(sb[:], dr[:])

            h_lo = h_t[0:n_lo_rows, :]
            h_hi = h_t[n_lo_rows:n_src_rows, :] if n_hi_rows > 0 else None

            col_lo = 0
            col_hi = 0
            qn = [0]  # round-robin SWDGE queue counter (one ring set per Q7 pair)

            def next_q():
                q = qn[0] % nq
                qn[0] += 1
                return q

            for bi, ws in enumerate(batches):
                # one gather call per stream per batch (split at max_ke)
                bsp = os.environ.get("GCN_BSP", "0") == "1"
                glo = gpool.tile([128, glo_max, 128], gdt, tag="glo")
                c0 = 0
                while c0 < lo_tot[bi]:
                    ke = min(max_ke, lo_tot[bi] - c0)
                    nc.gpsimd.dma_gather(
                        glo[:, c0 : c0 + ke, :], h_lo,
                        idx_lo[:, (col_lo + c0) * 8 : (col_lo + c0 + ke) * 8],
                        num_idxs=ke * 128, num_idxs_reg=ke * 128, elem_size=D,
                        single_packet=bsp, queue_num=next_q(),
                    )
                    c0 += ke
                ghi = gpool.tile([128, ghi_max, 128], gdt, tag="ghi")
                marker = None
                if hi_indirect:
                    nc.gpsimd.indirect_dma_start(
                        out=ghi[:, : hi_tot[bi], :],
                        out_offset=None,
                        in_=h_t[:],
                        in_offset=bass.IndirectOffsetOnAxis(
                            ap=idx_hi[:, col_hi : col_hi + hi_tot[bi]],
                            axis=0,
                        ),
                    )
                    # FIFO-ordering completion marker: a tiny SWDGE gather on
                    # the same queue whose (working) DMA semaphore fires only
                    # after the ring has drained past the indirect's
                    # descriptors. Every hi matmul is made to depend on it via
                    # a bypass op over s_hi.
                    marker = gpool.tile([128, 1, 128], gdt, tag="mrk")
                    nc.gpsimd.dma_gather(
                        marker[:, :1, :], h_lo, idx_lo[:, 0:8],
                        num_idxs=128, num_idxs_reg=128, elem_size=D,
                        single_packet=False,
                    )
                else:
                    c0 = 0
                    while c0 < hi_tot[bi]:
                        ke = min(max_ke, hi_tot[bi] - c0)
                        nc.gpsimd.dma_gather(
                            ghi[:, c0 : c0 + ke, :], h_hi,
                            idx_hi[:, (col_hi + c0) * 8 : (col_hi + c0 + ke) * 8],
                            num_idxs=ke * 128, num_idxs_reg=ke * 128, elem_size=D,
                            single_packet=bsp, queue_num=next_q(),
                        )
                        c0 += ke
                col_lo += lo_tot[bi]
                col_hi += hi_tot[bi]

                for wg in ws:
                    def build_s(meta_d, meta_w, g, tag):
                        s = spool.tile([128, g, 128], gdt, tag=tag)
                        sh = (128, g, 128)
                        c0m = wg * g
                        nc.vector.tensor_tensor(
                            s[:], iota[:, None, :].broadcast_to(sh),
                            meta_d[:, c0m : c0m + g, None].broadcast_to(sh),
                            mybir.AluOpType.is_equal,
                        )
                        nc.vector.tensor_tensor(
                            s[:], s[:],
                            meta_w[:, c0m : c0m + g, None].broadcast_to(sh),
                            mybir.AluOpType.mult,
                        )
                        return s

                    s_lo = build_s(drel_lo, wgt_lo, g_lo, "slo")
                    s_hi = build_s(drel_hi, wgt_hi, g_hi, "shi") if h_hi is not None else None
                    if s_hi is not None and marker is not None:
                        # bypass: out = in0 (s_hi unchanged) but creates a dep
                        # on the marker tile for every s_hi subtile.
                        nc.vector.tensor_tensor(
                            s_hi[:], s_hi[:],
                            marker[:, 0, None, 0:1].broadcast_to((128, g_hi, 128)),
                            mybir.AluOpType.bypass,
                        )

                    psum = ps_agg.tile([128, 128], f32, tag="psagg")
                    n_groups = gw_lo[wg] + (gw_hi[wg] if s_hi is not None else 0)
                    k_idx = 0
                    for j in range(gw_lo[wg]):
                        nc.tensor.matmul(
                            psum[:], glo[:, lo_off[wg] + j, :], s_lo[:, j, :],
                            start=(k_idx == 0), stop=(k_idx == n_groups - 1),
                        )
                        k_idx += 1
                    if s_hi is not None:
                        for j in range(gw_hi[wg]):
                            nc.tensor.matmul(
                                psum[:], ghi[:, hi_off[wg] + j, :], s_hi[:, j, :],
                                start=(k_idx == 0), stop=(k_idx == n_groups - 1),
                            )
                            k_idx += 1
                    nc.scalar.copy(agg_all[:, wg * WIN : (wg + 1) * WIN], psum[:])

                    # incremental output transform: as soon as a 512-col chunk
                    # of agg_all is complete, run W.T @ chunk so the tail
                    # after the last gather is just one chunk.
                    CH = 512
                    done = (wg + 1) * WIN
                    t0 = (done // CH - 1) * CH
                    if t0 >= 0 and done % CH == 0:
                        n = min(CH, npad - t0)
                        po = ps_out.tile([128, CH], f32, tag="psout")
                        nc.tensor.matmul(
                            po[:, :n], wmat[:], agg_all[:, t0 : t0 + n],
                            start=True, stop=True,
                        )
                        ob = opool.tile([128, CH], f32, tag="outsb")
                        nc.scalar.add(ob[:, :n], po[:, :n], bcol[:])
                        nc.sync.dma_start(out_t[:, t0 : t0 + n], ob[:, :n])

            CH = 512
            for t0 in range((npad // CH) * CH - (CH if npad % CH == 0 else 0),
                            npad, CH):
                if t0 < 0:
                    continue
                n = min(CH, npad - t0)
                if n <= 0:
                    continue
                po = ps_out.tile([128, CH], f32, tag="psout")
                nc.tensor.matmul(
                    po[:, :n], wmat[:], agg_all[:, t0 : t0 + n],
                    start=True, stop=True,
                )
                ob = opool.tile([128, CH], f32, tag="outsb")
                nc.scalar.add(ob[:, :n], po[:, :n], bcol[:])
                nc.sync.dma_start(out_t[:, t0 : t0 + n], ob[:, :n])

    nc.compile()
    return nc, batches, lo_tot, hi_tot


def _device_arrays_batched(wins, nwin, g, stream, gw, batches, np_meta_dtype,
                           ind_offset=None):
    """Dense-packed idx array (wrapped-16 per batch-call column blocks) plus
    per-window padded drel/wgt arrays (same layout as _device_arrays).

    ind_offset: if not None, build int32 indirect-layout idxs ([128, ncol]
    natural [p, col] order, absolute row ids = stored + ind_offset)."""
    ne = g * 128
    idx = np.zeros((nwin, ne), np.int32 if ind_offset is not None else np.int16)
    drel = np.zeros((nwin, ne), np_meta_dtype)
    wgt = np.zeros((nwin, ne), np_meta_dtype)
    for wi in range(nwin):
        sw, dw, ww = wins[wi][stream]
        n = len(sw)
        if ind_offset is not None:
            idx[wi, :n] = sw.astype(np.int32) + ind_offset
        else:
            idx[wi, :n] = sw.astype(np.int16)
        drel[wi, :n] = dw.astype(np_meta_dtype)
        wgt[wi, :n] = ww.astype(np_meta_dtype)
    if ind_offset is not None:
        # [p, dense col] where col runs over (batch, window, group j<gw[w])
        parts = []
        for ws in batches:
            for wi in ws:
                parts.append(idx[wi, : gw[wi] * 128].reshape(gw[wi], 128).T)
        idx_dev = np.ascontiguousarray(np.concatenate(parts, axis=1))
    else:
        # wrapped in 16 partitions, tiled x8 (SWDGE layout)
        parts = []
        for ws in batches:
            for wi in ws:
                flat = idx[wi, : gw[wi] * 128]
                parts.append(flat.reshape(-1, 16).T)  # [16, gw*8]
        idx_dev = np.tile(np.concatenate(parts, axis=1), (8, 1))
    drel_dev = np.ascontiguousarray(
        drel.reshape(nwin, g, 128).transpose(2, 0, 1).reshape(128, nwin * g)
    )
    wgt_dev = np.ascontiguousarray(
        wgt.reshape(nwin, g, 128).transpose(2, 0, 1).reshape(128, nwin * g)
    )
    return idx_dev, drel_dev, wgt_dev


def _build_program(nwin, g_lo, g_hi, ch_lo, ch_hi, n_src_rows, n_cores=N_CORES,
                   gw_lo=None, gw_hi=None):
    """Trace the (single, SPMD-shared) Bass program."""
    from contextlib import ExitStack

    import concourse.bass as bass
    import concourse.tile as tile
    from concourse import bacc, mybir

    f32 = mybir.dt.float32
    gdt = mybir.dt.float16 if GDTYPE == "f16" else mybir.dt.float32
    i16 = mybir.dt.int16

    nc = bacc.Bacc(
        "TRN2",
        target_bir_lowering=False,
        debug=False,
        num_devices=n_cores,
    )

    npad = nwin * WIN
    n_lo_rows = min(SPLIT, n_src_rows)
    n_hi_rows = n_src_rows - n_lo_rows

    h_t = nc.dram_tensor("h_src", [n_src_rows, D], gdt, kind="ExternalInput")
    idx_lo_t = nc.dram_tensor(
        "idx_lo", [128, nwin * g_lo * 8], i16, kind="ExternalInput",
    )
    idx_hi_t = nc.dram_tensor(
        "idx_hi", [128, nwin * g_hi * 8], i16, kind="ExternalInput",
    )
    drel_lo_t = nc.dram_tensor("drel_lo", [128, nwin * g_lo], gdt, kind="ExternalInput")
    wgt_lo_t = nc.dram_tensor("wgt_lo", [128, nwin * g_lo], gdt, kind="ExternalInput")
    drel_hi_t = nc.dram_tensor("drel_hi", [128, nwin * g_hi], gdt, kind="ExternalInput")
    wgt_hi_t = nc.dram_tensor("wgt_hi", [128, nwin * g_hi], gdt, kind="ExternalInput")
    iota_t = nc.dram_tensor("iota", [128, 128], gdt, kind="ExternalInput")
    w_t = nc.dram_tensor("wmat", [D, D], gdt, kind="ExternalInput")
    b_t = nc.dram_tensor("bcol", [D, 1], f32, kind="ExternalInput")
    out_t = nc.dram_tensor("outT", [D, npad], f32, kind="ExternalOutput")

    with tile.TileContext(nc) as tc:
        with ExitStack() as ctx:
            const = ctx.enter_context(tc.tile_pool(name="const", bufs=1))
            gpool = ctx.enter_context(tc.tile_pool(name="gather", bufs=6))
            spool = ctx.enter_context(tc.tile_pool(name="sel", bufs=3))
            opool = ctx.enter_context(tc.tile_pool(name="outsb", bufs=2))
            ps_agg = ctx.enter_context(
                tc.tile_pool(name="ps_agg", bufs=2, space="PSUM")
            )
            ps_out = ctx.enter_context(
                tc.tile_pool(name="ps_out", bufs=2, space="PSUM")
            )

            # resident constants / metadata
            idx_lo = const.tile(list(idx_lo_t.shape), i16)
            idx_hi = const.tile(list(idx_hi_t.shape), i16)
            drel_lo = const.tile(list(drel_lo_t.shape), gdt)
            wgt_lo = const.tile(list(wgt_lo_t.shape), gdt)
            drel_hi = const.tile(list(drel_hi_t.shape), gdt)
            wgt_hi = const.tile(list(wgt_hi_t.shape), gdt)
            iota = const.tile([128, 128], gdt)
            wmat = const.tile([D, D], gdt)
            bcol = const.tile([D, 1], f32)
            agg_all = const.tile([128, npad], gdt, tag="agg_all")

            for sb, dr in (
                (idx_lo, idx_lo_t), (idx_hi, idx_hi_t),
                (drel_lo, drel_lo_t), (wgt_lo, wgt_lo_t),
                (drel_hi, drel_hi_t), (wgt_hi, wgt_hi_t),
                (iota, iota_t), (wmat, w_t), (bcol, b_t),
            ):
                nc.sync.dma_start(sb[:], dr[:])

            h_lo = h_t[0:n_lo_rows, :]
            h_hi = h_t[n_lo_rows:n_src_rows, :] if n_hi_rows > 0 else None
            use_hi = h_hi is not None

            for wg in range(nwin):
                # gather this window's edges: one SWDGE call per chunk.
                # A call of k*128 idxs needs 8k+1 SWDGE ring entries; calls
                # with 97 entries (k=12) crash the exec unit on HW, k<=8 is
                # proven safe.
                # effective groups this window (shared across cores): groups
                # beyond the max valid count are pure padding -> not gathered,
                # not matmul'd. Every issued call is fully valid, so no tile
                # region is ever read without having been written.
                gwl = gw_lo[wg] if gw_lo else g_lo
                gwh = gw_hi[wg] if gw_hi else g_hi
                sp = os.environ.get("GCN_SP", "1") == "1"
                gtiles_lo = []
                for (c0, k) in ch_lo:
                    ke = min(max(gwl - c0, 0), k)
                    if ke == 0:
                        continue
                    gt = gpool.tile([128, ch_lo[0][1], 128], gdt, tag="glo")
                    col = (wg * g_lo + c0) * 8
                    nc.gpsimd.dma_gather(
                        gt[:, :ke, :], h_lo, idx_lo[:, col : col + ke * 8],
                        num_idxs=ke * 128, num_idxs_reg=ke * 128, elem_size=D,
                        single_packet=sp,
                    )
                    gtiles_lo.append((gt, c0, ke))
                gtiles_hi = []
                if use_hi:
                    for (c0, k) in ch_hi:
                        ke = min(max(gwh - c0, 0), k)
                        if ke == 0:
                            continue
                        gt = gpool.tile([128, ch_hi[0][1], 128], gdt, tag="ghi")
                        col = (wg * g_hi + c0) * 8
                        nc.gpsimd.dma_gather(
                            gt[:, :ke, :], h_hi, idx_hi[:, col : col + ke * 8],
                            num_idxs=ke * 128, num_idxs_reg=ke * 128, elem_size=D,
                            single_packet=sp,
                        )
                        gtiles_hi.append((gt, c0, ke))

                # S for the whole window in 2 DVE ops per stream:
                # S[p, j, n] = (n == drel[p, j]) * w[p, j] via step-0
                # broadcast APs on both operands.
                def build_s(meta_d, meta_w, g, tag):
                    s = spool.tile([128, g, 128], gdt, tag=tag)
                    sh = (128, g, 128)
                    c0m = wg * g
                    nc.vector.tensor_tensor(
                        s[:], iota[:, None, :].broadcast_to(sh),
                        meta_d[:, c0m : c0m + g, None].broadcast_to(sh),
                        mybir.AluOpType.is_equal,
                    )
                    nc.vector.tensor_tensor(
                        s[:], s[:],
                        meta_w[:, c0m : c0m + g, None].broadcast_to(sh),
                        mybir.AluOpType.mult,
                    )
                    return s

                s_lo = build_s(drel_lo, wgt_lo, g_lo, "slo")
                s_hi = build_s(drel_hi, wgt_hi, g_hi, "shi") if use_hi else None

                psum = ps_agg.tile([128, 128], f32, tag="psagg")
                n_groups = sum(k for _, _, k in gtiles_lo)
                n_groups += sum(k for _, _, k in gtiles_hi)
                k_idx = 0
                for (gt, c0, k), s_all in (
                    [(t, s_lo) for t in gtiles_lo]
                    + [(t, s_hi) for t in gtiles_hi]
                ):
                    for j in range(k):
                        nc.tensor.matmul(
                            psum[:], gt[:, j, :], s_all[:, c0 + j, :],
                            start=(k_idx == 0), stop=(k_idx == n_groups - 1),
                        )
                        k_idx += 1
                # aggT window -> SBUF (cast to gather dtype)
                nc.scalar.copy(agg_all[:, wg * WIN : (wg + 1) * WIN], psum[:])

            # out.T = W.T @ aggT + b, in 512-column chunks
            CH = 512
            for t0 in range(0, npad, CH):
                n = min(CH, npad - t0)
                po = ps_out.tile([128, CH], f32, tag="psout")
                nc.tensor.matmul(
                    po[:, :n], wmat[:], agg_all[:, t0 : t0 + n],
                    start=True, stop=True,
                )
                ob = opool.tile([128, CH], f32, tag="outsb")
                nc.scalar.add(ob[:, :n], po[:, :n], bcol[:])
                nc.sync.dma_start(out_t[:, t0 : t0 + n], ob[:, :n])

    nc.compile()
    return nc


def _make_in_maps(H, edge_src, edge_dst, edge_weight, W, b, per_core, nwin,
                  g_lo, g_hi, ch_lo, ch_hi):
    np_g = np.float16 if GDTYPE == "f16" else np.float32
    h_src = np.ascontiguousarray(H.astype(np_g))
    iota = np.tile(np.arange(128, dtype=np_g), (128, 1))
    wmat = np.ascontiguousarray(W.astype(np_g))
    bcol = np.ascontiguousarray(b.astype(np.float32).reshape(D, 1))
    in_maps = []
    for wins in per_core:
        idx_lo, drel_lo, wgt_lo = _device_arrays(wins, nwin, g_lo, 0, ch_lo, np_g)
        idx_hi, drel_hi, wgt_hi = _device_arrays(wins, nwin, g_hi, 1, ch_hi, np_g)
        in_maps.append(
            {
                "h_src": h_src,
                "idx_lo": idx_lo, "idx_hi": idx_hi,
                "drel_lo": drel_lo, "wgt_lo": wgt_lo,
                "drel_hi": drel_hi, "wgt_hi": wgt_hi,
                "iota": iota, "wmat": wmat, "bcol": bcol,
            }
        )
    return in_maps


def kernel(H, edge_src, edge_dst, edge_weight, W, b):
    global LAST_EXEC_NS
    from concourse import bass_utils

    H = np.asarray(H, dtype=np.float32)
    edge_src = np.asarray(edge_src, dtype=np.int32)
    edge_dst = np.asarray(edge_dst, dtype=np.int32)
    edge_weight = np.asarray(edge_weight, dtype=np.float32)
    W = np.asarray(W, dtype=np.float32)
    b = np.asarray(b, dtype=np.float32)

    per_core, nwin, g_lo, g_hi, g_all = _prep(H, edge_src, edge_dst, edge_weight)
    mode = os.environ.get("GCN_GATHER", "batched")
    if mode == "batched":
        batch = int(os.environ.get("GCN_BATCH", "4"))
        max_ke = int(os.environ.get("GCN_MAXKE", "63"))
        scratch = int(os.environ.get("GCN_SCRATCH", "32768"))
        gw_lo = []
        gw_hi = []
        for wi in range(nwin):
            m_lo = max(len(wins[wi][0][0]) for wins in per_core)
            m_hi = max(len(wins[wi][1][0]) for wins in per_core)
            gw_lo.append(min(g_lo, max(1, _ceil_div(m_lo, 128))))
            gw_hi.append(min(g_hi, max(1, _ceil_div(m_hi, 128))))
        hi_ind = os.environ.get("GCN_HI_IND", "0") == "1"
        nc, batches, lo_tot, hi_tot = _build_program_batched(
            nwin, g_lo, g_hi, gw_lo, gw_hi, N_NODES,
            batch=batch, max_ke=max_ke, scratch=scratch, hi_indirect=hi_ind,
        )
        np_g = np.float16 if GDTYPE == "f16" else np.float32
        h_src = np.ascontiguousarray(H.astype(np_g))
        iota = np.tile(np.arange(128, dtype=np_g), (128, 1))
        wmat = np.ascontiguousarray(W.astype(np_g))
        bcol = np.ascontiguousarray(b.astype(np.float32).reshape(D, 1))
        in_maps = []
        for wins in per_core:
            idx_lo, drel_lo, wgt_lo = _device_arrays_batched(
                wins, nwin, g_lo, 0, gw_lo, batches, np_g)
            idx_hi, drel_hi, wgt_hi = _device_arrays_batched(
                wins, nwin, g_hi, 1, gw_hi, batches, np_g,
                ind_offset=SPLIT if hi_ind else None)
            in_maps.append(
                {
                    "h_src": h_src,
                    "idx_lo": idx_lo, "idx_hi": idx_hi,
                    "drel_lo": drel_lo, "wgt_lo": wgt_lo,
                    "drel_hi": drel_hi, "wgt_hi": wgt_hi,
                    "iota": iota, "wmat": wmat, "bcol": bcol,
                }
            )
    elif mode == "indirect":
        maxg = int(os.environ.get("GCN_MAXG", "8"))
        ch = _chunks(g_all, maxg)
        nc = _build_program_ind(nwin, g_all, ch, N_NODES)
        np_g = np.float16 if GDTYPE == "f16" else np.float32
        h_src = np.ascontiguousarray(H.astype(np_g))
        iota = np.tile(np.arange(128, dtype=np_g), (128, 1))
        wmat = np.ascontiguousarray(W.astype(np_g))
        bcol = np.ascontiguousarray(b.astype(np.float32).reshape(D, 1))
        in_maps = []
        for wins in per_core:
            idx_all, drel, wgt = _device_arrays_ind(wins, nwin, g_all, np_g)
            in_maps.append({
                "h_src": h_src, "idx_all": idx_all, "drel": drel, "wgt": wgt,
                "iota": iota, "wmat": wmat, "bcol": bcol,
            })
    else:
        maxg = int(os.environ.get("GCN_MAXG", "8"))
        # round group counts up so every chunk has equal size: a gather tile
        # slot must always be written over its FULL extent when fully valid,
        # otherwise a smaller earlier write leaves never-written (non-finite)
        # columns that a later partially-valid call exposes to the matmul.
        g_lo = _ceil_div(g_lo, maxg) * maxg if g_lo > maxg else g_lo
        g_hi = _ceil_div(g_hi, maxg) * maxg if g_hi > maxg else g_hi
        ch_lo = _chunks(g_lo, maxg)
        ch_hi = _chunks(g_hi, maxg)
        assert len({k for _, k in ch_lo}) == 1 and len({k for _, k in ch_hi}) == 1
        # per-window effective group counts (shared across cores): only
        # gather/matmul groups that contain at least one real edge on the
        # max-count core; the rest are pure padding.
        trim = os.environ.get("GCN_TRIM", "1") == "1"
        gw_lo = []
        gw_hi = []
        for wi in range(nwin):
            m_lo = max(len(wins[wi][0][0]) for wins in per_core)
            m_hi = max(len(wins[wi][1][0]) for wins in per_core)
            gw_lo.append(min(g_lo, max(1, _ceil_div(m_lo, 128))) if trim else g_lo)
            gw_hi.append(min(g_hi, max(1, _ceil_div(m_hi, 128))) if trim else g_hi)
        nc = _build_program(nwin, g_lo, g_hi, ch_lo, ch_hi, N_NODES,
                            gw_lo=gw_lo, gw_hi=gw_hi)
        in_maps = _make_in_maps(
            H, edge_src, edge_dst, edge_weight, W, b, per_core, nwin, g_lo,
            g_hi, ch_lo, ch_hi,
        )

    if os.environ.get("GCN_SIM", "0") == "1":  # CoreSim path for testing
        from concourse.bass_interp import CoreSim

        out = np.empty((N_NODES, D), np.float32)
        for c in range(N_CORES):
            sim = CoreSim(nc)
            for k2, v2 in in_maps[c].items():
                sim.tensor(k2)[:] = v2
            sim.simulate()
            out[c * NPW : (c + 1) * NPW, :] = np.array(
                sim.tensor("outT")).T[:NPW]
        return out

    trace = os.environ.get("GCN_TRACE", "0") == "1"
    kw = {}
    if trace:
        import shutil
        td = "/tmp/gcn_ntff"
        shutil.rmtree(td, ignore_errors=True)
        os.makedirs(td, exist_ok=True)
        kw["tmpdir"] = td
    # a previously crashed NEFF can leave the exec unit transiently
    # unrecoverable; recovery has been observed to take up to a few minutes,
    # so retry with escalating backoff
    import time as _time
    last_err = None
    for backoff in (15, 45, 90, 0):
        try:
            res = bass_utils.run_bass_kernel_spmd(
                nc, in_maps, core_ids=list(range(N_CORES)), trace=trace, **kw
            )
            break
        except Exception as e:
            last_err = e
            if backoff:
                _time.sleep(backoff)
    else:
        raise last_err
    LAST_EXEC_NS = res.exec_time_ns
    global LAST_RESULTS
    LAST_RESULTS = res

    out = np.empty((N_NODES, D), np.float32)
    for c in range(N_CORES):
        outT = res.results[c]["outT"]
        out[c * NPW : (c + 1) * NPW, :] = outT.T[:NPW]
    return out



# revision 20
# speedup vs baseline: 2.7793x; 1.1546x over previous
"""GCN layer (gather -> weighted scatter-sum -> dense transform) on 8 trn2 cores.

Default path (GCN_GATHER=batched): same algorithm as the per-window path
below, but gather calls are batched — one dma_gather per (4-window batch,
stream) with single_packet=False (multi-packet rings; calls may exceed the
65-descriptor single-packet limit) and idx columns packed dense (per-window
trimmed group counts). Performance notes (HW-measured):
  - SWDGE dma_gather costs ~5.5-7.9 ns/idx, engine-serial on the Q7 pair;
    with per-desc packets (single_packet=False) the SDMA drain of 256B
    descriptors (~125ns/desc/engine) binds at ~7.8ns/idx. All SWDGE
    configurations converge to ~1.66-1.71ms for the ~212k idxs/core.
  - gpsimd.indirect_dma_start is NOT usable: on HW it lands rows on
    partition 0 only (CoreSim models it differently) and signals no DMA
    completion semaphores.
  - SBUF-source dma_gather (transpose=True) crashes the exec unit
    (NRT_EXEC_UNIT_UNRECOVERABLE).

Strategy (1-D row partitioning of destination nodes):
  - Core c owns destination nodes [c*NPW, (c+1)*NPW). edge_dst is sorted, so
    each core's edges are a contiguous slice of the edge list.
  - Within a core, dst nodes are processed in windows of 128 (the PSUM
    partition size). Every window's edges are padded to a fixed number of
    128-edge groups so all 8 cores run the same program.
  - Per 128-edge group:
      * dma_gather pulls the 128 source rows H[src] (fp16) from HBM into an
        SBUF tile G [128 edges x 128 feat] (edge e=j*128+p lands on
        partition p, slot j).
      * DVE builds S [128 edges x 128 nodes] = (iota == dstrel) * w with one
        fused tensor_scalar op.
      * TensorE accumulates aggT[feat, node] += G.T @ S in PSUM.
  - dma_gather indices are int16 (< 32768), so edges are split into a "lo"
    stream (src < 32768, gathered from H[:32768]) and a "hi" stream
    (src >= 32768, gathered from H[32768:]); both accumulate into the same
    PSUM window.
  - Final transform: out.T = W.T @ aggT (+ b) with W stationary, computed in
    512-column chunks; bias is added during the PSUM->SBUF copy (per-partition
    ACT bias, since the output is transposed: partitions = out features).
    The kernel writes out.T [128, NWIN*128] per core; the host transposes and
    concatenates.
"""

import os
import numpy as np

N_CORES = 8
N_NODES = 50000
D = 128
NPW = N_NODES // N_CORES  # 6250 dst nodes per core
WIN = 128
SPLIT = 32768  # int16-addressable row limit for dma_gather

# gather dtype: "f16" (half gather traffic, rel err ~3e-4) or "f32" (exact)
GDTYPE = os.environ.get("GCN_GDTYPE", "f16")

LAST_EXEC_NS = None  # set when GCN_TRACE=1
LAST_RESULTS = None


def _ceil_div(a, b):
    return -(-a // b)


def _prep(H, edge_src, edge_dst, edge_weight, n_cores=N_CORES):
    """Host-side sharding: per-core, per-window, per-stream edge lists with
    padding to common sizes. Returns per-core arrays + common geometry."""
    nwin = _ceil_div(NPW, WIN)
    # per (core, window, stream) edge index lists
    per_core = []
    max_lo = 0
    max_hi = 0
    max_all = 0
    for c in range(n_cores):
        n0, n1 = c * NPW, (c + 1) * NPW
        e0, e1 = np.searchsorted(edge_dst, [n0, n1])
        d = edge_dst[e0:e1] - n0
        s = edge_src[e0:e1]
        w = edge_weight[e0:e1]
        wins = []
        for wi in range(nwin):
            i0, i1 = np.searchsorted(d, [wi * WIN, wi * WIN + WIN])
            sw, dw, ww = s[i0:i1], d[i0:i1] - wi * WIN, w[i0:i1]
            lo = sw < SPLIT
            wins.append(
                (
                    (sw[lo], dw[lo], ww[lo]),
                    (sw[~lo] - SPLIT, dw[~lo], ww[~lo]),
                    (sw, dw, ww),
                )
            )
            max_lo = max(max_lo, int(lo.sum()))
            max_hi = max(max_hi, int((~lo).sum()))
            max_all = max(max_all, int(i1 - i0))
        per_core.append(wins)
    g_lo = max(1, _ceil_div(max_lo, 128))
    g_hi = max(1, _ceil_div(max_hi, 128))
    g_all = max(1, _ceil_div(max_all, 128))
    return per_core, nwin, g_lo, g_hi, g_all


def _chunks(g, maxg):
    """Split g groups into near-even chunks of <= maxg groups."""
    n = _ceil_div(g, maxg)
    base, rem = divmod(g, n)
    out = []
    c0 = 0
    for i in range(n):
        k = base + (1 if i < rem else 0)
        out.append((c0, k))
        c0 += k
    return out


def _device_arrays(wins, nwin, g, stream, chunks, np_meta_dtype, regs=None):
    """Build idx (wrapped-16 per gather call; call = (window, chunk)) +
    dstrel/weight arrays for one core and one stream ('lo'=0, 'hi'=1).

    regs is unused (kept for signature compat)."""
    ne = g * 128
    idx = np.zeros((nwin, ne), np.int16)
    drel = np.zeros((nwin, ne), np_meta_dtype)
    wgt = np.zeros((nwin, ne), np_meta_dtype)
    for wi in range(nwin):
        sw, dw, ww = wins[wi][stream]
        n = len(sw)
        idx[wi, :n] = sw.astype(np.int16)
        drel[wi, :n] = dw.astype(np_meta_dtype)
        wgt[wi, :n] = ww.astype(np_meta_dtype)
    parts = []
    for wi in range(nwin):
        for (c0, k) in chunks:
            flat = idx[wi, c0 * 128 : (c0 + k) * 128]
            parts.append(flat.reshape(-1, 16).T)  # [16, k*8]
    idx_dev = np.tile(np.concatenate(parts, axis=1), (8, 1))  # [128, nwin*g*8]
    # meta: [p, w*g + j] = value of edge e=j*128+p in window w
    drel_dev = np.ascontiguousarray(
        drel.reshape(nwin, g, 128).transpose(2, 0, 1).reshape(128, nwin * g)
    )
    wgt_dev = np.ascontiguousarray(
        wgt.reshape(nwin, g, 128).transpose(2, 0, 1).reshape(128, nwin * g)
    )
    return idx_dev, drel_dev, wgt_dev


def _device_arrays_ind(wins, nwin, g, np_meta_dtype):
    """idx (int32, natural [p, w*g+j] layout) + dstrel/weight arrays for the
    combined stream (indirect_dma_start variant)."""
    ne = g * 128
    idx = np.zeros((nwin, ne), np.int32)
    drel = np.zeros((nwin, ne), np_meta_dtype)
    wgt = np.zeros((nwin, ne), np_meta_dtype)
    for wi in range(nwin):
        sw, dw, ww = wins[wi][2]
        n = len(sw)
        idx[wi, :n] = sw
        drel[wi, :n] = dw.astype(np_meta_dtype)
        wgt[wi, :n] = ww.astype(np_meta_dtype)

    def dev(a):
        return np.ascontiguousarray(
            a.reshape(nwin, g, 128).transpose(2, 0, 1).reshape(128, nwin * g)
        )

    return dev(idx), dev(drel), dev(wgt)


def _build_program_ind(nwin, g_all, ch, n_src_rows, n_cores=N_CORES):
    """Indirect_dma_start variant: int32 indices, no lo/hi split."""
    from contextlib import ExitStack

    import concourse.bass as bass
    import concourse.tile as tile
    from concourse import bacc, mybir

    f32 = mybir.dt.float32
    gdt = mybir.dt.float16 if GDTYPE == "f16" else mybir.dt.float32
    i32 = mybir.dt.int32

    nc = bacc.Bacc(
        "TRN2", target_bir_lowering=False, debug=False, num_devices=n_cores,
    )

    npad = nwin * WIN
    h_t = nc.dram_tensor("h_src", [n_src_rows, D], gdt, kind="ExternalInput")
    idx_t = nc.dram_tensor("idx_all", [128, nwin * g_all], i32, kind="ExternalInput")
    drel_t = nc.dram_tensor("drel", [128, nwin * g_all], gdt, kind="ExternalInput")
    wgt_t = nc.dram_tensor("wgt", [128, nwin * g_all], gdt, kind="ExternalInput")
    iota_t = nc.dram_tensor("iota", [128, 128], gdt, kind="ExternalInput")
    w_t = nc.dram_tensor("wmat", [D, D], gdt, kind="ExternalInput")
    b_t = nc.dram_tensor("bcol", [D, 1], f32, kind="ExternalInput")
    out_t = nc.dram_tensor("outT", [D, npad], f32, kind="ExternalOutput")

    with tile.TileContext(nc) as tc:
        with ExitStack() as ctx:
            const = ctx.enter_context(tc.tile_pool(name="const", bufs=1))
            gpool = ctx.enter_context(tc.tile_pool(name="gather", bufs=6))
            spool = ctx.enter_context(tc.tile_pool(name="sel", bufs=3))
            opool = ctx.enter_context(tc.tile_pool(name="outsb", bufs=2))
            ps_agg = ctx.enter_context(tc.tile_pool(name="ps_agg", bufs=2, space="PSUM"))
            ps_out = ctx.enter_context(tc.tile_pool(name="ps_out", bufs=2, space="PSUM"))

            idx = const.tile(list(idx_t.shape), i32)
            drel = const.tile(list(drel_t.shape), gdt)
            wgt = const.tile(list(wgt_t.shape), gdt)
            iota = const.tile([128, 128], gdt)
            wmat = const.tile([D, D], gdt)
            bcol = const.tile([D, 1], f32)
            agg_all = const.tile([128, npad], gdt, tag="agg_all")

            for sb, dr in ((idx, idx_t), (drel, drel_t), (wgt, wgt_t),
                           (iota, iota_t), (wmat, w_t), (bcol, b_t)):
                nc.sync.dma_start(sb[:], dr[:])

            for wg in range(nwin):
                gtiles = []
                for (c0, k) in ch:
                    gt = gpool.tile([128, ch[0][1], 128], gdt, tag="g")
                    nc.gpsimd.indirect_dma_start(
                        out=gt[:, :k, :],
                        out_offset=None,
                        in_=h_t[:],
                        in_offset=bass.IndirectOffsetOnAxis(
                            ap=idx[:, wg * g_all + c0 : wg * g_all + c0 + k],
                            axis=0,
                        ),
                    )
                    gtiles.append((gt, c0, k))

                sh = (128, g_all, 128)
                c0m = wg * g_all
                s = spool.tile([128, g_all, 128], gdt, tag="sel")
                nc.vector.tensor_tensor(
                    s[:], iota[:, None, :].broadcast_to(sh),
                    drel[:, c0m : c0m + g_all, None].broadcast_to(sh),
                    mybir.AluOpType.is_equal,
                )
                nc.vector.tensor_tensor(
                    s[:], s[:], wgt[:, c0m : c0m + g_all, None].broadcast_to(sh),
                    mybir.AluOpType.mult,
                )

                psum = ps_agg.tile([128, 128], f32, tag="psagg")
                k_idx = 0
                for (gt, c0, k) in gtiles:
                    for j in range(k):
                        nc.tensor.matmul(
                            psum[:], gt[:, j, :], s[:, c0 + j, :],
                            start=(k_idx == 0), stop=(k_idx == g_all - 1),
                        )
                        k_idx += 1
                nc.scalar.copy(agg_all[:, wg * WIN : (wg + 1) * WIN], psum[:])

            CH = 512
            for t0 in range(0, npad, CH):
                n = min(CH, npad - t0)
                po = ps_out.tile([128, CH], f32, tag="psout")
                nc.tensor.matmul(
                    po[:, :n], wmat[:], agg_all[:, t0 : t0 + n],
                    start=True, stop=True,
                )
                ob = opool.tile([128, CH], f32, tag="outsb")
                nc.scalar.add(ob[:, :n], po[:, :n], bcol[:])
                nc.sync.dma_start(out_t[:, t0 : t0 + n], ob[:, :n])

    nc.compile()
    return nc


def _build_program_batched(nwin, g_lo, g_hi, gw_lo, gw_hi, n_src_rows,
                           batch=2, max_ke=63, n_cores=N_CORES,
                           scratch=32768, hi_indirect=False):
    """Batched-call variant: one dma_gather call per (batch of windows,
    stream), idx columns packed dense (only the first gw[w] groups of each
    window are gathered). single_packet=False so calls may exceed 65 ring
    descriptors; scratch sized so ring (scratch/64 descs) >= 8*max_ke+1."""
    from contextlib import ExitStack

    import concourse.bass as bass
    import concourse.tile as tile
    from concourse import bacc, mybir

    f32 = mybir.dt.float32
    gdt = mybir.dt.float16 if GDTYPE == "f16" else mybir.dt.float32
    i16 = mybir.dt.int16

    nq = int(os.environ.get("GCN_NQ", "4"))
    nc = bacc.Bacc(
        "TRN2", target_bir_lowering=False, debug=False, num_devices=n_cores,
        dynamic_dma_scratch_size=scratch, num_swdge_queues=nq,
    )

    npad = nwin * WIN
    n_lo_rows = min(SPLIT, n_src_rows)
    n_hi_rows = n_src_rows - n_lo_rows

    batches = [list(range(b, min(b + batch, nwin))) for b in range(0, nwin, batch)]
    # per-batch dense group counts and per-window offsets
    lo_off = {}
    hi_off = {}
    lo_tot = []
    hi_tot = []
    for bi, ws in enumerate(batches):
        o = 0
        for w in ws:
            lo_off[w] = o
            o += gw_lo[w]
        lo_tot.append(o)
        o = 0
        for w in ws:
            hi_off[w] = o
            o += gw_hi[w]
        hi_tot.append(o)
    glo_max = max(lo_tot)
    ghi_max = max(hi_tot)
    ncol_lo = sum(lo_tot)
    ncol_hi = sum(hi_tot)

    h_t = nc.dram_tensor("h_src", [n_src_rows, D], gdt, kind="ExternalInput")
    idx_lo_t = nc.dram_tensor("idx_lo", [128, ncol_lo * 8], i16, kind="ExternalInput")
    if hi_indirect:
        idx_hi_t = nc.dram_tensor(
            "idx_hi", [128, ncol_hi], mybir.dt.int32, kind="ExternalInput")
    else:
        idx_hi_t = nc.dram_tensor(
            "idx_hi", [128, ncol_hi * 8], i16, kind="ExternalInput")
    drel_lo_t = nc.dram_tensor("drel_lo", [128, nwin * g_lo], gdt, kind="ExternalInput")
    wgt_lo_t = nc.dram_tensor("wgt_lo", [128, nwin * g_lo], gdt, kind="ExternalInput")
    drel_hi_t = nc.dram_tensor("drel_hi", [128, nwin * g_hi], gdt, kind="ExternalInput")
    wgt_hi_t = nc.dram_tensor("wgt_hi", [128, nwin * g_hi], gdt, kind="ExternalInput")
    iota_t = nc.dram_tensor("iota", [128, 128], gdt, kind="ExternalInput")
    w_t = nc.dram_tensor("wmat", [D, D], gdt, kind="ExternalInput")
    b_t = nc.dram_tensor("bcol", [D, 1], f32, kind="ExternalInput")
    out_t = nc.dram_tensor("outT", [D, npad], f32, kind="ExternalOutput")

    with tile.TileContext(nc) as tc:
        with ExitStack() as ctx:
            const = ctx.enter_context(tc.tile_pool(name="const", bufs=1))
            gpool = ctx.enter_context(tc.tile_pool(name="gather", bufs=2))
            spool = ctx.enter_context(tc.tile_pool(name="sel", bufs=3))
            opool = ctx.enter_context(tc.tile_pool(name="outsb", bufs=2))
            ps_agg = ctx.enter_context(tc.tile_pool(name="ps_agg", bufs=2, space="PSUM"))
            ps_out = ctx.enter_context(tc.tile_pool(name="ps_out", bufs=2, space="PSUM"))

            idx_lo = const.tile(list(idx_lo_t.shape), i16)
            idx_hi = const.tile(
                list(idx_hi_t.shape),
                mybir.dt.int32 if hi_indirect else i16)
            drel_lo = const.tile(list(drel_lo_t.shape), gdt)
            wgt_lo = const.tile(list(wgt_lo_t.shape), gdt)
            drel_hi = const.tile(list(drel_hi_t.shape), gdt)
            wgt_hi = const.tile(list(wgt_hi_t.shape), gdt)
            iota = const.tile([128, 128], gdt)
            wmat = const.tile([D, D], gdt)
            bcol = const.tile([D, 1], f32)
            agg_all = const.tile([128, npad], gdt, tag="agg_all")

            for sb, dr in (
                (idx_lo, idx_lo_t), (idx_hi, idx_hi_t),
                (drel_lo, drel_lo_t), (wgt_lo, wgt_lo_t),
                (drel_hi, drel_hi_t), (wgt_hi, wgt_hi_t),
                (iota, iota_t), (wmat, w_t), (bcol, b_t),
            ):
                nc.sync.dma_start(sb[:], dr[:])

            h_lo = h_t[0:n_lo_rows, :]
            h_hi = h_t[n_lo_rows:n_src_rows, :] if n_hi_rows > 0 else None

            col_lo = 0
            col_hi = 0
            qn = [0]  # round-robin SWDGE queue counter (one ring set per Q7 pair)

            def next_q():
                q = qn[0] % nq
                qn[0] += 1
                return q

            for bi, ws in enumerate(batches):
                # one gather call per stream per batch (split at max_ke)
                bsp = os.environ.get("GCN_BSP", "0") == "1"
                glo = gpool.tile([128, glo_max, 128], gdt, tag="glo")
                c0 = 0
                while c0 < lo_tot[bi]:
                    ke = min(max_ke, lo_tot[bi] - c0)
                    nc.gpsimd.dma_gather(
                        glo[:, c0 : c0 + ke, :], h_lo,
                        idx_lo[:, (col_lo + c0) * 8 : (col_lo + c0 + ke) * 8],
                        num_idxs=ke * 128, num_idxs_reg=ke * 128, elem_size=D,
                        single_packet=bsp, queue_num=next_q(),
                    )
                    c0 += ke
                ghi = gpool.tile([128, ghi_max, 128], gdt, tag="ghi")
                marker = None
                if hi_indirect:
                    nc.gpsimd.indirect_dma_start(
                        out=ghi[:, : hi_tot[bi], :],
                        out_offset=None,
                        in_=h_t[:],
                        in_offset=bass.IndirectOffsetOnAxis(
                            ap=idx_hi[:, col_hi : col_hi + hi_tot[bi]],
                            axis=0,
                        ),
                    )
                    # FIFO-ordering completion marker: a tiny SWDGE gather on
                    # the same queue whose (working) DMA semaphore fires only
                    # after the ring has drained past the indirect's
                    # descriptors. Every hi matmul is made to depend on it via
                    # a bypass op over s_hi.
                    marker = gpool.tile([128, 1, 128], gdt, tag="mrk")
                    nc.gpsimd.dma_gather(
                        marker[:, :1, :], h_lo, idx_lo[:, 0:8],
                        num_idxs=128, num_idxs_reg=128, elem_size=D,
                        single_packet=False,
                    )
                else:
                    c0 = 0
                    while c0 < hi_tot[bi]:
                        ke = min(max_ke, hi_tot[bi] - c0)
                        nc.gpsimd.dma_gather(
                            ghi[:, c0 : c0 + ke, :], h_hi,
                            idx_hi[:, (col_hi + c0) * 8 : (col_hi + c0 + ke) * 8],
                            num_idxs=ke * 128, num_idxs_reg=ke * 128, elem_size=D,
                            single_packet=bsp, queue_num=next_q(),
                        )
                        c0 += ke
                col_lo += lo_tot[bi]
                col_hi += hi_tot[bi]

                for wg in ws:
                    def build_s(meta_d, meta_w, g, tag):
                        s = spool.tile([128, g, 128], gdt, tag=tag)
                        sh = (128, g, 128)
                        c0m = wg * g
                        nc.vector.tensor_tensor(
                            s[:], iota[:, None, :].broadcast_to(sh),
                            meta_d[:, c0m : c0m + g, None].broadcast_to(sh),
                            mybir.AluOpType.is_equal,
                        )
                        nc.vector.tensor_tensor(
                            s[:], s[:],
                            meta_w[:, c0m : c0m + g, None].broadcast_to(sh),
                            mybir.AluOpType.mult,
                        )
                        return s

                    s_lo = build_s(drel_lo, wgt_lo, g_lo, "slo")
                    s_hi = build_s(drel_hi, wgt_hi, g_hi, "shi") if h_hi is not None else None
                    if s_hi is not None and marker is not None:
                        # bypass: out = in0 (s_hi unchanged) but creates a dep
                        # on the marker tile for every s_hi subtile.
                        nc.vector.tensor_tensor(
                            s_hi[:], s_hi[:],
                            marker[:, 0, None, 0:1].broadcast_to((128, g_hi, 128)),
                            mybir.AluOpType.bypass,
                        )

                    psum = ps_agg.tile([128, 128], f32, tag="psagg")
                    n_groups = gw_lo[wg] + (gw_hi[wg] if s_hi is not None else 0)
                    k_idx = 0
                    for j in range(gw_lo[wg]):
                        nc.tensor.matmul(
                            psum[:], glo[:, lo_off[wg] + j, :], s_lo[:, j, :],
                            start=(k_idx == 0), stop=(k_idx == n_groups - 1),
                        )
                        k_idx += 1
                    if s_hi is not None:
                        for j in range(gw_hi[wg]):
                            nc.tensor.matmul(
                                psum[:], ghi[:, hi_off[wg] + j, :], s_hi[:, j, :],
                                start=(k_idx == 0), stop=(k_idx == n_groups - 1),
                            )
                            k_idx += 1
                    nc.scalar.copy(agg_all[:, wg * WIN : (wg + 1) * WIN], psum[:])

                    # incremental output transform: as soon as a 512-col chunk
                    # of agg_all is complete, run W.T @ chunk so the tail
                    # after the last gather is just one chunk.
                    CH = 512
                    done = (wg + 1) * WIN
                    t0 = (done // CH - 1) * CH
                    if t0 >= 0 and done % CH == 0:
                        n = min(CH, npad - t0)
                        po = ps_out.tile([128, CH], f32, tag="psout")
                        nc.tensor.matmul(
                            po[:, :n], wmat[:], agg_all[:, t0 : t0 + n],
                            start=True, stop=True,
                        )
                        ob = opool.tile([128, CH], f32, tag="outsb")
                        nc.scalar.add(ob[:, :n], po[:, :n], bcol[:])
                        nc.sync.dma_start(out_t[:, t0 : t0 + n], ob[:, :n])

            CH = 512
            for t0 in range((npad // CH) * CH - (CH if npad % CH == 0 else 0),
                            npad, CH):
                if t0 < 0:
                    continue
                n = min(CH, npad - t0)
                if n <= 0:
                    continue
                po = ps_out.tile([128, CH], f32, tag="psout")
                nc.tensor.matmul(
                    po[:, :n], wmat[:], agg_all[:, t0 : t0 + n],
                    start=True, stop=True,
                )
                ob = opool.tile([128, CH], f32, tag="outsb")
                nc.scalar.add(ob[:, :n], po[:, :n], bcol[:])
                nc.sync.dma_start(out_t[:, t0 : t0 + n], ob[:, :n])

    nc.compile()
    return nc, batches, lo_tot, hi_tot


def _device_arrays_batched(wins, nwin, g, stream, gw, batches, np_meta_dtype,
                           ind_offset=None):
    """Dense-packed idx array (wrapped-16 per batch-call column blocks) plus
    per-window padded drel/wgt arrays (same layout as _device_arrays).

    ind_offset: if not None, build int32 indirect-layout idxs ([128, ncol]
    natural [p, col] order, absolute row ids = stored + ind_offset)."""
    ne = g * 128
    idx = np.zeros((nwin, ne), np.int32 if ind_offset is not None else np.int16)
    drel = np.zeros((nwin, ne), np_meta_dtype)
    wgt = np.zeros((nwin, ne), np_meta_dtype)
    for wi in range(nwin):
        sw, dw, ww = wins[wi][stream]
        n = len(sw)
        if ind_offset is not None:
            idx[wi, :n] = sw.astype(np.int32) + ind_offset
        else:
            idx[wi, :n] = sw.astype(np.int16)
        drel[wi, :n] = dw.astype(np_meta_dtype)
        wgt[wi, :n] = ww.astype(np_meta_dtype)
    if ind_offset is not None:
        # [p, dense col] where col runs over (batch, window, group j<gw[w])
        parts = []
        for ws in batches:
            for wi in ws:
                parts.append(idx[wi, : gw[wi] * 128].reshape(gw[wi], 128).T)
        idx_dev = np.ascontiguousarray(np.concatenate(parts, axis=1))
    else:
        # wrapped in 16 partitions, tiled x8 (SWDGE layout)
        parts = []
        for ws in batches:
            for wi in ws:
                flat = idx[wi, : gw[wi] * 128]
                parts.append(flat.reshape(-1, 16).T)  # [16, gw*8]
        idx_dev = np.tile(np.concatenate(parts, axis=1), (8, 1))
    drel_dev = np.ascontiguousarray(
        drel.reshape(nwin, g, 128).transpose(2, 0, 1).reshape(128, nwin * g)
    )
    wgt_dev = np.ascontiguousarray(
        wgt.reshape(nwin, g, 128).transpose(2, 0, 1).reshape(128, nwin * g)
    )
    return idx_dev, drel_dev, wgt_dev


def _build_program(nwin, g_lo, g_hi, ch_lo, ch_hi, n_src_rows, n_cores=N_CORES,
                   gw_lo=None, gw_hi=None):
    """Trace the (single, SPMD-shared) Bass program."""
    from contextlib import ExitStack

    import concourse.bass as bass
    import concourse.tile as tile
    from concourse import bacc, mybir

    f32 = mybir.dt.float32
    gdt = mybir.dt.float16 if GDTYPE == "f16" else mybir.dt.float32
    i16 = mybir.dt.int16

    nc = bacc.Bacc(
        "TRN2",
        target_bir_lowering=False,
        debug=False,
        num_devices=n_cores,
    )

    npad = nwin * WIN
    n_lo_rows = min(SPLIT, n_src_rows)
    n_hi_rows = n_src_rows - n_lo_rows

    h_t = nc.dram_tensor("h_src", [n_src_rows, D], gdt, kind="ExternalInput")
    idx_lo_t = nc.dram_tensor(
        "idx_lo", [128, nwin * g_lo * 8], i16, kind="ExternalInput",
    )
    idx_hi_t = nc.dram_tensor(
        "idx_hi", [128, nwin * g_hi * 8], i16, kind="ExternalInput",
    )
    drel_lo_t = nc.dram_tensor("drel_lo", [128, nwin * g_lo], gdt, kind="ExternalInput")
    wgt_lo_t = nc.dram_tensor("wgt_lo", [128, nwin * g_lo], gdt, kind="ExternalInput")
    drel_hi_t = nc.dram_tensor("drel_hi", [128, nwin * g_hi], gdt, kind="ExternalInput")
    wgt_hi_t = nc.dram_tensor("wgt_hi", [128, nwin * g_hi], gdt, kind="ExternalInput")
    iota_t = nc.dram_tensor("iota", [128, 128], gdt, kind="ExternalInput")
    w_t = nc.dram_tensor("wmat", [D, D], gdt, kind="ExternalInput")
    b_t = nc.dram_tensor("bcol", [D, 1], f32, kind="ExternalInput")
    out_t = nc.dram_tensor("outT", [D, npad], f32, kind="ExternalOutput")

    with tile.TileContext(nc) as tc:
        with ExitStack() as ctx:
            const = ctx.enter_context(tc.tile_pool(name="const", bufs=1))
            gpool = ctx.enter_context(tc.tile_pool(name="gather", bufs=6))
            spool = ctx.enter_context(tc.tile_pool(name="sel", bufs=3))
            opool = ctx.enter_context(tc.tile_pool(name="outsb", bufs=2))
            ps_agg = ctx.enter_context(
                tc.tile_pool(name="ps_agg", bufs=2, space="PSUM")
            )
            ps_out = ctx.enter_context(
                tc.tile_pool(name="ps_out", bufs=2, space="PSUM")
            )

            # resident constants / metadata
            idx_lo = const.tile(list(idx_lo_t.shape), i16)
            idx_hi = const.tile(list(idx_hi_t.shape), i16)
            drel_lo = const.tile(list(drel_lo_t.shape), gdt)
            wgt_lo = const.tile(list(wgt_lo_t.shape), gdt)
            drel_hi = const.tile(list(drel_hi_t.shape), gdt)
            wgt_hi = const.tile(list(wgt_hi_t.shape), gdt)
            iota = const.tile([128, 128], gdt)
            wmat = const.tile([D, D], gdt)
            bcol = const.tile([D, 1], f32)
            agg_all = const.tile([128, npad], gdt, tag="agg_all")

            for sb, dr in (
                (idx_lo, idx_lo_t), (idx_hi, idx_hi_t),
                (drel_lo, drel_lo_t), (wgt_lo, wgt_lo_t),
                (drel_hi, drel_hi_t), (wgt_hi, wgt_hi_t),
                (iota, iota_t), (wmat, w_t), (bcol, b_t),
            ):
                nc.sync.dma_start(sb[:], dr[:])

            h_lo = h_t[0:n_lo_rows, :]
            h_hi = h_t[n_lo_rows:n_src_rows, :] if n_hi_rows > 0 else None
            use_hi = h_hi is not None

            for wg in range(nwin):
                # gather this window's edges: one SWDGE call per chunk.
                # A call of k*128 idxs needs 8k+1 SWDGE ring entries; calls
                # with 97 entries (k=12) crash the exec unit on HW, k<=8 is
                # proven safe.
                # effective groups this window (shared across cores): groups
                # beyond the max valid count are pure padding -> not gathered,
                # not matmul'd. Every issued call is fully valid, so no tile
                # region is ever read without having been written.
                gwl = gw_lo[wg] if gw_lo else g_lo
                gwh = gw_hi[wg] if gw_hi else g_hi
                sp = os.environ.get("GCN_SP", "1") == "1"
                gtiles_lo = []
                for (c0, k) in ch_lo:
                    ke = min(max(gwl - c0, 0), k)
                    if ke == 0:
                        continue
                    gt = gpool.tile([128, ch_lo[0][1], 128], gdt, tag="glo")
                    col = (wg * g_lo + c0) * 8
                    nc.gpsimd.dma_gather(
                        gt[:, :ke, :], h_lo, idx_lo[:, col : col + ke * 8],
                        num_idxs=ke * 128, num_idxs_reg=ke * 128, elem_size=D,
                        single_packet=sp,
                    )
                    gtiles_lo.append((gt, c0, ke))
                gtiles_hi = []
                if use_hi:
                    for (c0, k) in ch_hi:
                        ke = min(max(gwh - c0, 0), k)
                        if ke == 0:
                            continue
                        gt = gpool.tile([128, ch_hi[0][1], 128], gdt, tag="ghi")
                        col = (wg * g_hi + c0) * 8
                        nc.gpsimd.dma_gather(
                            gt[:, :ke, :], h_hi, idx_hi[:, col : col + ke * 8],
                            num_idxs=ke * 128, num_idxs_reg=ke * 128, elem_size=D,
                            single_packet=sp,
                        )
                        gtiles_hi.append((gt, c0, ke))

                # S for the whole window in 2 DVE ops per stream:
                # S[p, j, n] = (n == drel[p, j]) * w[p, j] via step-0
                # broadcast APs on both operands.
                def build_s(meta_d, meta_w, g, tag):
                    s = spool.tile([128, g, 128], gdt, tag=tag)
                    sh = (128, g, 128)
                    c0m = wg * g
                    nc.vector.tensor_tensor(
                        s[:], iota[:, None, :].broadcast_to(sh),
                        meta_d[:, c0m : c0m + g, None].broadcast_to(sh),
                        mybir.AluOpType.is_equal,
                    )
                    nc.vector.tensor_tensor(
                        s[:], s[:],
                        meta_w[:, c0m : c0m + g, None].broadcast_to(sh),
                        mybir.AluOpType.mult,
                    )
                    return s

                s_lo = build_s(drel_lo, wgt_lo, g_lo, "slo")
                s_hi = build_s(drel_hi, wgt_hi, g_hi, "shi") if use_hi else None

                psum = ps_agg.tile([128, 128], f32, tag="psagg")
                n_groups = sum(k for _, _, k in gtiles_lo)
                n_groups += sum(k for _, _, k in gtiles_hi)
                k_idx = 0
                for (gt, c0, k), s_all in (
                    [(t, s_lo) for t in gtiles_lo]
                    + [(t, s_hi) for t in gtiles_hi]
                ):
                    for j in range(k):
                        nc.tensor.matmul(
                            psum[:], gt[:, j, :], s_all[:, c0 + j, :],
                            start=(k_idx == 0), stop=(k_idx == n_groups - 1),
                        )
                        k_idx += 1
                # aggT window -> SBUF (cast to gather dtype)
                nc.scalar.copy(agg_all[:, wg * WIN : (wg + 1) * WIN], psum[:])

            # out.T = W.T @ aggT + b, in 512-column chunks
            CH = 512
            for t0 in range(0, npad, CH):
                n = min(CH, npad - t0)
                po = ps_out.tile([128, CH], f32, tag="psout")
                nc.tensor.matmul(
                    po[:, :n], wmat[:], agg_all[:, t0 : t0 + n],
                    start=True, stop=True,
                )
                ob = opool.tile([128, CH], f32, tag="outsb")
                nc.scalar.add(ob[:, :n], po[:, :n], bcol[:])
                nc.sync.dma_start(out_t[:, t0 : t0 + n], ob[:, :n])

    nc.compile()
    return nc


def _make_in_maps(H, edge_src, edge_dst, edge_weight, W, b, per_core, nwin,
                  g_lo, g_hi, ch_lo, ch_hi):
    np_g = np.float16 if GDTYPE == "f16" else np.float32
    h_src = np.ascontiguousarray(H.astype(np_g))
    iota = np.tile(np.arange(128, dtype=np_g), (128, 1))
    wmat = np.ascontiguousarray(W.astype(np_g))
    bcol = np.ascontiguousarray(b.astype(np.float32).reshape(D, 1))
    in_maps = []
    for wins in per_core:
        idx_lo, drel_lo, wgt_lo = _device_arrays(wins, nwin, g_lo, 0, ch_lo, np_g)
        idx_hi, drel_hi, wgt_hi = _device_arrays(wins, nwin, g_hi, 1, ch_hi, np_g)
        in_maps.append(
            {
                "h_src": h_src,
                "idx_lo": idx_lo, "idx_hi": idx_hi,
                "drel_lo": drel_lo, "wgt_lo": wgt_lo,
                "drel_hi": drel_hi, "wgt_hi": wgt_hi,
                "iota": iota, "wmat": wmat, "bcol": bcol,
            }
        )
    return in_maps


def kernel(H, edge_src, edge_dst, edge_weight, W, b):
    global LAST_EXEC_NS
    from concourse import bass_utils

    H = np.asarray(H, dtype=np.float32)
    edge_src = np.asarray(edge_src, dtype=np.int32)
    edge_dst = np.asarray(edge_dst, dtype=np.int32)
    edge_weight = np.asarray(edge_weight, dtype=np.float32)
    W = np.asarray(W, dtype=np.float32)
    b = np.asarray(b, dtype=np.float32)

    per_core, nwin, g_lo, g_hi, g_all = _prep(H, edge_src, edge_dst, edge_weight)
    mode = os.environ.get("GCN_GATHER", "batched")
    if mode == "batched":
        batch = int(os.environ.get("GCN_BATCH", "4"))
        max_ke = int(os.environ.get("GCN_MAXKE", "44"))
        scratch = int(os.environ.get("GCN_SCRATCH", "32768"))
        gw_lo = []
        gw_hi = []
        for wi in range(nwin):
            m_lo = max(len(wins[wi][0][0]) for wins in per_core)
            m_hi = max(len(wins[wi][1][0]) for wins in per_core)
            gw_lo.append(min(g_lo, max(1, _ceil_div(m_lo, 128))))
            gw_hi.append(min(g_hi, max(1, _ceil_div(m_hi, 128))))
        hi_ind = os.environ.get("GCN_HI_IND", "0") == "1"
        nc, batches, lo_tot, hi_tot = _build_program_batched(
            nwin, g_lo, g_hi, gw_lo, gw_hi, N_NODES,
            batch=batch, max_ke=max_ke, scratch=scratch, hi_indirect=hi_ind,
        )
        np_g = np.float16 if GDTYPE == "f16" else np.float32
        h_src = np.ascontiguousarray(H.astype(np_g))
        iota = np.tile(np.arange(128, dtype=np_g), (128, 1))
        wmat = np.ascontiguousarray(W.astype(np_g))
        bcol = np.ascontiguousarray(b.astype(np.float32).reshape(D, 1))
        in_maps = []
        for wins in per_core:
            idx_lo, drel_lo, wgt_lo = _device_arrays_batched(
                wins, nwin, g_lo, 0, gw_lo, batches, np_g)
            idx_hi, drel_hi, wgt_hi = _device_arrays_batched(
                wins, nwin, g_hi, 1, gw_hi, batches, np_g,
                ind_offset=SPLIT if hi_ind else None)
            in_maps.append(
                {
                    "h_src": h_src,
                    "idx_lo": idx_lo, "idx_hi": idx_hi,
                    "drel_lo": drel_lo, "wgt_lo": wgt_lo,
                    "drel_hi": drel_hi, "wgt_hi": wgt_hi,
                    "iota": iota, "wmat": wmat, "bcol": bcol,
                }
            )
    elif mode == "indirect":
        maxg = int(os.environ.get("GCN_MAXG", "8"))
        ch = _chunks(g_all, maxg)
        nc = _build_program_ind(nwin, g_all, ch, N_NODES)
        np_g = np.float16 if GDTYPE == "f16" else np.float32
        h_src = np.ascontiguousarray(H.astype(np_g))
        iota = np.tile(np.arange(128, dtype=np_g), (128, 1))
        wmat = np.ascontiguousarray(W.astype(np_g))
        bcol = np.ascontiguousarray(b.astype(np.float32).reshape(D, 1))
        in_maps = []
        for wins in per_core:
            idx_all, drel, wgt = _device_arrays_ind(wins, nwin, g_all, np_g)
            in_maps.append({
                "h_src": h_src, "idx_all": idx_all, "drel": drel, "wgt": wgt,
                "iota": iota, "wmat": wmat, "bcol": bcol,
            })
    else:
        maxg = int(os.environ.get("GCN_MAXG", "8"))
        # round group counts up so every chunk has equal size: a gather tile
        # slot must always be written over its FULL extent when fully valid,
        # otherwise a smaller earlier write leaves never-written (non-finite)
        # columns that a later partially-valid call exposes to the matmul.
        g_lo = _ceil_div(g_lo, maxg) * maxg if g_lo > maxg else g_lo
        g_hi = _ceil_div(g_hi, maxg) * maxg if g_hi > maxg else g_hi
        ch_lo = _chunks(g_lo, maxg)
        ch_hi = _chunks(g_hi, maxg)
        assert len({k for _, k in ch_lo}) == 1 and len({k for _, k in ch_hi}) == 1
        # per-window effective group counts (shared across cores): only
        # gather/matmul groups that contain at least one real edge on the
        # max-count core; the rest are pure padding.
        trim = os.environ.get("GCN_TRIM", "1") == "1"
        gw_lo = []
        gw_hi = []
        for wi in range(nwin):
            m_lo = max(len(wins[wi][0][0]) for wins in per_core)
            m_hi = max(len(wins[wi][1][0]) for wins in per_core)
            gw_lo.append(min(g_lo, max(1, _ceil_div(m_lo, 128))) if trim else g_lo)
            gw_hi.append(min(g_hi, max(1, _ceil_div(m_hi, 128))) if trim else g_hi)
        nc = _build_program(nwin, g_lo, g_hi, ch_lo, ch_hi, N_NODES,
                            gw_lo=gw_lo, gw_hi=gw_hi)
        in_maps = _make_in_maps(
            H, edge_src, edge_dst, edge_weight, W, b, per_core, nwin, g_lo,
            g_hi, ch_lo, ch_hi,
        )

    if os.environ.get("GCN_SIM", "0") == "1":  # CoreSim path for testing
        from concourse.bass_interp import CoreSim

        out = np.empty((N_NODES, D), np.float32)
        for c in range(N_CORES):
            sim = CoreSim(nc)
            for k2, v2 in in_maps[c].items():
                sim.tensor(k2)[:] = v2
            sim.simulate()
            out[c * NPW : (c + 1) * NPW, :] = np.array(
                sim.tensor("outT")).T[:NPW]
        return out

    trace = os.environ.get("GCN_TRACE", "0") == "1"
    kw = {}
    if trace:
        import shutil
        td = "/tmp/gcn_ntff"
        shutil.rmtree(td, ignore_errors=True)
        os.makedirs(td, exist_ok=True)
        kw["tmpdir"] = td
    # a previously crashed NEFF can leave the exec unit transiently
    # unrecoverable; recovery has been observed to take up to a few minutes,
    # so retry with escalating backoff
    import time as _time
    last_err = None
    for backoff in (15, 45, 90, 0):
        try:
            res = bass_utils.run_bass_kernel_spmd(
                nc, in_maps, core_ids=list(range(N_CORES)), trace=trace, **kw
            )
            break
        except Exception as e:
            last_err = e
            if backoff:
                _time.sleep(backoff)
    else:
        raise last_err
    LAST_EXEC_NS = res.exec_time_ns
    global LAST_RESULTS
    LAST_RESULTS = res

    out = np.empty((N_NODES, D), np.float32)
    for c in range(N_CORES):
        outT = res.results[c]["outT"]
        out[c * NPW : (c + 1) * NPW, :] = outT.T[:NPW]
    return out

